# revision 3
# baseline (speedup 1.0000x reference)
"""Trainium2 Bass kernel for nn_MultiHeadAttention (head-axis softmax quirk).

v2 strategy (8 NeuronCores, 2 batch-groups x 4 cores):
  - NO pre-attention collectives. Softmax over HEADS is local per (q,k), so
    attention is sharded over the K/V length: each core projects its own
    512-row k-slice of K and V, and redundantly projects Q for the whole
    group's 2048 q rows (+41us PE, but saves ~240us of AllGather).
  - Per core: scores^T [k,q] tiles (128k x 512q x 16h), exp on ScalarE,
    head-sum via DVE tree-adds (bf16 4x mode), normalize split DVE/GpSimd,
    context accumulated across the 4 local k-chunks in PSUM chains (no
    SBUF accumulation traffic).
  - One bf16 ReduceScatter of ctx^T partials (4MB in -> 1MB out) hands each
    core the k-summed context for its own 512 q rows.
  - Output projection + residual + LayerNorm on own q rows.
  - All matmuls bf16 with fp32 PSUM accumulation.
"""

import numpy as np
import ml_dtypes

D = 1024
H = 16
DK = 64
P = 128
M = 512  # q rows per core (own slice) / k rows per core
SK = 2048
G = 4  # cores per batch group
GQ = 2048  # q rows per group
EC = D // P  # 8
HP = H // 2  # 8
KC = M // P  # 4 local k chunks
QB = GQ // M  # 4 q blocks (one per destination core)
LN_EPS = 1e-5

_CACHE = {}


def _build():
    import concourse.bass as bass
    import concourse.mybir as mybir
    import concourse.tile as tile
    from concourse import bacc

    f32 = mybir.dt.float32
    bf16 = mybir.dt.bfloat16
    AF = mybir.ActivationFunctionType
    OP = mybir.AluOpType
    AX = mybir.AxisListType

    nc = bacc.Bacc("TRN2", target_bir_lowering=False, debug=False, num_devices=8)

    xqT = nc.dram_tensor("xqT", [D, GQ], bf16, kind="ExternalInput").ap()
    xkT = nc.dram_tensor("xkT", [D, M], bf16, kind="ExternalInput").ap()
    xvT = nc.dram_tensor("xvT", [D, M], bf16, kind="ExternalInput").ap()
    xres = nc.dram_tensor("xres", [M, D], f32, kind="ExternalInput").ap()
    wq = nc.dram_tensor("wq", [D, D], bf16, kind="ExternalInput").ap()
    wk = nc.dram_tensor("wk", [D, D], bf16, kind="ExternalInput").ap()
    wv = nc.dram_tensor("wv", [D, D], bf16, kind="ExternalInput").ap()
    wo = nc.dram_tensor("wo", [D, D], bf16, kind="ExternalInput").ap()
    gam = nc.dram_tensor("gam", [P, D], f32, kind="ExternalInput").ap()
    bet = nc.dram_tensor("bet", [P, D], f32, kind="ExternalInput").ap()
    out = nc.dram_tensor("out", [M, D], f32, kind="ExternalOutput").ap()

    rg = [[0, 1, 2, 3], [4, 5, 6, 7]]

    from contextlib import ExitStack

    with tile.TileContext(nc) as tc:
        with ExitStack() as ctx:
            const = ctx.enter_context(tc.tile_pool(name="const", bufs=1))
            smal = ctx.enter_context(tc.tile_pool(name="smal", bufs=2))
            dram = ctx.enter_context(tc.tile_pool(name="dram", bufs=1, space="DRAM"))

            # persistent tiles
            kT_sb = const.tile([P, HP, M], bf16)  # K^T own slice, dk-pair packed
            vt_sb = const.tile([P, KC, D], bf16)  # V own slice [k-chunk, v]
            qT_sb = const.tile([P, HP, GQ], bf16)  # Q^T whole group

            ctx_cc_in = dram.tile([QB * D, M], bf16)
            ctx_cc_out = dram.tile([D, M], bf16)

            # ---- Phase A: K^T and Q^T projections (V overlaps Phase C) ----
            with tc.tile_pool(name="wpool", bufs=2) as wpool, \
                 tc.tile_pool(name="xk_p", bufs=1) as xk_p, \
                 tc.tile_pool(name="xq_p", bufs=1) as xq_p, \
                 tc.tile_pool(name="psA", bufs=2, space="PSUM") as psA:
                wk_sb = wpool.tile([P, EC, D], bf16, tag="w")
                nc.sync.dma_start(wk_sb[:], wk.rearrange("(o p) e -> p o e", p=P))
                xk_sb = xk_p.tile([P, EC, M], bf16)
                nc.sync.dma_start(xk_sb[:], xkT.rearrange("(o p) q -> p o q", p=P))
                wq_sb = wpool.tile([P, EC, D], bf16, tag="w")
                nc.sync.dma_start(wq_sb[:], wq.rearrange("(o p) e -> p o e", p=P))
                xq_sb = xq_p.tile([P, EC, GQ], bf16)
                nc.sync.dma_start(xq_sb[:], xqT.rearrange("(o p) q -> p o q", p=P))

                # K^T projection: own 512 k rows -> kT_sb [P, hp, 512]
                for w0 in range(0, HP, 3):
                    hps = list(range(w0, min(w0 + 3, HP)))
                    n = len(hps)
                    ps = psA.tile([P, GQ], f32, tag="a")
                    for j, hp in enumerate(hps):
                        for dc in range(EC):
                            nc.tensor.matmul(
                                ps[:, j * M : (j + 1) * M],
                                lhsT=wk_sb[:, dc, hp * P : (hp + 1) * P],
                                rhs=xk_sb[:, dc, :],
                                start=(dc == 0),
                                stop=(dc == EC - 1),
                            )
                    nc.scalar.copy(kT_sb[:, w0 : w0 + n, :], ps[:, : n * M])

                # Q^T projection for the whole group's 2048 q
                for hp in range(HP):
                    ps = psA.tile([P, GQ], f32, tag="a")
                    for qu in range(4):
                        for dc in range(EC):
                            nc.tensor.matmul(
                                ps[:, qu * M : (qu + 1) * M],
                                lhsT=wq_sb[:, dc, hp * P : (hp + 1) * P],
                                rhs=xq_sb[:, dc, qu * M : (qu + 1) * M],
                                start=(dc == 0),
                                stop=(dc == EC - 1),
                            )
                    nc.scalar.copy(qT_sb[:, hp, :], ps[:])

            # ---- Phase C: attention tiles + ctx PSUM chains + partial out ----
            # Software-pipelined: the 8 ctx chains of q-block qb interleave
            # with the first score/exp tiles of qb+1 so neither PE nor the
            # Activation engine idles across the qb boundary.
            cc_view = ctx_cc_in.rearrange("(b o p) q -> p b o q", b=QB, p=P)
            with tc.tile_pool(name="etp", bufs=7) as etp, \
                 tc.tile_pool(name="t8p", bufs=1) as t8p, \
                 tc.tile_pool(name="stg", bufs=3) as stg, \
                 tc.tile_pool(name="psC", bufs=2, space="PSUM") as psC:

                def emit_tile(qb, kc):
                    et = etp.tile([P, H, M], bf16, tag="et")
                    # scores + exp, 3-head waves; head-sum accumulates wave
                    # partials as exps land so `den` is ready ~1 op after the
                    # last wave (keeps ctx chains from stalling on the tree).
                    par = t8p.tile([P, 8, M], bf16, tag="t8")
                    for wi, w0 in enumerate(range(0, H, 3)):
                        hs = list(range(w0, min(w0 + 3, H)))
                        n = len(hs)
                        ps = psC.tile([P, 3 * M], f32, tag="sc")
                        for j, h in enumerate(hs):
                            hp, half = divmod(h, 2)
                            pb = half * DK
                            nc.tensor.matmul(
                                ps[:, j * M : (j + 1) * M],
                                lhsT=kT_sb[pb : pb + DK, hp, kc * P : (kc + 1) * P],
                                rhs=qT_sb[pb : pb + DK, hp, qb * M : (qb + 1) * M],
                                start=True,
                                stop=True,
                            )
                        nc.scalar.activation(
                            et[:, w0 : w0 + n, :], ps[:, : n * M], AF.Exp,
                            scale=0.125,
                        )
                        if n == 3:
                            nc.vector.tensor_tensor(
                                par[:, wi, :], et[:, w0, :], et[:, w0 + 1, :], OP.add
                            )
                            nc.vector.tensor_tensor(
                                par[:, wi, :], par[:, wi, :], et[:, w0 + 2, :], OP.add
                            )
                        if wi == 1:
                            nc.vector.tensor_tensor(
                                par[:, 6, :], par[:, 0, :], par[:, 1, :], OP.add
                            )
                        if wi == 3:
                            nc.vector.tensor_tensor(
                                par[:, 7, :], par[:, 2, :], par[:, 3, :], OP.add
                            )
                    # waves: 5x3 heads + 1 single head (h15)
                    nc.vector.tensor_tensor(
                        par[:, 6, :], par[:, 6, :], par[:, 7, :], OP.add
                    )
                    nc.vector.tensor_tensor(
                        par[:, 4, :], par[:, 4, :], et[:, 15, :], OP.add
                    )
                    den = smal.tile([P, M], f32, tag="den")
                    nc.vector.tensor_tensor(den[:], par[:, 6, :], par[:, 4, :], OP.add)
                    rf = smal.tile([P, M], f32, tag="rf")
                    nc.vector.reciprocal_approx_fast(rf[:], den[:])
                    rb = smal.tile([P, M], bf16, tag="rb")
                    nc.vector.tensor_copy(rb[:], rf[:])
                    # normalize: attn = e * r (in place; split DVE/GpSimd)
                    nsp = 8
                    nc.vector.tensor_tensor(
                        et[:, :nsp, :],
                        et[:, :nsp, :],
                        rb[:, None, :].to_broadcast((P, nsp, M)),
                        OP.mult,
                    )
                    nc.gpsimd.tensor_tensor(
                        et[:, nsp:, :],
                        et[:, nsp:, :],
                        rb[:, None, :].to_broadcast((P, H - nsp, M)),
                        OP.mult,
                    )
                    return et

                def emit_chain(qb, hp, ets):
                    cps = psC.tile([P, M], f32, tag="cx")
                    for kc in range(KC):
                        nc.tensor.matmul(
                            cps[0:DK, :],
                            lhsT=vt_sb[:, kc, (2 * hp) * DK : (2 * hp + 1) * DK],
                            rhs=ets[kc][:, 2 * hp, :],
                            start=(kc == 0),
                            stop=(kc == KC - 1),
                            tile_position=(0, 0),
                        )
                        nc.tensor.matmul(
                            cps[DK:P, :],
                            lhsT=vt_sb[:, kc, (2 * hp + 1) * DK : (2 * hp + 2) * DK],
                            rhs=ets[kc][:, 2 * hp + 1, :],
                            start=(kc == 0),
                            stop=(kc == KC - 1),
                            tile_position=(0, DK),
                        )
                    ost = stg.tile([P, M], bf16, tag="ost")
                    if hp % 8 < 5:
                        nc.scalar.copy(ost[:], cps[:])
                    else:
                        nc.vector.tensor_copy(ost[:], cps[:])
                    nc.sync.dma_start(cc_view[:, qb, hp, :], ost[:])

                # V projection interleaved with the first q-block's tiles so
                # the Activation engine warms up while PE projects V.
                with tc.tile_pool(name="wv_p", bufs=1) as wv_p, \
                     tc.tile_pool(name="xv_p", bufs=1) as xv_p:
                    wv_sb = wv_p.tile([P, EC, D], bf16)
                    nc.sync.dma_start(wv_sb[:], wv.rearrange("(o p) e -> p o e", p=P))
                    xv_sb = xv_p.tile([P, EC, M], bf16)
                    nc.sync.dma_start(xv_sb[:], xvT.rearrange("(o p) q -> p o q", p=P))

                    def emit_vproj(kc):
                        ps = psC.tile([P, 3 * M], f32, tag="sc")
                        for half in range(2):
                            for dc in range(EC):
                                nc.tensor.matmul(
                                    ps[:, half * M : (half + 1) * M],
                                    lhsT=xv_sb[:, dc, kc * P : (kc + 1) * P],
                                    rhs=wv_sb[:, dc, half * M : (half + 1) * M],
                                    start=(dc == 0),
                                    stop=(dc == EC - 1),
                                )
                        nc.vector.tensor_copy(vt_sb[:, kc, :], ps[:, :D])

                    ets_cur = [emit_tile(0, 0)]
                    emit_vproj(0)
                    emit_vproj(1)
                    ets_cur.append(emit_tile(0, 1))
                    emit_vproj(2)
                    emit_vproj(3)
                    ets_cur.append(emit_tile(0, 2))
                    ets_cur.append(emit_tile(0, 3))
                for qb in range(QB):
                    ets_next = []
                    for hp in range(HP):
                        emit_chain(qb, hp, ets_cur)
                        # 3-tile lookahead into qb+1 (etp bufs=7 allows it)
                        if qb + 1 < QB and hp in (1, 3, 5):
                            ets_next.append(emit_tile(qb + 1, len(ets_next)))
                    if qb + 1 < QB:
                        while len(ets_next) < KC:
                            ets_next.append(emit_tile(qb + 1, len(ets_next)))
                    ets_cur = ets_next

            # ---- ReduceScatter + Phase D: O-proj + residual + LayerNorm ----
            res_view = xres.rearrange("(o p) e -> o p e", p=P)
            out_view = out.rearrange("(o p) e -> o p e", p=P)
            with tc.tile_pool(name="dpool", bufs=1) as dpool, \
                 tc.tile_pool(name="resp", bufs=2) as resp, \
                 tc.tile_pool(name="lnp", bufs=2) as lnp, \
                 tc.tile_pool(name="psD", bufs=2, space="PSUM") as psD:
                wo_sb = dpool.tile([P, EC, D], bf16)
                nc.sync.dma_start(wo_sb[:], wo.rearrange("(o p) e -> p o e", p=P))
                gam_sb = dpool.tile([P, D], f32)
                nc.sync.dma_start(gam_sb[:], gam[:])
                bet_sb = dpool.tile([P, D], f32)
                nc.sync.dma_start(bet_sb[:], bet[:])
                ctxT_sb = dpool.tile([P, EC, M], bf16)  # summed ctx^T own q
                nc.gpsimd.collective_compute(
                    "ReduceScatter",
                    mybir.AluOpType.add,
                    replica_groups=rg,
                    ins=[ctx_cc_in.opt()],
                    outs=[ctx_cc_out.opt()],
                )
                nc.sync.dma_start(
                    ctxT_sb[:], ctx_cc_out.rearrange("(o p) q -> p o q", p=P)
                )
                for qc in range(M // P):
                    rest = resp.tile([P, D], f32, tag="res")
                    nc.sync.dma_start(rest[:], res_view[qc])
                    ps = psD.tile([P, D], f32, tag="o")
                    for half in range(2):
                        for vc in range(EC):
                            nc.tensor.matmul(
                                ps[:, half * M : (half + 1) * M],
                                lhsT=ctxT_sb[:, vc, qc * P : (qc + 1) * P],
                                rhs=wo_sb[:, vc, half * M : (half + 1) * M],
                                start=(vc == 0),
                                stop=(vc == EC - 1),
                            )
                    xsb = lnp.tile([P, D], f32, tag="x")
                    nc.vector.tensor_tensor(xsb[:], ps[:], rest[:], OP.add)

                    # mean/var in one DVE pass: bn_stats over 2 chunks of 512
                    bst = smal.tile([P, 2, 6], f32, tag="bst")
                    nc.vector.bn_stats(bst[:, 0, :], xsb[:, 0:M])
                    nc.vector.bn_stats(bst[:, 1, :], xsb[:, M:D])
                    agg = smal.tile([P, 2], f32, tag="agg")
                    nc.vector.bn_aggr(agg[:], bst[:])
                    veps = smal.tile([P, 1], f32, tag="veps")
                    nc.vector.tensor_scalar(
                        veps[:], agg[:, 1:2], 1.0, LN_EPS, OP.mult, OP.add
                    )
                    std = smal.tile([P, 1], f32, tag="std")
                    nc.scalar.activation(std[:], veps[:], AF.Sqrt)
                    inv = smal.tile([P, 1], f32, tag="inv")
                    nc.vector.reciprocal(inv[:], std[:])
                    # one Newton-Raphson step: inv *= 1.5 - 0.5*veps*inv^2
                    t1 = smal.tile([P, 1], f32, tag="t1")
                    nc.vector.tensor_tensor(t1[:], inv[:], inv[:], OP.mult)
                    nc.vector.tensor_tensor(t1[:], t1[:], veps[:], OP.mult)
                    nc.vector.tensor_scalar(t1[:], t1[:], -0.5, 1.5, OP.mult, OP.add)
                    nc.vector.tensor_tensor(inv[:], inv[:], t1[:], OP.mult)
                    # xn = (x - mu) * inv ; then *gamma on DVE, +beta on Pool
                    nc.vector.tensor_scalar(
                        xsb[:], xsb[:], agg[:, 0:1], inv[:], OP.subtract, OP.mult
                    )
                    nc.vector.tensor_tensor(xsb[:], xsb[:], gam_sb[:], OP.mult)
                    ot = lnp.tile([P, D], f32, tag="ot")
                    nc.gpsimd.tensor_tensor(ot[:], xsb[:], bet_sb[:], OP.add)
                    nc.sync.dma_start(out_view[qc], ot[:])

    nc.compile()
    return nc


def _get_nc():
    if "nc" not in _CACHE:
        _CACHE["nc"] = _build()
    return _CACHE["nc"]


def _in_maps(input_Q, input_K, input_V, W_Q, W_K, W_V, W_O, ln_gamma, ln_beta):
    bf = ml_dtypes.bfloat16
    f32 = np.float32
    Q_ = np.asarray(input_Q, dtype=f32)
    K_ = np.asarray(input_K, dtype=f32)
    V_ = np.asarray(input_V, dtype=f32)
    wq_b = np.asarray(W_Q, dtype=f32).astype(bf)
    wk_b = np.asarray(W_K, dtype=f32).astype(bf)
    wv_b = np.asarray(W_V, dtype=f32).astype(bf)
    wo_b = np.asarray(W_O, dtype=f32).astype(bf)
    gam_b = np.ascontiguousarray(
        np.broadcast_to(np.asarray(ln_gamma, dtype=f32), (P, D))
    )
    bet_b = np.ascontiguousarray(
        np.broadcast_to(np.asarray(ln_beta, dtype=f32), (P, D))
    )
    maps = []
    for c in range(8):
        b, r = divmod(c, G)
        sl = slice(r * M, (r + 1) * M)
        maps.append(
            {
                "xqT": np.ascontiguousarray(Q_[b].T).astype(bf),
                "xkT": np.ascontiguousarray(K_[b, sl].T).astype(bf),
                "xvT": np.ascontiguousarray(V_[b, sl].T).astype(bf),
                "xres": np.ascontiguousarray(Q_[b, sl]),
                "wq": wq_b,
                "wk": wk_b,
                "wv": wv_b,
                "wo": wo_b,
                "gam": gam_b,
                "bet": bet_b,
            }
        )
    return maps


def _assemble(results):
    B = 2
    out = np.empty((B, SK, D), np.float32)
    for c in range(8):
        b, r = divmod(c, G)
        out[b, r * M : (r + 1) * M] = results[c]["out"]
    return out


def run_traced(trace=False, **inputs):
    """Run on HW; returns (output, BassKernelResults)."""
    from concourse.bass_utils import run_bass_kernel_spmd

    nc = _get_nc()
    maps = _in_maps(**inputs)
    res = run_bass_kernel_spmd(nc, maps, list(range(8)), trace=trace)
    return _assemble(res.results), res


def kernel(**inputs) -> np.ndarray:
    out, _ = run_traced(trace=False, **inputs)
    return out


# revision 4
# speedup vs baseline: 1.0308x; 1.0308x over previous
"""Trainium2 Bass kernel for nn_MultiHeadAttention (head-axis softmax quirk).

v2 strategy (8 NeuronCores, 2 batch-groups x 4 cores):
  - NO pre-attention collectives. Softmax over HEADS is local per (q,k), so
    attention is sharded over the K/V length: each core projects its own
    512-row k-slice of K and V, and redundantly projects Q for the whole
    group's 2048 q rows (+41us PE, but saves ~240us of AllGather).
  - Per core: scores^T [k,q] tiles (128k x 512q x 16h), exp on ScalarE,
    head-sum via DVE tree-adds (bf16 4x mode), normalize split DVE/GpSimd,
    context accumulated across the 4 local k-chunks in PSUM chains (no
    SBUF accumulation traffic).
  - One bf16 ReduceScatter of ctx^T partials (4MB in -> 1MB out) hands each
    core the k-summed context for its own 512 q rows.
  - Output projection + residual + LayerNorm on own q rows.
  - All matmuls bf16 with fp32 PSUM accumulation.
"""

import numpy as np
import ml_dtypes

D = 1024
H = 16
DK = 64
P = 128
M = 512  # q rows per core (own slice) / k rows per core
SK = 2048
G = 4  # cores per batch group
GQ = 2048  # q rows per group
EC = D // P  # 8
HP = H // 2  # 8
KC = M // P  # 4 local k chunks
QB = GQ // M  # 4 q blocks (one per destination core)
LN_EPS = 1e-5

_CACHE = {}


def _build():
    import concourse.bass as bass
    import concourse.mybir as mybir
    import concourse.tile as tile
    from concourse import bacc

    f32 = mybir.dt.float32
    bf16 = mybir.dt.bfloat16
    AF = mybir.ActivationFunctionType
    OP = mybir.AluOpType
    AX = mybir.AxisListType

    nc = bacc.Bacc("TRN2", target_bir_lowering=False, debug=False, num_devices=8)

    xqT = nc.dram_tensor("xqT", [D, GQ], bf16, kind="ExternalInput").ap()
    xkT = nc.dram_tensor("xkT", [D, M], bf16, kind="ExternalInput").ap()
    xvT = nc.dram_tensor("xvT", [D, M], bf16, kind="ExternalInput").ap()
    xres = nc.dram_tensor("xres", [M, D], f32, kind="ExternalInput").ap()
    wq = nc.dram_tensor("wq", [D, D], bf16, kind="ExternalInput").ap()
    wk = nc.dram_tensor("wk", [D, D], bf16, kind="ExternalInput").ap()
    wv = nc.dram_tensor("wv", [D, D], bf16, kind="ExternalInput").ap()
    wo = nc.dram_tensor("wo", [D, D], bf16, kind="ExternalInput").ap()
    gam = nc.dram_tensor("gam", [P, D], f32, kind="ExternalInput").ap()
    bet = nc.dram_tensor("bet", [P, D], f32, kind="ExternalInput").ap()
    out = nc.dram_tensor("out", [M, D], f32, kind="ExternalOutput").ap()

    rg = [[0, 1, 2, 3], [4, 5, 6, 7]]

    from contextlib import ExitStack

    with tile.TileContext(nc) as tc:
        with ExitStack() as ctx:
            const = ctx.enter_context(tc.tile_pool(name="const", bufs=1))
            smal = ctx.enter_context(tc.tile_pool(name="smal", bufs=2))
            dram = ctx.enter_context(tc.tile_pool(name="dram", bufs=1, space="DRAM"))

            # persistent tiles
            kT_sb = const.tile([P, HP, M], bf16)  # K^T own slice, dk-pair packed
            vt_sb = const.tile([P, KC, D], bf16)  # V own slice [k-chunk, v]
            qT_sb = const.tile([P, HP, GQ], bf16)  # Q^T whole group

            ctx_cc_in = dram.tile([QB * D, M], bf16)
            ctx_cc_out = dram.tile([D, M], bf16)

            # ---- Phase A: K^T and Q^T projections (V overlaps Phase C) ----
            with tc.tile_pool(name="wpool", bufs=2) as wpool, \
                 tc.tile_pool(name="xk_p", bufs=1) as xk_p, \
                 tc.tile_pool(name="xq_p", bufs=1) as xq_p, \
                 tc.tile_pool(name="psA", bufs=2, space="PSUM") as psA:
                wk_sb = wpool.tile([P, EC, D], bf16, tag="w")
                nc.sync.dma_start(wk_sb[:], wk.rearrange("(o p) e -> p o e", p=P))
                xk_sb = xk_p.tile([P, EC, M], bf16)
                nc.sync.dma_start(xk_sb[:], xkT.rearrange("(o p) q -> p o q", p=P))
                wq_sb = wpool.tile([P, EC, D], bf16, tag="w")
                nc.sync.dma_start(wq_sb[:], wq.rearrange("(o p) e -> p o e", p=P))
                xq_sb = xq_p.tile([P, EC, GQ], bf16)
                nc.sync.dma_start(xq_sb[:], xqT.rearrange("(o p) q -> p o q", p=P))

                # K^T projection: own 512 k rows -> kT_sb [P, hp, 512]
                for w0 in range(0, HP, 3):
                    hps = list(range(w0, min(w0 + 3, HP)))
                    n = len(hps)
                    ps = psA.tile([P, GQ], f32, tag="a")
                    for j, hp in enumerate(hps):
                        for dc in range(EC):
                            nc.tensor.matmul(
                                ps[:, j * M : (j + 1) * M],
                                lhsT=wk_sb[:, dc, hp * P : (hp + 1) * P],
                                rhs=xk_sb[:, dc, :],
                                start=(dc == 0),
                                stop=(dc == EC - 1),
                            )
                    nc.scalar.copy(kT_sb[:, w0 : w0 + n, :], ps[:, : n * M])

                # Q^T projection for the whole group's 2048 q
                for hp in range(HP):
                    ps = psA.tile([P, GQ], f32, tag="a")
                    for qu in range(4):
                        for dc in range(EC):
                            nc.tensor.matmul(
                                ps[:, qu * M : (qu + 1) * M],
                                lhsT=wq_sb[:, dc, hp * P : (hp + 1) * P],
                                rhs=xq_sb[:, dc, qu * M : (qu + 1) * M],
                                start=(dc == 0),
                                stop=(dc == EC - 1),
                            )
                    nc.scalar.copy(qT_sb[:, hp, :], ps[:])

            # ---- Phase C: attention tiles + ctx PSUM chains + partial out ----
            # Software-pipelined: the 8 ctx chains of q-block qb interleave
            # with the first score/exp tiles of qb+1 so neither PE nor the
            # Activation engine idles across the qb boundary.
            cc_view = ctx_cc_in.rearrange("(b o p) q -> p b o q", b=QB, p=P)
            with tc.tile_pool(name="etp", bufs=7) as etp, \
                 tc.tile_pool(name="t8p", bufs=1) as t8p, \
                 tc.tile_pool(name="stg", bufs=3) as stg, \
                 tc.tile_pool(name="psC", bufs=2, space="PSUM") as psC:

                def emit_tile(qb, kc):
                    et = etp.tile([P, H, M], bf16, tag="et")
                    # scores + exp, 3-head waves; head-sum accumulates wave
                    # partials as exps land so `den` is ready ~1 op after the
                    # last wave (keeps ctx chains from stalling on the tree).
                    par = t8p.tile([P, 8, M], bf16, tag="t8")
                    for wi, w0 in enumerate(range(0, H, 3)):
                        hs = list(range(w0, min(w0 + 3, H)))
                        n = len(hs)
                        ps = psC.tile([P, 3 * M], f32, tag="sc")
                        for j, h in enumerate(hs):
                            hp, half = divmod(h, 2)
                            pb = half * DK
                            nc.tensor.matmul(
                                ps[:, j * M : (j + 1) * M],
                                lhsT=kT_sb[pb : pb + DK, hp, kc * P : (kc + 1) * P],
                                rhs=qT_sb[pb : pb + DK, hp, qb * M : (qb + 1) * M],
                                start=True,
                                stop=True,
                            )
                        nc.scalar.activation(
                            et[:, w0 : w0 + n, :], ps[:, : n * M], AF.Exp,
                            scale=0.125,
                        )
                        if n == 3:
                            nc.vector.tensor_tensor(
                                par[:, wi, :], et[:, w0, :], et[:, w0 + 1, :], OP.add
                            )
                            nc.vector.tensor_tensor(
                                par[:, wi, :], par[:, wi, :], et[:, w0 + 2, :], OP.add
                            )
                        if wi == 1:
                            nc.vector.tensor_tensor(
                                par[:, 6, :], par[:, 0, :], par[:, 1, :], OP.add
                            )
                        if wi == 3:
                            nc.vector.tensor_tensor(
                                par[:, 7, :], par[:, 2, :], par[:, 3, :], OP.add
                            )
                    # waves: 5x3 heads + 1 single head (h15)
                    nc.vector.tensor_tensor(
                        par[:, 6, :], par[:, 6, :], par[:, 7, :], OP.add
                    )
                    nc.vector.tensor_tensor(
                        par[:, 4, :], par[:, 4, :], et[:, 15, :], OP.add
                    )
                    den = smal.tile([P, M], f32, tag="den")
                    nc.vector.tensor_tensor(den[:], par[:, 6, :], par[:, 4, :], OP.add)
                    rf = smal.tile([P, M], f32, tag="rf")
                    nc.vector.reciprocal_approx_fast(rf[:], den[:])
                    rb = smal.tile([P, M], bf16, tag="rb")
                    nc.vector.tensor_copy(rb[:], rf[:])
                    # normalize: attn = e * r (in place; split DVE/GpSimd)
                    nsp = 8
                    nc.vector.tensor_tensor(
                        et[:, :nsp, :],
                        et[:, :nsp, :],
                        rb[:, None, :].to_broadcast((P, nsp, M)),
                        OP.mult,
                    )
                    nc.gpsimd.tensor_tensor(
                        et[:, nsp:, :],
                        et[:, nsp:, :],
                        rb[:, None, :].to_broadcast((P, H - nsp, M)),
                        OP.mult,
                    )
                    return et

                def emit_chain(qb, hp, ets):
                    cps = psC.tile([P, M], f32, tag="cx")
                    for kc in range(KC):
                        nc.tensor.matmul(
                            cps[0:DK, :],
                            lhsT=vt_sb[:, kc, (2 * hp) * DK : (2 * hp + 1) * DK],
                            rhs=ets[kc][:, 2 * hp, :],
                            start=(kc == 0),
                            stop=(kc == KC - 1),
                            tile_position=(0, 0),
                        )
                        nc.tensor.matmul(
                            cps[DK:P, :],
                            lhsT=vt_sb[:, kc, (2 * hp + 1) * DK : (2 * hp + 2) * DK],
                            rhs=ets[kc][:, 2 * hp + 1, :],
                            start=(kc == 0),
                            stop=(kc == KC - 1),
                            tile_position=(0, DK),
                        )
                    ost = stg.tile([P, M], bf16, tag="ost")
                    if hp % 8 < 5:
                        nc.scalar.copy(ost[:], cps[:])
                    else:
                        nc.vector.tensor_copy(ost[:], cps[:])
                    nc.sync.dma_start(cc_view[:, qb, hp, :], ost[:])

                # V projection interleaved with the first q-block's tiles so
                # the Activation engine warms up while PE projects V.
                with tc.tile_pool(name="wv_p", bufs=1) as wv_p, \
                     tc.tile_pool(name="xv_p", bufs=1) as xv_p:
                    wv_sb = wv_p.tile([P, EC, D], bf16)
                    nc.sync.dma_start(wv_sb[:], wv.rearrange("(o p) e -> p o e", p=P))
                    xv_sb = xv_p.tile([P, EC, M], bf16)
                    nc.sync.dma_start(xv_sb[:], xvT.rearrange("(o p) q -> p o q", p=P))

                    def emit_vproj(kc):
                        ps = psC.tile([P, 3 * M], f32, tag="sc")
                        for half in range(2):
                            for dc in range(EC):
                                nc.tensor.matmul(
                                    ps[:, half * M : (half + 1) * M],
                                    lhsT=xv_sb[:, dc, kc * P : (kc + 1) * P],
                                    rhs=wv_sb[:, dc, half * M : (half + 1) * M],
                                    start=(dc == 0),
                                    stop=(dc == EC - 1),
                                )
                        nc.vector.tensor_copy(vt_sb[:, kc, :], ps[:, :D])

                    ets_cur = [emit_tile(0, 0)]
                    emit_vproj(0)
                    emit_vproj(1)
                    ets_cur.append(emit_tile(0, 1))
                    emit_vproj(2)
                    emit_vproj(3)
                    ets_cur.append(emit_tile(0, 2))
                    ets_cur.append(emit_tile(0, 3))
                for qb in range(QB):
                    ets_next = []
                    for hp in range(HP):
                        emit_chain(qb, hp, ets_cur)
                        # 3-tile lookahead into qb+1 (etp bufs=7 allows it)
                        if qb + 1 < QB and hp in (1, 3, 5):
                            ets_next.append(emit_tile(qb + 1, len(ets_next)))
                    if qb + 1 < QB:
                        while len(ets_next) < KC:
                            ets_next.append(emit_tile(qb + 1, len(ets_next)))
                    ets_cur = ets_next

            # ---- ReduceScatter + Phase D: O-proj + residual + LayerNorm ----
            res_view = xres.rearrange("(o p) e -> o p e", p=P)
            out_view = out.rearrange("(o p) e -> o p e", p=P)
            with tc.tile_pool(name="dpool", bufs=1) as dpool, \
                 tc.tile_pool(name="resp", bufs=2) as resp, \
                 tc.tile_pool(name="lnp", bufs=2) as lnp, \
                 tc.tile_pool(name="psD", bufs=2, space="PSUM") as psD:
                wo_sb = dpool.tile([P, EC, D], bf16)
                nc.sync.dma_start(wo_sb[:], wo.rearrange("(o p) e -> p o e", p=P))
                gam_sb = dpool.tile([P, D], f32)
                nc.sync.dma_start(gam_sb[:], gam[:])
                bet_sb = dpool.tile([P, D], f32)
                nc.sync.dma_start(bet_sb[:], bet[:])
                ctxT_sb = dpool.tile([P, EC, M], bf16)  # summed ctx^T own q
                nc.gpsimd.collective_compute(
                    "ReduceScatter",
                    mybir.AluOpType.add,
                    replica_groups=rg,
                    ins=[ctx_cc_in.opt()],
                    outs=[ctx_cc_out.opt()],
                )
                ccv = ctx_cc_out.rearrange("(o p) q -> p o q", p=P)
                for vc in range(EC):
                    nc.sync.dma_start(ctxT_sb[:, vc, :], ccv[:, vc, :])
                for qc in range(M // P):
                    rest = resp.tile([P, D], f32, tag="res")
                    nc.sync.dma_start(rest[:], res_view[qc])
                    ps = psD.tile([P, D], f32, tag="o")
                    for half in range(2):
                        for vc in range(EC):
                            nc.tensor.matmul(
                                ps[:, half * M : (half + 1) * M],
                                lhsT=ctxT_sb[:, vc, qc * P : (qc + 1) * P],
                                rhs=wo_sb[:, vc, half * M : (half + 1) * M],
                                start=(vc == 0),
                                stop=(vc == EC - 1),
                            )
                    xsb = lnp.tile([P, D], f32, tag="x")
                    nc.vector.tensor_tensor(xsb[:], ps[:], rest[:], OP.add)

                    # mean/var in one DVE pass: bn_stats over 2 chunks of 512
                    bst = smal.tile([P, 2, 6], f32, tag="bst")
                    nc.vector.bn_stats(bst[:, 0, :], xsb[:, 0:M])
                    nc.vector.bn_stats(bst[:, 1, :], xsb[:, M:D])
                    agg = smal.tile([P, 2], f32, tag="agg")
                    nc.vector.bn_aggr(agg[:], bst[:])
                    veps = smal.tile([P, 1], f32, tag="veps")
                    nc.vector.tensor_scalar(
                        veps[:], agg[:, 1:2], 1.0, LN_EPS, OP.mult, OP.add
                    )
                    std = smal.tile([P, 1], f32, tag="std")
                    nc.scalar.activation(std[:], veps[:], AF.Sqrt)
                    inv = smal.tile([P, 1], f32, tag="inv")
                    nc.vector.reciprocal(inv[:], std[:])
                    # xn = (x - mu) * inv ; then *gamma on DVE, +beta on Pool
                    nc.vector.tensor_scalar(
                        xsb[:], xsb[:], agg[:, 0:1], inv[:], OP.subtract, OP.mult
                    )
                    nc.vector.tensor_tensor(xsb[:], xsb[:], gam_sb[:], OP.mult)
                    ot = lnp.tile([P, D], f32, tag="ot")
                    nc.gpsimd.tensor_tensor(ot[:], xsb[:], bet_sb[:], OP.add)
                    nc.sync.dma_start(out_view[qc], ot[:])

    nc.compile()
    return nc


def _get_nc():
    if "nc" not in _CACHE:
        _CACHE["nc"] = _build()
    return _CACHE["nc"]


def _in_maps(input_Q, input_K, input_V, W_Q, W_K, W_V, W_O, ln_gamma, ln_beta):
    bf = ml_dtypes.bfloat16
    f32 = np.float32
    Q_ = np.asarray(input_Q, dtype=f32)
    K_ = np.asarray(input_K, dtype=f32)
    V_ = np.asarray(input_V, dtype=f32)
    wq_b = np.asarray(W_Q, dtype=f32).astype(bf)
    wk_b = np.asarray(W_K, dtype=f32).astype(bf)
    wv_b = np.asarray(W_V, dtype=f32).astype(bf)
    wo_b = np.asarray(W_O, dtype=f32).astype(bf)
    gam_b = np.ascontiguousarray(
        np.broadcast_to(np.asarray(ln_gamma, dtype=f32), (P, D))
    )
    bet_b = np.ascontiguousarray(
        np.broadcast_to(np.asarray(ln_beta, dtype=f32), (P, D))
    )
    maps = []
    for c in range(8):
        b, r = divmod(c, G)
        sl = slice(r * M, (r + 1) * M)
        maps.append(
            {
                "xqT": np.ascontiguousarray(Q_[b].T).astype(bf),
                "xkT": np.ascontiguousarray(K_[b, sl].T).astype(bf),
                "xvT": np.ascontiguousarray(V_[b, sl].T).astype(bf),
                "xres": np.ascontiguousarray(Q_[b, sl]),
                "wq": wq_b,
                "wk": wk_b,
                "wv": wv_b,
                "wo": wo_b,
                "gam": gam_b,
                "bet": bet_b,
            }
        )
    return maps


def _assemble(results):
    B = 2
    out = np.empty((B, SK, D), np.float32)
    for c in range(8):
        b, r = divmod(c, G)
        out[b, r * M : (r + 1) * M] = results[c]["out"]
    return out


def run_traced(trace=False, **inputs):
    """Run on HW; returns (output, BassKernelResults)."""
    from concourse.bass_utils import run_bass_kernel_spmd

    nc = _get_nc()
    maps = _in_maps(**inputs)
    res = run_bass_kernel_spmd(nc, maps, list(range(8)), trace=trace)
    return _assemble(res.results), res


def kernel(**inputs) -> np.ndarray:
    out, _ = run_traced(trace=False, **inputs)
    return out


# revision 5
# speedup vs baseline: 1.0677x; 1.0358x over previous
"""Trainium2 Bass kernel for nn_MultiHeadAttention (head-axis softmax quirk).

v2 strategy (8 NeuronCores, 2 batch-groups x 4 cores):
  - NO pre-attention collectives. Softmax over HEADS is local per (q,k), so
    attention is sharded over the K/V length: each core projects its own
    512-row k-slice of K and V, and redundantly projects Q for the whole
    group's 2048 q rows (+41us PE, but saves ~240us of AllGather).
  - Per core: scores^T [k,q] tiles (128k x 512q x 16h), exp on ScalarE,
    head-sum via DVE tree-adds (bf16 4x mode), normalize split DVE/GpSimd,
    context accumulated across the 4 local k-chunks in PSUM chains (no
    SBUF accumulation traffic).
  - One bf16 ReduceScatter of ctx^T partials (4MB in -> 1MB out) hands each
    core the k-summed context for its own 512 q rows.
  - Output projection + residual + LayerNorm on own q rows.
  - All matmuls bf16 with fp32 PSUM accumulation.
"""

import numpy as np
import ml_dtypes

D = 1024
H = 16
DK = 64
P = 128
M = 512  # q rows per core (own slice) / k rows per core
SK = 2048
G = 4  # cores per batch group
GQ = 2048  # q rows per group
EC = D // P  # 8
HP = H // 2  # 8
KC = M // P  # 4 local k chunks
QB = GQ // M  # 4 q blocks (one per destination core)
LN_EPS = 1e-5

_CACHE = {}


def _build():
    import concourse.bass as bass
    import concourse.mybir as mybir
    import concourse.tile as tile
    from concourse import bacc

    f32 = mybir.dt.float32
    bf16 = mybir.dt.bfloat16
    AF = mybir.ActivationFunctionType
    OP = mybir.AluOpType
    AX = mybir.AxisListType

    nc = bacc.Bacc("TRN2", target_bir_lowering=False, debug=False, num_devices=8)

    xqT = nc.dram_tensor("xqT", [D, GQ], bf16, kind="ExternalInput").ap()
    xkT = nc.dram_tensor("xkT", [D, M], bf16, kind="ExternalInput").ap()
    xvT = nc.dram_tensor("xvT", [D, M], bf16, kind="ExternalInput").ap()
    xres = nc.dram_tensor("xres", [M, D], f32, kind="ExternalInput").ap()
    wq = nc.dram_tensor("wq", [D, D], bf16, kind="ExternalInput").ap()
    wk = nc.dram_tensor("wk", [D, D], bf16, kind="ExternalInput").ap()
    wv = nc.dram_tensor("wv", [D, D], bf16, kind="ExternalInput").ap()
    wo = nc.dram_tensor("wo", [D, D], bf16, kind="ExternalInput").ap()
    gam = nc.dram_tensor("gam", [P, D], f32, kind="ExternalInput").ap()
    bet = nc.dram_tensor("bet", [P, D], f32, kind="ExternalInput").ap()
    out = nc.dram_tensor("out", [M, D], f32, kind="ExternalOutput").ap()

    rg = [[0, 1, 2, 3], [4, 5, 6, 7]]

    from contextlib import ExitStack

    with tile.TileContext(nc) as tc:
        with ExitStack() as ctx:
            const = ctx.enter_context(tc.tile_pool(name="const", bufs=1))
            smal = ctx.enter_context(tc.tile_pool(name="smal", bufs=2))
            dram = ctx.enter_context(tc.tile_pool(name="dram", bufs=1, space="DRAM"))

            # persistent tiles
            kT_sb = const.tile([P, HP, M], bf16)  # K^T own slice, dk-pair packed
            vt_sb = const.tile([P, KC, D], bf16)  # V own slice [k-chunk, v]
            qT_sb = const.tile([P, HP, GQ], bf16)  # Q^T whole group

            ctx_cc_in = dram.tile([QB * D, M], bf16)
            ctx_cc_out = dram.tile([D, M], bf16)

            # ---- Phase A: K^T, V, and first-quarter Q^T projections ----
            # Q quarters 1-3 are emitted inside Phase C where PE has idle
            # slots (C is Activation-paced); wq/xq pools outlive Phase A.
            wq_p = ctx.enter_context(tc.tile_pool(name="wq_p", bufs=1))
            xq_p = ctx.enter_context(tc.tile_pool(name="xq_p", bufs=2))
            wq_sb = wq_p.tile([P, EC, D], bf16)
            xqv = xqT.rearrange("(o p) q -> p o q", p=P)

            def emit_qproj_qu(qu, pool, tag, evac_dve):
                xq_qu = xq_p.tile([P, EC, M], bf16, tag="xq")
                nc.sync.dma_start(xq_qu[:], xqv[:, :, qu * M : (qu + 1) * M])
                for hp in range(HP):
                    ps = pool.tile([P, M] if tag == "cx" else [P, GQ], f32, tag=tag)
                    for dc in range(EC):
                        nc.tensor.matmul(
                            ps[:, 0:M],
                            lhsT=wq_sb[:, dc, hp * P : (hp + 1) * P],
                            rhs=xq_qu[:, dc, :],
                            start=(dc == 0),
                            stop=(dc == EC - 1),
                        )
                    dst = qT_sb[:, hp, qu * M : (qu + 1) * M]
                    if evac_dve:
                        nc.vector.tensor_copy(dst, ps[:, 0:M])
                    else:
                        nc.scalar.copy(dst, ps[:, 0:M])

            with tc.tile_pool(name="wpool", bufs=2) as wpool, \
                 tc.tile_pool(name="xk_p", bufs=1) as xk_p, \
                 tc.tile_pool(name="xv_p", bufs=1) as xv_p, \
                 tc.tile_pool(name="psA", bufs=2, space="PSUM") as psA:
                wk_sb = wpool.tile([P, EC, D], bf16, tag="w")
                nc.sync.dma_start(wk_sb[:], wk.rearrange("(o p) e -> p o e", p=P))
                xk_sb = xk_p.tile([P, EC, M], bf16)
                nc.sync.dma_start(xk_sb[:], xkT.rearrange("(o p) q -> p o q", p=P))
                wv_sb = wpool.tile([P, EC, D], bf16, tag="w")
                nc.sync.dma_start(wv_sb[:], wv.rearrange("(o p) e -> p o e", p=P))
                xv_sb = xv_p.tile([P, EC, M], bf16)
                nc.sync.dma_start(xv_sb[:], xvT.rearrange("(o p) q -> p o q", p=P))
                nc.sync.dma_start(wq_sb[:], wq.rearrange("(o p) e -> p o e", p=P))

                # K^T projection: own 512 k rows -> kT_sb [P, hp, 512]
                for w0 in range(0, HP, 3):
                    hps = list(range(w0, min(w0 + 3, HP)))
                    n = len(hps)
                    ps = psA.tile([P, GQ], f32, tag="a")
                    for j, hp in enumerate(hps):
                        for dc in range(EC):
                            nc.tensor.matmul(
                                ps[:, j * M : (j + 1) * M],
                                lhsT=wk_sb[:, dc, hp * P : (hp + 1) * P],
                                rhs=xk_sb[:, dc, :],
                                start=(dc == 0),
                                stop=(dc == EC - 1),
                            )
                    nc.scalar.copy(kT_sb[:, w0 : w0 + n, :], ps[:, : n * M])

                # V projection: own 512 k rows -> vt_sb [P(k), kc, 1024(v)]
                for kc in range(KC):
                    ps = psA.tile([P, GQ], f32, tag="a")
                    for half in range(2):
                        for dc in range(EC):
                            nc.tensor.matmul(
                                ps[:, half * M : (half + 1) * M],
                                lhsT=xv_sb[:, dc, kc * P : (kc + 1) * P],
                                rhs=wv_sb[:, dc, half * M : (half + 1) * M],
                                start=(dc == 0),
                                stop=(dc == EC - 1),
                            )
                    nc.scalar.copy(vt_sb[:, kc, :], ps[:, :D])

                # Q^T projection, first quarter (covers q-block 0)
                emit_qproj_qu(0, psA, "a", evac_dve=False)

            # ---- Phase C: attention tiles + ctx PSUM chains + partial out ----
            # Software-pipelined: the 8 ctx chains of q-block qb interleave
            # with the first score/exp tiles of qb+1 so neither PE nor the
            # Activation engine idles across the qb boundary.
            cc_view = ctx_cc_in.rearrange("(b o p) q -> p b o q", b=QB, p=P)
            with tc.tile_pool(name="etp", bufs=6) as etp, \
                 tc.tile_pool(name="t8p", bufs=1) as t8p, \
                 tc.tile_pool(name="stg", bufs=3) as stg, \
                 tc.tile_pool(name="psC", bufs=2, space="PSUM") as psC:

                def emit_tile(qb, kc):
                    et = etp.tile([P, H, M], bf16, tag="et")
                    # scores + exp, 3-head waves; head-sum accumulates wave
                    # partials as exps land so `den` is ready ~1 op after the
                    # last wave (keeps ctx chains from stalling on the tree).
                    par = t8p.tile([P, 8, M], bf16, tag="t8")
                    for wi, w0 in enumerate(range(0, H, 3)):
                        hs = list(range(w0, min(w0 + 3, H)))
                        n = len(hs)
                        ps = psC.tile([P, 3 * M], f32, tag="sc")
                        for j, h in enumerate(hs):
                            hp, half = divmod(h, 2)
                            pb = half * DK
                            nc.tensor.matmul(
                                ps[:, j * M : (j + 1) * M],
                                lhsT=kT_sb[pb : pb + DK, hp, kc * P : (kc + 1) * P],
                                rhs=qT_sb[pb : pb + DK, hp, qb * M : (qb + 1) * M],
                                start=True,
                                stop=True,
                            )
                        nc.scalar.activation(
                            et[:, w0 : w0 + n, :], ps[:, : n * M], AF.Exp,
                            scale=0.125,
                        )
                        if n == 3:
                            nc.vector.tensor_tensor(
                                par[:, wi, :], et[:, w0, :], et[:, w0 + 1, :], OP.add
                            )
                            nc.vector.tensor_tensor(
                                par[:, wi, :], par[:, wi, :], et[:, w0 + 2, :], OP.add
                            )
                        if wi == 1:
                            nc.vector.tensor_tensor(
                                par[:, 6, :], par[:, 0, :], par[:, 1, :], OP.add
                            )
                        if wi == 3:
                            nc.vector.tensor_tensor(
                                par[:, 7, :], par[:, 2, :], par[:, 3, :], OP.add
                            )
                    # waves: 5x3 heads + 1 single head (h15)
                    nc.vector.tensor_tensor(
                        par[:, 6, :], par[:, 6, :], par[:, 7, :], OP.add
                    )
                    nc.vector.tensor_tensor(
                        par[:, 4, :], par[:, 4, :], et[:, 15, :], OP.add
                    )
                    den = smal.tile([P, M], f32, tag="den")
                    nc.vector.tensor_tensor(den[:], par[:, 6, :], par[:, 4, :], OP.add)
                    rf = smal.tile([P, M], f32, tag="rf")
                    nc.vector.reciprocal_approx_fast(rf[:], den[:])
                    rb = smal.tile([P, M], bf16, tag="rb")
                    nc.vector.tensor_copy(rb[:], rf[:])
                    # normalize: attn = e * r (in place; split DVE/GpSimd)
                    nsp = 8
                    nc.vector.tensor_tensor(
                        et[:, :nsp, :],
                        et[:, :nsp, :],
                        rb[:, None, :].to_broadcast((P, nsp, M)),
                        OP.mult,
                    )
                    nc.gpsimd.tensor_tensor(
                        et[:, nsp:, :],
                        et[:, nsp:, :],
                        rb[:, None, :].to_broadcast((P, H - nsp, M)),
                        OP.mult,
                    )
                    return et

                def emit_chain(qb, hp, ets):
                    cps = psC.tile([P, M], f32, tag="cx")
                    for kc in range(KC):
                        nc.tensor.matmul(
                            cps[0:DK, :],
                            lhsT=vt_sb[:, kc, (2 * hp) * DK : (2 * hp + 1) * DK],
                            rhs=ets[kc][:, 2 * hp, :],
                            start=(kc == 0),
                            stop=(kc == KC - 1),
                            tile_position=(0, 0),
                        )
                        nc.tensor.matmul(
                            cps[DK:P, :],
                            lhsT=vt_sb[:, kc, (2 * hp + 1) * DK : (2 * hp + 2) * DK],
                            rhs=ets[kc][:, 2 * hp + 1, :],
                            start=(kc == 0),
                            stop=(kc == KC - 1),
                            tile_position=(0, DK),
                        )
                    ost = stg.tile([P, M], bf16, tag="ost")
                    if hp % 8 < 5:
                        nc.scalar.copy(ost[:], cps[:])
                    else:
                        nc.vector.tensor_copy(ost[:], cps[:])
                    nc.sync.dma_start(cc_view[:, qb, hp, :], ost[:])

                # Prologue: q-block 0 tiles interleaved with Q^T quarters 1-3
                # (those matmuls fill PE slots while Act runs the exps; their
                # chains borrow the idle "cx" PSUM tag, evacs go to DVE).
                ets_cur = [emit_tile(0, 0)]
                emit_qproj_qu(1, psC, "cx", evac_dve=True)
                ets_cur.append(emit_tile(0, 1))
                emit_qproj_qu(2, psC, "cx", evac_dve=True)
                ets_cur.append(emit_tile(0, 2))
                emit_qproj_qu(3, psC, "cx", evac_dve=True)
                ets_cur.append(emit_tile(0, 3))
                for qb in range(QB):
                    ets_next = []
                    for hp in range(HP):
                        emit_chain(qb, hp, ets_cur)
                        # 3-tile lookahead into qb+1 (etp bufs=7 allows it)
                        if qb + 1 < QB and hp in (1, 3, 5):
                            ets_next.append(emit_tile(qb + 1, len(ets_next)))
                    if qb + 1 < QB:
                        while len(ets_next) < KC:
                            ets_next.append(emit_tile(qb + 1, len(ets_next)))
                    ets_cur = ets_next

            # ---- ReduceScatter + Phase D: O-proj + residual + LayerNorm ----
            res_view = xres.rearrange("(o p) e -> o p e", p=P)
            out_view = out.rearrange("(o p) e -> o p e", p=P)
            with tc.tile_pool(name="dpool", bufs=1) as dpool, \
                 tc.tile_pool(name="resp", bufs=2) as resp, \
                 tc.tile_pool(name="lnp", bufs=2) as lnp, \
                 tc.tile_pool(name="psD", bufs=2, space="PSUM") as psD:
                wo_sb = dpool.tile([P, EC, D], bf16)
                nc.sync.dma_start(wo_sb[:], wo.rearrange("(o p) e -> p o e", p=P))
                gam_sb = dpool.tile([P, D], f32)
                nc.sync.dma_start(gam_sb[:], gam[:])
                bet_sb = dpool.tile([P, D], f32)
                nc.sync.dma_start(bet_sb[:], bet[:])
                ctxT_sb = dpool.tile([P, EC, M], bf16)  # summed ctx^T own q
                nc.gpsimd.collective_compute(
                    "ReduceScatter",
                    mybir.AluOpType.add,
                    replica_groups=rg,
                    ins=[ctx_cc_in.opt()],
                    outs=[ctx_cc_out.opt()],
                )
                ccv = ctx_cc_out.rearrange("(o p) q -> p o q", p=P)
                for vc in range(EC):
                    nc.sync.dma_start(ctxT_sb[:, vc, :], ccv[:, vc, :])
                for qc in range(M // P):
                    rest = resp.tile([P, D], f32, tag="res")
                    nc.sync.dma_start(rest[:], res_view[qc])
                    ps = psD.tile([P, D], f32, tag="o")
                    for half in range(2):
                        for vc in range(EC):
                            nc.tensor.matmul(
                                ps[:, half * M : (half + 1) * M],
                                lhsT=ctxT_sb[:, vc, qc * P : (qc + 1) * P],
                                rhs=wo_sb[:, vc, half * M : (half + 1) * M],
                                start=(vc == 0),
                                stop=(vc == EC - 1),
                            )
                    xsb = lnp.tile([P, D], f32, tag="x")
                    nc.vector.tensor_tensor(xsb[:], ps[:], rest[:], OP.add)

                    # mean/var in one DVE pass: bn_stats over 2 chunks of 512
                    bst = smal.tile([P, 2, 6], f32, tag="bst")
                    nc.vector.bn_stats(bst[:, 0, :], xsb[:, 0:M])
                    nc.vector.bn_stats(bst[:, 1, :], xsb[:, M:D])
                    agg = smal.tile([P, 2], f32, tag="agg")
                    nc.vector.bn_aggr(agg[:], bst[:])
                    veps = smal.tile([P, 1], f32, tag="veps")
                    nc.vector.tensor_scalar(
                        veps[:], agg[:, 1:2], 1.0, LN_EPS, OP.mult, OP.add
                    )
                    std = smal.tile([P, 1], f32, tag="std")
                    nc.scalar.activation(std[:], veps[:], AF.Sqrt)
                    inv = smal.tile([P, 1], f32, tag="inv")
                    nc.vector.reciprocal(inv[:], std[:])
                    # xn = (x - mu) * inv ; then *gamma on DVE, +beta on Pool
                    nc.vector.tensor_scalar(
                        xsb[:], xsb[:], agg[:, 0:1], inv[:], OP.subtract, OP.mult
                    )
                    nc.vector.tensor_tensor(xsb[:], xsb[:], gam_sb[:], OP.mult)
                    ot = lnp.tile([P, D], f32, tag="ot")
                    nc.gpsimd.tensor_tensor(ot[:], xsb[:], bet_sb[:], OP.add)
                    nc.sync.dma_start(out_view[qc], ot[:])

    nc.compile()
    return nc


def _get_nc():
    if "nc" not in _CACHE:
        _CACHE["nc"] = _build()
    return _CACHE["nc"]


def _in_maps(input_Q, input_K, input_V, W_Q, W_K, W_V, W_O, ln_gamma, ln_beta):
    bf = ml_dtypes.bfloat16
    f32 = np.float32
    Q_ = np.asarray(input_Q, dtype=f32)
    K_ = np.asarray(input_K, dtype=f32)
    V_ = np.asarray(input_V, dtype=f32)
    wq_b = np.asarray(W_Q, dtype=f32).astype(bf)
    wk_b = np.asarray(W_K, dtype=f32).astype(bf)
    wv_b = np.asarray(W_V, dtype=f32).astype(bf)
    wo_b = np.asarray(W_O, dtype=f32).astype(bf)
    gam_b = np.ascontiguousarray(
        np.broadcast_to(np.asarray(ln_gamma, dtype=f32), (P, D))
    )
    bet_b = np.ascontiguousarray(
        np.broadcast_to(np.asarray(ln_beta, dtype=f32), (P, D))
    )
    maps = []
    for c in range(8):
        b, r = divmod(c, G)
        sl = slice(r * M, (r + 1) * M)
        maps.append(
            {
                "xqT": np.ascontiguousarray(Q_[b].T).astype(bf),
                "xkT": np.ascontiguousarray(K_[b, sl].T).astype(bf),
                "xvT": np.ascontiguousarray(V_[b, sl].T).astype(bf),
                "xres": np.ascontiguousarray(Q_[b, sl]),
                "wq": wq_b,
                "wk": wk_b,
                "wv": wv_b,
                "wo": wo_b,
                "gam": gam_b,
                "bet": bet_b,
            }
        )
    return maps


def _assemble(results):
    B = 2
    out = np.empty((B, SK, D), np.float32)
    for c in range(8):
        b, r = divmod(c, G)
        out[b, r * M : (r + 1) * M] = results[c]["out"]
    return out


def run_traced(trace=False, **inputs):
    """Run on HW; returns (output, BassKernelResults)."""
    from concourse.bass_utils import run_bass_kernel_spmd

    nc = _get_nc()
    maps = _in_maps(**inputs)
    res = run_bass_kernel_spmd(nc, maps, list(range(8)), trace=trace)
    return _assemble(res.results), res


def kernel(**inputs) -> np.ndarray:
    out, _ = run_traced(trace=False, **inputs)
    return out


# revision 6
# speedup vs baseline: 1.0688x; 1.0010x over previous
"""Trainium2 Bass kernel for nn_MultiHeadAttention (head-axis softmax quirk).

v2 strategy (8 NeuronCores, 2 batch-groups x 4 cores):
  - NO pre-attention collectives. Softmax over HEADS is local per (q,k), so
    attention is sharded over the K/V length: each core projects its own
    512-row k-slice of K and V, and redundantly projects Q for the whole
    group's 2048 q rows (+41us PE, but saves ~240us of AllGather).
  - Per core: scores^T [k,q] tiles (128k x 512q x 16h), exp on ScalarE,
    head-sum via DVE tree-adds (bf16 4x mode), normalize split DVE/GpSimd,
    context accumulated across the 4 local k-chunks in PSUM chains (no
    SBUF accumulation traffic).
  - One bf16 ReduceScatter of ctx^T partials (4MB in -> 1MB out) hands each
    core the k-summed context for its own 512 q rows.
  - Output projection + residual + LayerNorm on own q rows.
  - All matmuls bf16 with fp32 PSUM accumulation.
"""

import numpy as np
import ml_dtypes

D = 1024
H = 16
DK = 64
P = 128
M = 512  # q rows per core (own slice) / k rows per core
SK = 2048
G = 4  # cores per batch group
GQ = 2048  # q rows per group
EC = D // P  # 8
HP = H // 2  # 8
KC = M // P  # 4 local k chunks
QB = GQ // M  # 4 q blocks (one per destination core)
LN_EPS = 1e-5

_CACHE = {}


def _build():
    import concourse.bass as bass
    import concourse.mybir as mybir
    import concourse.tile as tile
    from concourse import bacc

    f32 = mybir.dt.float32
    bf16 = mybir.dt.bfloat16
    AF = mybir.ActivationFunctionType
    OP = mybir.AluOpType
    AX = mybir.AxisListType

    nc = bacc.Bacc("TRN2", target_bir_lowering=False, debug=False, num_devices=8)

    xqT = nc.dram_tensor("xqT", [D, GQ], bf16, kind="ExternalInput").ap()
    xkT = nc.dram_tensor("xkT", [D, M], bf16, kind="ExternalInput").ap()
    xvT = nc.dram_tensor("xvT", [D, M], bf16, kind="ExternalInput").ap()
    xres = nc.dram_tensor("xres", [M, D], f32, kind="ExternalInput").ap()
    wq = nc.dram_tensor("wq", [D, D], bf16, kind="ExternalInput").ap()
    wk = nc.dram_tensor("wk", [D, D], bf16, kind="ExternalInput").ap()
    wv = nc.dram_tensor("wv", [D, D], bf16, kind="ExternalInput").ap()
    wo = nc.dram_tensor("wo", [D, D], bf16, kind="ExternalInput").ap()
    gam = nc.dram_tensor("gam", [P, D], f32, kind="ExternalInput").ap()
    bet = nc.dram_tensor("bet", [P, D], f32, kind="ExternalInput").ap()
    out = nc.dram_tensor("out", [M, D], f32, kind="ExternalOutput").ap()

    rg = [[0, 1, 2, 3], [4, 5, 6, 7]]

    from contextlib import ExitStack

    with tile.TileContext(nc) as tc:
        with ExitStack() as ctx:
            const = ctx.enter_context(tc.tile_pool(name="const", bufs=1))
            smal = ctx.enter_context(tc.tile_pool(name="smal", bufs=2))
            dram = ctx.enter_context(tc.tile_pool(name="dram", bufs=1, space="DRAM"))

            # persistent tiles
            kT_sb = const.tile([P, HP, M], bf16)  # K^T own slice, dk-pair packed
            vt_sb = const.tile([P, KC, D], bf16)  # V own slice [k-chunk, v]
            qT_sb = const.tile([P, HP, GQ], bf16)  # Q^T whole group

            ctx_cc_in = dram.tile([QB * D, M], bf16)
            ctx_cc_out = dram.tile([D, M], bf16)

            # ---- Phase A: K^T, V, and first-quarter Q^T projections ----
            # Q quarters 1-3 are emitted inside Phase C where PE has idle
            # slots (C is Activation-paced); wq/xq pools outlive Phase A.
            wq_p = ctx.enter_context(tc.tile_pool(name="wq_p", bufs=1))
            xq_p = ctx.enter_context(tc.tile_pool(name="xq_p", bufs=2))
            wq_sb = wq_p.tile([P, EC, D], bf16)
            xqv = xqT.rearrange("(o p) q -> p o q", p=P)

            def emit_qproj_qu(qu, pool, tag, evac_dve):
                xq_qu = xq_p.tile([P, EC, M], bf16, tag="xq")
                nc.sync.dma_start(xq_qu[:], xqv[:, :, qu * M : (qu + 1) * M])
                for hp in range(HP):
                    ps = pool.tile([P, M] if tag == "cx" else [P, GQ], f32, tag=tag)
                    for dc in range(EC):
                        nc.tensor.matmul(
                            ps[:, 0:M],
                            lhsT=wq_sb[:, dc, hp * P : (hp + 1) * P],
                            rhs=xq_qu[:, dc, :],
                            start=(dc == 0),
                            stop=(dc == EC - 1),
                        )
                    dst = qT_sb[:, hp, qu * M : (qu + 1) * M]
                    if evac_dve:
                        nc.vector.tensor_copy(dst, ps[:, 0:M])
                    else:
                        nc.scalar.copy(dst, ps[:, 0:M])

            with tc.tile_pool(name="wpool", bufs=2) as wpool, \
                 tc.tile_pool(name="xk_p", bufs=1) as xk_p, \
                 tc.tile_pool(name="xv_p", bufs=1) as xv_p, \
                 tc.tile_pool(name="psA", bufs=2, space="PSUM") as psA:
                wk_sb = wpool.tile([P, EC, D], bf16, tag="w")
                nc.sync.dma_start(wk_sb[:], wk.rearrange("(o p) e -> p o e", p=P))
                xk_sb = xk_p.tile([P, EC, M], bf16)
                nc.sync.dma_start(xk_sb[:], xkT.rearrange("(o p) q -> p o q", p=P))
                wv_sb = wpool.tile([P, EC, D], bf16, tag="w")
                nc.sync.dma_start(wv_sb[:], wv.rearrange("(o p) e -> p o e", p=P))
                xv_sb = xv_p.tile([P, EC, M], bf16)
                nc.sync.dma_start(xv_sb[:], xvT.rearrange("(o p) q -> p o q", p=P))
                nc.sync.dma_start(wq_sb[:], wq.rearrange("(o p) e -> p o e", p=P))

                # K^T projection: own 512 k rows -> kT_sb [P, hp, 512]
                for w0 in range(0, HP, 3):
                    hps = list(range(w0, min(w0 + 3, HP)))
                    n = len(hps)
                    ps = psA.tile([P, GQ], f32, tag="a")
                    for j, hp in enumerate(hps):
                        for dc in range(EC):
                            nc.tensor.matmul(
                                ps[:, j * M : (j + 1) * M],
                                lhsT=wk_sb[:, dc, hp * P : (hp + 1) * P],
                                rhs=xk_sb[:, dc, :],
                                start=(dc == 0),
                                stop=(dc == EC - 1),
                            )
                    nc.scalar.copy(kT_sb[:, w0 : w0 + n, :], ps[:, : n * M])

                # V projection: own 512 k rows -> vt_sb [P(k), kc, 1024(v)]
                for kc in range(KC):
                    ps = psA.tile([P, GQ], f32, tag="a")
                    for half in range(2):
                        for dc in range(EC):
                            nc.tensor.matmul(
                                ps[:, half * M : (half + 1) * M],
                                lhsT=xv_sb[:, dc, kc * P : (kc + 1) * P],
                                rhs=wv_sb[:, dc, half * M : (half + 1) * M],
                                start=(dc == 0),
                                stop=(dc == EC - 1),
                            )
                    nc.scalar.copy(vt_sb[:, kc, :], ps[:, :D])

                # Q^T projection, first quarter (covers q-block 0)
                emit_qproj_qu(0, psA, "a", evac_dve=False)

            # ---- Phase C: attention tiles + ctx PSUM chains + partial out ----
            # Software-pipelined: the 8 ctx chains of q-block qb interleave
            # with the first score/exp tiles of qb+1 so neither PE nor the
            # Activation engine idles across the qb boundary.
            cc_view = ctx_cc_in.rearrange("(b o p) q -> p b o q", b=QB, p=P)
            with tc.tile_pool(name="etp", bufs=6) as etp, \
                 tc.tile_pool(name="t8p", bufs=1) as t8p, \
                 tc.tile_pool(name="stg", bufs=3) as stg, \
                 tc.tile_pool(name="psC", bufs=2, space="PSUM") as psC:

                def emit_tile(qb, kc):
                    et = etp.tile([P, H, M], bf16, tag="et")
                    # scores + exp, 3-head waves; head-sum accumulates wave
                    # partials as exps land so `den` is ready ~1 op after the
                    # last wave (keeps ctx chains from stalling on the tree).
                    par = t8p.tile([P, 8, M], bf16, tag="t8")
                    for wi, w0 in enumerate(range(0, H, 3)):
                        hs = list(range(w0, min(w0 + 3, H)))
                        n = len(hs)
                        ps = psC.tile([P, 3 * M], f32, tag="sc")
                        for j, h in enumerate(hs):
                            hp, half = divmod(h, 2)
                            pb = half * DK
                            nc.tensor.matmul(
                                ps[:, j * M : (j + 1) * M],
                                lhsT=kT_sb[pb : pb + DK, hp, kc * P : (kc + 1) * P],
                                rhs=qT_sb[pb : pb + DK, hp, qb * M : (qb + 1) * M],
                                start=True,
                                stop=True,
                            )
                        nc.scalar.activation(
                            et[:, w0 : w0 + n, :], ps[:, : n * M], AF.Exp,
                            scale=0.125,
                        )
                        if n == 3:
                            nc.vector.tensor_tensor(
                                par[:, wi, :], et[:, w0, :], et[:, w0 + 1, :], OP.add
                            )
                            nc.vector.tensor_tensor(
                                par[:, wi, :], par[:, wi, :], et[:, w0 + 2, :], OP.add
                            )
                        if wi == 1:
                            nc.vector.tensor_tensor(
                                par[:, 6, :], par[:, 0, :], par[:, 1, :], OP.add
                            )
                        if wi == 3:
                            nc.vector.tensor_tensor(
                                par[:, 7, :], par[:, 2, :], par[:, 3, :], OP.add
                            )
                    # waves: 5x3 heads + 1 single head (h15)
                    nc.vector.tensor_tensor(
                        par[:, 6, :], par[:, 6, :], par[:, 7, :], OP.add
                    )
                    nc.vector.tensor_tensor(
                        par[:, 4, :], par[:, 4, :], et[:, 15, :], OP.add
                    )
                    den = smal.tile([P, M], f32, tag="den")
                    nc.vector.tensor_tensor(den[:], par[:, 6, :], par[:, 4, :], OP.add)
                    rf = smal.tile([P, M], f32, tag="rf")
                    nc.vector.reciprocal_approx_fast(rf[:], den[:])
                    rb = smal.tile([P, M], bf16, tag="rb")
                    nc.vector.tensor_copy(rb[:], rf[:])
                    # normalize: attn = e * r (in place; split DVE/GpSimd)
                    nsp = 8
                    nc.vector.tensor_tensor(
                        et[:, :nsp, :],
                        et[:, :nsp, :],
                        rb[:, None, :].to_broadcast((P, nsp, M)),
                        OP.mult,
                    )
                    nc.gpsimd.tensor_tensor(
                        et[:, nsp:, :],
                        et[:, nsp:, :],
                        rb[:, None, :].to_broadcast((P, H - nsp, M)),
                        OP.mult,
                    )
                    return et

                def emit_chain(qb, hp, ets):
                    cps = psC.tile([P, M], f32, tag="cx")
                    for kc in range(KC):
                        nc.tensor.matmul(
                            cps[0:DK, :],
                            lhsT=vt_sb[:, kc, (2 * hp) * DK : (2 * hp + 1) * DK],
                            rhs=ets[kc][:, 2 * hp, :],
                            start=(kc == 0),
                            stop=(kc == KC - 1),
                            tile_position=(0, 0),
                        )
                        nc.tensor.matmul(
                            cps[DK:P, :],
                            lhsT=vt_sb[:, kc, (2 * hp + 1) * DK : (2 * hp + 2) * DK],
                            rhs=ets[kc][:, 2 * hp + 1, :],
                            start=(kc == 0),
                            stop=(kc == KC - 1),
                            tile_position=(0, DK),
                        )
                    ost = stg.tile([P, M], bf16, tag="ost")
                    if hp % 8 < 5:
                        nc.scalar.copy(ost[:], cps[:])
                    else:
                        nc.vector.tensor_copy(ost[:], cps[:])
                    nc.sync.dma_start(cc_view[:, qb, hp, :], ost[:])

                # Prologue: q-block 0 tiles interleaved with Q^T quarters 1-3
                # (those matmuls fill PE slots while Act runs the exps; their
                # chains borrow the idle "cx" PSUM tag, evacs go to DVE).
                ets_cur = [emit_tile(0, 0)]
                emit_qproj_qu(1, psC, "cx", evac_dve=True)
                ets_cur.append(emit_tile(0, 1))
                emit_qproj_qu(2, psC, "cx", evac_dve=False)
                ets_cur.append(emit_tile(0, 2))
                emit_qproj_qu(3, psC, "cx", evac_dve=True)
                ets_cur.append(emit_tile(0, 3))
                for qb in range(QB):
                    ets_next = []
                    for hp in range(HP):
                        emit_chain(qb, hp, ets_cur)
                        # 3-tile lookahead into qb+1 (etp bufs=7 allows it)
                        if qb + 1 < QB and hp in (1, 3, 5):
                            ets_next.append(emit_tile(qb + 1, len(ets_next)))
                    if qb + 1 < QB:
                        while len(ets_next) < KC:
                            ets_next.append(emit_tile(qb + 1, len(ets_next)))
                    ets_cur = ets_next

            # ---- ReduceScatter + Phase D: O-proj + residual + LayerNorm ----
            res_view = xres.rearrange("(o p) e -> o p e", p=P)
            out_view = out.rearrange("(o p) e -> o p e", p=P)
            with tc.tile_pool(name="dpool", bufs=1) as dpool, \
                 tc.tile_pool(name="resp", bufs=4) as resp, \
                 tc.tile_pool(name="lnp", bufs=2) as lnp, \
                 tc.tile_pool(name="psD", bufs=2, space="PSUM") as psD:
                wo_sb = dpool.tile([P, EC, D], bf16)
                nc.sync.dma_start(wo_sb[:], wo.rearrange("(o p) e -> p o e", p=P))
                gam_sb = dpool.tile([P, D], f32)
                nc.sync.dma_start(gam_sb[:], gam[:])
                bet_sb = dpool.tile([P, D], f32)
                nc.sync.dma_start(bet_sb[:], bet[:])
                ctxT_sb = dpool.tile([P, EC, M], bf16)  # summed ctx^T own q
                nc.gpsimd.collective_compute(
                    "ReduceScatter",
                    mybir.AluOpType.add,
                    replica_groups=rg,
                    ins=[ctx_cc_in.opt()],
                    outs=[ctx_cc_out.opt()],
                )
                ccv = ctx_cc_out.rearrange("(o p) q -> p o q", p=P)
                for vc in range(EC):
                    nc.sync.dma_start(ctxT_sb[:, vc, :], ccv[:, vc, :])
                rests = []
                for qc in range(M // P):
                    rest = resp.tile([P, D], f32, tag="res")
                    nc.sync.dma_start(rest[:], res_view[qc])
                    rests.append(rest)
                for qc in range(M // P):
                    rest = rests[qc]
                    ps = psD.tile([P, D], f32, tag="o")
                    for half in range(2):
                        for vc in range(EC):
                            nc.tensor.matmul(
                                ps[:, half * M : (half + 1) * M],
                                lhsT=ctxT_sb[:, vc, qc * P : (qc + 1) * P],
                                rhs=wo_sb[:, vc, half * M : (half + 1) * M],
                                start=(vc == 0),
                                stop=(vc == EC - 1),
                            )
                    xsb = lnp.tile([P, D], f32, tag="x")
                    nc.vector.tensor_tensor(xsb[:], ps[:], rest[:], OP.add)

                    # mean/var in one DVE pass: bn_stats over 2 chunks of 512
                    bst = smal.tile([P, 2, 6], f32, tag="bst")
                    nc.vector.bn_stats(bst[:, 0, :], xsb[:, 0:M])
                    nc.vector.bn_stats(bst[:, 1, :], xsb[:, M:D])
                    agg = smal.tile([P, 2], f32, tag="agg")
                    nc.vector.bn_aggr(agg[:], bst[:])
                    veps = smal.tile([P, 1], f32, tag="veps")
                    nc.vector.tensor_scalar(
                        veps[:], agg[:, 1:2], 1.0, LN_EPS, OP.mult, OP.add
                    )
                    std = smal.tile([P, 1], f32, tag="std")
                    nc.scalar.activation(std[:], veps[:], AF.Sqrt)
                    inv = smal.tile([P, 1], f32, tag="inv")
                    nc.vector.reciprocal(inv[:], std[:])
                    # xn = (x - mu) * inv ; then *gamma on DVE, +beta on Pool
                    nc.vector.tensor_scalar(
                        xsb[:], xsb[:], agg[:, 0:1], inv[:], OP.subtract, OP.mult
                    )
                    nc.vector.tensor_tensor(xsb[:], xsb[:], gam_sb[:], OP.mult)
                    ot = lnp.tile([P, D], f32, tag="ot")
                    nc.gpsimd.tensor_tensor(ot[:], xsb[:], bet_sb[:], OP.add)
                    nc.sync.dma_start(out_view[qc], ot[:])

    nc.compile()
    return nc


def _get_nc():
    if "nc" not in _CACHE:
        _CACHE["nc"] = _build()
    return _CACHE["nc"]


def _in_maps(input_Q, input_K, input_V, W_Q, W_K, W_V, W_O, ln_gamma, ln_beta):
    bf = ml_dtypes.bfloat16
    f32 = np.float32
    Q_ = np.asarray(input_Q, dtype=f32)
    K_ = np.asarray(input_K, dtype=f32)
    V_ = np.asarray(input_V, dtype=f32)
    wq_b = np.asarray(W_Q, dtype=f32).astype(bf)
    wk_b = np.asarray(W_K, dtype=f32).astype(bf)
    wv_b = np.asarray(W_V, dtype=f32).astype(bf)
    wo_b = np.asarray(W_O, dtype=f32).astype(bf)
    gam_b = np.ascontiguousarray(
        np.broadcast_to(np.asarray(ln_gamma, dtype=f32), (P, D))
    )
    bet_b = np.ascontiguousarray(
        np.broadcast_to(np.asarray(ln_beta, dtype=f32), (P, D))
    )
    maps = []
    for c in range(8):
        b, r = divmod(c, G)
        sl = slice(r * M, (r + 1) * M)
        maps.append(
            {
                "xqT": np.ascontiguousarray(Q_[b].T).astype(bf),
                "xkT": np.ascontiguousarray(K_[b, sl].T).astype(bf),
                "xvT": np.ascontiguousarray(V_[b, sl].T).astype(bf),
                "xres": np.ascontiguousarray(Q_[b, sl]),
                "wq": wq_b,
                "wk": wk_b,
                "wv": wv_b,
                "wo": wo_b,
                "gam": gam_b,
                "bet": bet_b,
            }
        )
    return maps


def _assemble(results):
    B = 2
    out = np.empty((B, SK, D), np.float32)
    for c in range(8):
        b, r = divmod(c, G)
        out[b, r * M : (r + 1) * M] = results[c]["out"]
    return out


def run_traced(trace=False, **inputs):
    """Run on HW; returns (output, BassKernelResults)."""
    from concourse.bass_utils import run_bass_kernel_spmd

    nc = _get_nc()
    maps = _in_maps(**inputs)
    res = run_bass_kernel_spmd(nc, maps, list(range(8)), trace=trace)
    return _assemble(res.results), res


def kernel(**inputs) -> np.ndarray:
    out, _ = run_traced(trace=False, **inputs)
    return out


# revision 7
# speedup vs baseline: 1.0836x; 1.0139x over previous
"""Trainium2 Bass kernel for nn_MultiHeadAttention (head-axis softmax quirk).

v2 strategy (8 NeuronCores, 2 batch-groups x 4 cores):
  - NO pre-attention collectives. Softmax over HEADS is local per (q,k), so
    attention is sharded over the K/V length: each core projects its own
    512-row k-slice of K and V, and redundantly projects Q for the whole
    group's 2048 q rows (+41us PE, but saves ~240us of AllGather).
  - Per core: scores^T [k,q] tiles (128k x 512q x 16h), exp on ScalarE,
    head-sum via DVE tree-adds (bf16 4x mode), normalize split DVE/GpSimd,
    context accumulated across the 4 local k-chunks in PSUM chains (no
    SBUF accumulation traffic).
  - One bf16 ReduceScatter of ctx^T partials (4MB in -> 1MB out) hands each
    core the k-summed context for its own 512 q rows.
  - Output projection + residual + LayerNorm on own q rows.
  - All matmuls bf16 with fp32 PSUM accumulation.
"""

import numpy as np
import ml_dtypes

D = 1024
H = 16
DK = 64
P = 128
M = 512  # q rows per core (own slice) / k rows per core
SK = 2048
G = 4  # cores per batch group
GQ = 2048  # q rows per group
EC = D // P  # 8
HP = H // 2  # 8
KC = M // P  # 4 local k chunks
QB = GQ // M  # 4 q blocks (one per destination core)
LN_EPS = 1e-5

_CACHE = {}


def _build():
    import concourse.bass as bass
    import concourse.mybir as mybir
    import concourse.tile as tile
    from concourse import bacc

    f32 = mybir.dt.float32
    bf16 = mybir.dt.bfloat16
    AF = mybir.ActivationFunctionType
    OP = mybir.AluOpType
    AX = mybir.AxisListType

    nc = bacc.Bacc("TRN2", target_bir_lowering=False, debug=False, num_devices=8)

    xqT = nc.dram_tensor("xqT", [D, GQ], bf16, kind="ExternalInput").ap()
    xkT = nc.dram_tensor("xkT", [D, M], bf16, kind="ExternalInput").ap()
    xvT = nc.dram_tensor("xvT", [D, M], bf16, kind="ExternalInput").ap()
    xres = nc.dram_tensor("xres", [M, D], f32, kind="ExternalInput").ap()
    wq = nc.dram_tensor("wq", [D, D], bf16, kind="ExternalInput").ap()
    wk = nc.dram_tensor("wk", [D, D], bf16, kind="ExternalInput").ap()
    wv = nc.dram_tensor("wv", [D, D], bf16, kind="ExternalInput").ap()
    wo = nc.dram_tensor("wo", [D, D], bf16, kind="ExternalInput").ap()
    gam = nc.dram_tensor("gam", [P, D], f32, kind="ExternalInput").ap()
    bet = nc.dram_tensor("bet", [P, D], f32, kind="ExternalInput").ap()
    out = nc.dram_tensor("out", [M, D], f32, kind="ExternalOutput").ap()

    rg = [[0, 1, 2, 3], [4, 5, 6, 7]]

    from contextlib import ExitStack

    with tile.TileContext(nc) as tc:
        with ExitStack() as ctx:
            const = ctx.enter_context(tc.tile_pool(name="const", bufs=1))
            smal = ctx.enter_context(tc.tile_pool(name="smal", bufs=2))
            dram = ctx.enter_context(tc.tile_pool(name="dram", bufs=1, space="DRAM"))

            # persistent tiles
            kT_sb = const.tile([P, HP, M], bf16)  # K^T own slice, dk-pair packed
            vt_sb = const.tile([P, KC, D], bf16)  # V own slice [k-chunk, v]
            qT_sb = const.tile([P, HP, GQ], bf16)  # Q^T whole group

            ctx_cc_in = dram.tile([QB * D, M], bf16)
            ctx_cc_out = dram.tile([D, M], bf16)

            # ---- Phase A: K^T, V, and first-quarter Q^T projections ----
            # Q quarters 1-3 are emitted inside Phase C where PE has idle
            # slots (C is Activation-paced); wq/xq pools outlive Phase A.
            wq_p = ctx.enter_context(tc.tile_pool(name="wq_p", bufs=1))
            xq_p = ctx.enter_context(tc.tile_pool(name="xq_p", bufs=2))
            wq_sb = wq_p.tile([P, EC, D], bf16)
            xqv = xqT.rearrange("(o p) q -> p o q", p=P)

            def emit_qproj_qu(qu, pool, tag, evac_dve):
                xq_qu = xq_p.tile([P, EC, M], bf16, tag="xq")
                nc.sync.dma_start(xq_qu[:], xqv[:, :, qu * M : (qu + 1) * M])
                for hp in range(HP):
                    ps = pool.tile([P, M] if tag == "cx" else [P, GQ], f32, tag=tag)
                    for dc in range(EC):
                        nc.tensor.matmul(
                            ps[:, 0:M],
                            lhsT=wq_sb[:, dc, hp * P : (hp + 1) * P],
                            rhs=xq_qu[:, dc, :],
                            start=(dc == 0),
                            stop=(dc == EC - 1),
                        )
                    dst = qT_sb[:, hp, qu * M : (qu + 1) * M]
                    if evac_dve:
                        nc.vector.tensor_copy(dst, ps[:, 0:M])
                    else:
                        nc.scalar.copy(dst, ps[:, 0:M])

            with tc.tile_pool(name="wpool", bufs=2) as wpool, \
                 tc.tile_pool(name="xk_p", bufs=1) as xk_p, \
                 tc.tile_pool(name="xv_p", bufs=1) as xv_p, \
                 tc.tile_pool(name="psA", bufs=2, space="PSUM") as psA:
                # Lead-in DMAs spread across engine DGE queues so wk/xk (the
                # Kproj gate) stream in parallel instead of serializing.
                wk_view = wk.rearrange("(o p) e -> p o e", p=P)
                wk_sb = wpool.tile([P, EC, D], bf16, tag="w")
                nc.sync.dma_start(wk_sb[:, 0:4, :], wk_view[:, 0:4, :])
                nc.scalar.dma_start(wk_sb[:, 4:EC, :], wk_view[:, 4:EC, :])
                xk_sb = xk_p.tile([P, EC, M], bf16)
                nc.gpsimd.dma_start(xk_sb[:], xkT.rearrange("(o p) q -> p o q", p=P))
                wv_sb = wpool.tile([P, EC, D], bf16, tag="w")
                nc.gpsimd.dma_start(wv_sb[:], wv.rearrange("(o p) e -> p o e", p=P))
                xv_sb = xv_p.tile([P, EC, M], bf16)
                nc.sync.dma_start(xv_sb[:], xvT.rearrange("(o p) q -> p o q", p=P))
                nc.sync.dma_start(wq_sb[:], wq.rearrange("(o p) e -> p o e", p=P))

                # K^T projection: own 512 k rows -> kT_sb [P, hp, 512]
                for w0 in range(0, HP, 3):
                    hps = list(range(w0, min(w0 + 3, HP)))
                    n = len(hps)
                    ps = psA.tile([P, GQ], f32, tag="a")
                    for j, hp in enumerate(hps):
                        for dc in range(EC):
                            nc.tensor.matmul(
                                ps[:, j * M : (j + 1) * M],
                                lhsT=wk_sb[:, dc, hp * P : (hp + 1) * P],
                                rhs=xk_sb[:, dc, :],
                                start=(dc == 0),
                                stop=(dc == EC - 1),
                            )
                    nc.scalar.copy(kT_sb[:, w0 : w0 + n, :], ps[:, : n * M])

                # V projection: own 512 k rows -> vt_sb [P(k), kc, 1024(v)]
                for kc in range(KC):
                    ps = psA.tile([P, GQ], f32, tag="a")
                    for half in range(2):
                        for dc in range(EC):
                            nc.tensor.matmul(
                                ps[:, half * M : (half + 1) * M],
                                lhsT=xv_sb[:, dc, kc * P : (kc + 1) * P],
                                rhs=wv_sb[:, dc, half * M : (half + 1) * M],
                                start=(dc == 0),
                                stop=(dc == EC - 1),
                            )
                    nc.scalar.copy(vt_sb[:, kc, :], ps[:, :D])

                # Q^T projection, first quarter (covers q-block 0)
                emit_qproj_qu(0, psA, "a", evac_dve=False)

            # ---- Phase C: attention tiles + ctx PSUM chains + partial out ----
            # Software-pipelined: the 8 ctx chains of q-block qb interleave
            # with the first score/exp tiles of qb+1 so neither PE nor the
            # Activation engine idles across the qb boundary.
            cc_view = ctx_cc_in.rearrange("(b o p) q -> p b o q", b=QB, p=P)
            with tc.tile_pool(name="etp", bufs=6) as etp, \
                 tc.tile_pool(name="t8p", bufs=1) as t8p, \
                 tc.tile_pool(name="stg", bufs=3) as stg, \
                 tc.tile_pool(name="psC", bufs=2, space="PSUM") as psC:

                def emit_tile(qb, kc):
                    et = etp.tile([P, H, M], bf16, tag="et")
                    # scores + exp, 3-head waves; head-sum accumulates wave
                    # partials as exps land so `den` is ready ~1 op after the
                    # last wave (keeps ctx chains from stalling on the tree).
                    par = t8p.tile([P, 8, M], bf16, tag="t8")
                    for wi, w0 in enumerate(range(0, H, 3)):
                        hs = list(range(w0, min(w0 + 3, H)))
                        n = len(hs)
                        ps = psC.tile([P, 3 * M], f32, tag="sc")
                        for j, h in enumerate(hs):
                            hp, half = divmod(h, 2)
                            pb = half * DK
                            nc.tensor.matmul(
                                ps[:, j * M : (j + 1) * M],
                                lhsT=kT_sb[pb : pb + DK, hp, kc * P : (kc + 1) * P],
                                rhs=qT_sb[pb : pb + DK, hp, qb * M : (qb + 1) * M],
                                start=True,
                                stop=True,
                            )
                        nc.scalar.activation(
                            et[:, w0 : w0 + n, :], ps[:, : n * M], AF.Exp,
                            scale=0.125,
                        )
                        if n == 3:
                            nc.vector.tensor_tensor(
                                par[:, wi, :], et[:, w0, :], et[:, w0 + 1, :], OP.add
                            )
                            nc.vector.tensor_tensor(
                                par[:, wi, :], par[:, wi, :], et[:, w0 + 2, :], OP.add
                            )
                        if wi == 1:
                            nc.vector.tensor_tensor(
                                par[:, 6, :], par[:, 0, :], par[:, 1, :], OP.add
                            )
                        if wi == 3:
                            nc.vector.tensor_tensor(
                                par[:, 7, :], par[:, 2, :], par[:, 3, :], OP.add
                            )
                    # waves: 5x3 heads + 1 single head (h15)
                    nc.vector.tensor_tensor(
                        par[:, 6, :], par[:, 6, :], par[:, 7, :], OP.add
                    )
                    nc.vector.tensor_tensor(
                        par[:, 4, :], par[:, 4, :], et[:, 15, :], OP.add
                    )
                    den = smal.tile([P, M], f32, tag="den")
                    nc.vector.tensor_tensor(den[:], par[:, 6, :], par[:, 4, :], OP.add)
                    rf = smal.tile([P, M], f32, tag="rf")
                    nc.vector.reciprocal_approx_fast(rf[:], den[:])
                    rb = smal.tile([P, M], bf16, tag="rb")
                    nc.vector.tensor_copy(rb[:], rf[:])
                    # normalize: attn = e * r (in place; split DVE/GpSimd)
                    nsp = 8
                    nc.vector.tensor_tensor(
                        et[:, :nsp, :],
                        et[:, :nsp, :],
                        rb[:, None, :].to_broadcast((P, nsp, M)),
                        OP.mult,
                    )
                    nc.gpsimd.tensor_tensor(
                        et[:, nsp:, :],
                        et[:, nsp:, :],
                        rb[:, None, :].to_broadcast((P, H - nsp, M)),
                        OP.mult,
                    )
                    return et

                def emit_chain(qb, hp, ets):
                    cps = psC.tile([P, M], f32, tag="cx")
                    for kc in range(KC):
                        nc.tensor.matmul(
                            cps[0:DK, :],
                            lhsT=vt_sb[:, kc, (2 * hp) * DK : (2 * hp + 1) * DK],
                            rhs=ets[kc][:, 2 * hp, :],
                            start=(kc == 0),
                            stop=(kc == KC - 1),
                            tile_position=(0, 0),
                        )
                        nc.tensor.matmul(
                            cps[DK:P, :],
                            lhsT=vt_sb[:, kc, (2 * hp + 1) * DK : (2 * hp + 2) * DK],
                            rhs=ets[kc][:, 2 * hp + 1, :],
                            start=(kc == 0),
                            stop=(kc == KC - 1),
                            tile_position=(0, DK),
                        )
                    ost = stg.tile([P, M], bf16, tag="ost")
                    if hp % 8 < 5:
                        nc.scalar.copy(ost[:], cps[:])
                    else:
                        nc.vector.tensor_copy(ost[:], cps[:])
                    nc.sync.dma_start(cc_view[:, qb, hp, :], ost[:])

                # Prologue: q-block 0 tiles interleaved with Q^T quarters 1-3
                # (those matmuls fill PE slots while Act runs the exps; their
                # chains borrow the idle "cx" PSUM tag, evacs go to DVE).
                ets_cur = [emit_tile(0, 0)]
                emit_qproj_qu(1, psC, "cx", evac_dve=True)
                ets_cur.append(emit_tile(0, 1))
                emit_qproj_qu(2, psC, "cx", evac_dve=False)
                ets_cur.append(emit_tile(0, 2))
                emit_qproj_qu(3, psC, "cx", evac_dve=True)
                ets_cur.append(emit_tile(0, 3))
                for qb in range(QB):
                    ets_next = []
                    for hp in range(HP):
                        emit_chain(qb, hp, ets_cur)
                        # 3-tile lookahead into qb+1 (etp bufs=7 allows it)
                        if qb + 1 < QB and hp in (1, 3, 5):
                            ets_next.append(emit_tile(qb + 1, len(ets_next)))
                    if qb + 1 < QB:
                        while len(ets_next) < KC:
                            ets_next.append(emit_tile(qb + 1, len(ets_next)))
                    ets_cur = ets_next

            # ---- ReduceScatter + Phase D: O-proj + residual + LayerNorm ----
            res_view = xres.rearrange("(o p) e -> o p e", p=P)
            out_view = out.rearrange("(o p) e -> o p e", p=P)
            with tc.tile_pool(name="dpool", bufs=1) as dpool, \
                 tc.tile_pool(name="resp", bufs=4) as resp, \
                 tc.tile_pool(name="lnp", bufs=2) as lnp, \
                 tc.tile_pool(name="psD", bufs=2, space="PSUM") as psD:
                wo_sb = dpool.tile([P, EC, D], bf16)
                nc.sync.dma_start(wo_sb[:], wo.rearrange("(o p) e -> p o e", p=P))
                gam_sb = dpool.tile([P, D], f32)
                nc.sync.dma_start(gam_sb[:], gam[:])
                bet_sb = dpool.tile([P, D], f32)
                nc.sync.dma_start(bet_sb[:], bet[:])
                ctxT_sb = dpool.tile([P, EC, M], bf16)  # summed ctx^T own q
                nc.gpsimd.collective_compute(
                    "ReduceScatter",
                    mybir.AluOpType.add,
                    replica_groups=rg,
                    ins=[ctx_cc_in.opt()],
                    outs=[ctx_cc_out.opt()],
                )
                ccv = ctx_cc_out.rearrange("(o p) q -> p o q", p=P)
                for vc in range(EC):
                    nc.sync.dma_start(ctxT_sb[:, vc, :], ccv[:, vc, :])
                rests = []
                for qc in range(M // P):
                    rest = resp.tile([P, D], f32, tag="res")
                    nc.sync.dma_start(rest[:], res_view[qc])
                    rests.append(rest)
                for qc in range(M // P):
                    rest = rests[qc]
                    ps = psD.tile([P, D], f32, tag="o")
                    for half in range(2):
                        for vc in range(EC):
                            nc.tensor.matmul(
                                ps[:, half * M : (half + 1) * M],
                                lhsT=ctxT_sb[:, vc, qc * P : (qc + 1) * P],
                                rhs=wo_sb[:, vc, half * M : (half + 1) * M],
                                start=(vc == 0),
                                stop=(vc == EC - 1),
                            )
                    # residual add + stats in column halves so downstream
                    # normalize/output passes pipeline (shorter drain)
                    xsb = lnp.tile([P, D], f32, tag="x")
                    bst = smal.tile([P, 2, 6], f32, tag="bst")
                    for h in range(2):
                        nc.vector.tensor_tensor(
                            xsb[:, h * M : (h + 1) * M],
                            ps[:, h * M : (h + 1) * M],
                            rest[:, h * M : (h + 1) * M],
                            OP.add,
                        )
                        nc.vector.bn_stats(bst[:, h, :], xsb[:, h * M : (h + 1) * M])
                    agg = smal.tile([P, 2], f32, tag="agg")
                    nc.vector.bn_aggr(agg[:], bst[:])
                    veps = smal.tile([P, 1], f32, tag="veps")
                    nc.vector.tensor_scalar(
                        veps[:], agg[:, 1:2], 1.0, LN_EPS, OP.mult, OP.add
                    )
                    std = smal.tile([P, 1], f32, tag="std")
                    nc.scalar.activation(std[:], veps[:], AF.Sqrt)
                    inv = smal.tile([P, 1], f32, tag="inv")
                    nc.vector.reciprocal(inv[:], std[:])
                    # xn = (x - mu) * inv ; *gamma on DVE, +beta on Pool;
                    # column halves pipeline into the output DMA
                    ot = lnp.tile([P, D], f32, tag="ot")
                    for h in range(2):
                        sl = slice(h * M, (h + 1) * M)
                        nc.vector.tensor_scalar(
                            xsb[:, sl], xsb[:, sl], agg[:, 0:1], inv[:],
                            OP.subtract, OP.mult,
                        )
                        nc.vector.tensor_tensor(
                            xsb[:, sl], xsb[:, sl], gam_sb[:, sl], OP.mult
                        )
                        nc.gpsimd.tensor_tensor(
                            ot[:, sl], xsb[:, sl], bet_sb[:, sl], OP.add
                        )
                        nc.sync.dma_start(out_view[qc][:, sl], ot[:, sl])

    nc.compile()
    return nc


def _get_nc():
    if "nc" not in _CACHE:
        _CACHE["nc"] = _build()
    return _CACHE["nc"]


def _in_maps(input_Q, input_K, input_V, W_Q, W_K, W_V, W_O, ln_gamma, ln_beta):
    bf = ml_dtypes.bfloat16
    f32 = np.float32
    Q_ = np.asarray(input_Q, dtype=f32)
    K_ = np.asarray(input_K, dtype=f32)
    V_ = np.asarray(input_V, dtype=f32)
    wq_b = np.asarray(W_Q, dtype=f32).astype(bf)
    wk_b = np.asarray(W_K, dtype=f32).astype(bf)
    wv_b = np.asarray(W_V, dtype=f32).astype(bf)
    wo_b = np.asarray(W_O, dtype=f32).astype(bf)
    gam_b = np.ascontiguousarray(
        np.broadcast_to(np.asarray(ln_gamma, dtype=f32), (P, D))
    )
    bet_b = np.ascontiguousarray(
        np.broadcast_to(np.asarray(ln_beta, dtype=f32), (P, D))
    )
    maps = []
    for c in range(8):
        b, r = divmod(c, G)
        sl = slice(r * M, (r + 1) * M)
        maps.append(
            {
                "xqT": np.ascontiguousarray(Q_[b].T).astype(bf),
                "xkT": np.ascontiguousarray(K_[b, sl].T).astype(bf),
                "xvT": np.ascontiguousarray(V_[b, sl].T).astype(bf),
                "xres": np.ascontiguousarray(Q_[b, sl]),
                "wq": wq_b,
                "wk": wk_b,
                "wv": wv_b,
                "wo": wo_b,
                "gam": gam_b,
                "bet": bet_b,
            }
        )
    return maps


def _assemble(results):
    B = 2
    out = np.empty((B, SK, D), np.float32)
    for c in range(8):
        b, r = divmod(c, G)
        out[b, r * M : (r + 1) * M] = results[c]["out"]
    return out


def run_traced(trace=False, **inputs):
    """Run on HW; returns (output, BassKernelResults)."""
    from concourse.bass_utils import run_bass_kernel_spmd

    nc = _get_nc()
    maps = _in_maps(**inputs)
    res = run_bass_kernel_spmd(nc, maps, list(range(8)), trace=trace)
    return _assemble(res.results), res


def kernel(**inputs) -> np.ndarray:
    out, _ = run_traced(trace=False, **inputs)
    return out


# revision 8
# speedup vs baseline: 1.0883x; 1.0043x over previous
"""Trainium2 Bass kernel for nn_MultiHeadAttention (head-axis softmax quirk).

v2 strategy (8 NeuronCores, 2 batch-groups x 4 cores):
  - NO pre-attention collectives. Softmax over HEADS is local per (q,k), so
    attention is sharded over the K/V length: each core projects its own
    512-row k-slice of K and V, and redundantly projects Q for the whole
    group's 2048 q rows (+41us PE, but saves ~240us of AllGather).
  - Per core: scores^T [k,q] tiles (128k x 512q x 16h), exp on ScalarE,
    head-sum via DVE tree-adds (bf16 4x mode), normalize split DVE/GpSimd,
    context accumulated across the 4 local k-chunks in PSUM chains (no
    SBUF accumulation traffic).
  - One bf16 ReduceScatter of ctx^T partials (4MB in -> 1MB out) hands each
    core the k-summed context for its own 512 q rows.
  - Output projection + residual + LayerNorm on own q rows.
  - All matmuls bf16 with fp32 PSUM accumulation.
"""

import numpy as np
import ml_dtypes

D = 1024
H = 16
DK = 64
P = 128
M = 512  # q rows per core (own slice) / k rows per core
SK = 2048
G = 4  # cores per batch group
GQ = 2048  # q rows per group
EC = D // P  # 8
HP = H // 2  # 8
KC = M // P  # 4 local k chunks
QB = GQ // M  # 4 q blocks (one per destination core)
LN_EPS = 1e-5

_CACHE = {}


def _build():
    import concourse.bass as bass
    import concourse.mybir as mybir
    import concourse.tile as tile
    from concourse import bacc

    f32 = mybir.dt.float32
    bf16 = mybir.dt.bfloat16
    AF = mybir.ActivationFunctionType
    OP = mybir.AluOpType
    AX = mybir.AxisListType

    nc = bacc.Bacc("TRN2", target_bir_lowering=False, debug=False, num_devices=8)

    xqT = nc.dram_tensor("xqT", [D, GQ], bf16, kind="ExternalInput").ap()
    xkT = nc.dram_tensor("xkT", [D, M], bf16, kind="ExternalInput").ap()
    xvT = nc.dram_tensor("xvT", [D, M], bf16, kind="ExternalInput").ap()
    xres = nc.dram_tensor("xres", [M, D], f32, kind="ExternalInput").ap()
    wq = nc.dram_tensor("wq", [D, D], bf16, kind="ExternalInput").ap()
    wk = nc.dram_tensor("wk", [D, D], bf16, kind="ExternalInput").ap()
    wv = nc.dram_tensor("wv", [D, D], bf16, kind="ExternalInput").ap()
    wo = nc.dram_tensor("wo", [D, D], bf16, kind="ExternalInput").ap()
    gam = nc.dram_tensor("gam", [P, D], f32, kind="ExternalInput").ap()
    bet = nc.dram_tensor("bet", [P, D], f32, kind="ExternalInput").ap()
    out = nc.dram_tensor("out", [M, D], f32, kind="ExternalOutput").ap()

    rg = [[0, 1, 2, 3], [4, 5, 6, 7]]

    from contextlib import ExitStack

    with tile.TileContext(nc) as tc:
        with ExitStack() as ctx:
            const = ctx.enter_context(tc.tile_pool(name="const", bufs=1))
            smal = ctx.enter_context(tc.tile_pool(name="smal", bufs=2))
            dram = ctx.enter_context(tc.tile_pool(name="dram", bufs=1, space="DRAM"))

            # persistent tiles
            kT_sb = const.tile([P, HP, M], bf16)  # K^T own slice, dk-pair packed
            vt_sb = const.tile([P, KC, D], bf16)  # V own slice [k-chunk, v]
            qT_sb = const.tile([P, HP, GQ], bf16)  # Q^T whole group

            ctx_cc_in = dram.tile([QB * D, M], bf16)
            ctx_cc_out = dram.tile([D, M], bf16)

            # ---- Phase A: K^T, V, and first-quarter Q^T projections ----
            # Q quarters 1-3 are emitted inside Phase C where PE has idle
            # slots (C is Activation-paced); wq/xq pools outlive Phase A.
            wq_p = ctx.enter_context(tc.tile_pool(name="wq_p", bufs=1))
            xq_p = ctx.enter_context(tc.tile_pool(name="xq_p", bufs=2))
            wq_sb = wq_p.tile([P, EC, D], bf16)
            xqv = xqT.rearrange("(o p) q -> p o q", p=P)

            def emit_qproj_qu(qu, pool, tag, evac_dve):
                xq_qu = xq_p.tile([P, EC, M], bf16, tag="xq")
                nc.sync.dma_start(xq_qu[:], xqv[:, :, qu * M : (qu + 1) * M])
                for hp in range(HP):
                    ps = pool.tile([P, M] if tag == "cx" else [P, GQ], f32, tag=tag)
                    for dc in range(EC):
                        nc.tensor.matmul(
                            ps[:, 0:M],
                            lhsT=wq_sb[:, dc, hp * P : (hp + 1) * P],
                            rhs=xq_qu[:, dc, :],
                            start=(dc == 0),
                            stop=(dc == EC - 1),
                        )
                    dst = qT_sb[:, hp, qu * M : (qu + 1) * M]
                    if evac_dve:
                        nc.vector.tensor_copy(dst, ps[:, 0:M])
                    else:
                        nc.scalar.copy(dst, ps[:, 0:M])

            with tc.tile_pool(name="wpool", bufs=2) as wpool, \
                 tc.tile_pool(name="xk_p", bufs=1) as xk_p, \
                 tc.tile_pool(name="xv_p", bufs=1) as xv_p, \
                 tc.tile_pool(name="psA", bufs=2, space="PSUM") as psA:
                # Lead-in DMAs spread across engine DGE queues so wk/xk (the
                # Kproj gate) stream in parallel instead of serializing.
                wk_view = wk.rearrange("(o p) e -> p o e", p=P)
                wk_sb = wpool.tile([P, EC, D], bf16, tag="w")
                nc.sync.dma_start(wk_sb[:, 0:4, :], wk_view[:, 0:4, :])
                nc.scalar.dma_start(wk_sb[:, 4:EC, :], wk_view[:, 4:EC, :])
                xk_sb = xk_p.tile([P, EC, M], bf16)
                nc.gpsimd.dma_start(xk_sb[:], xkT.rearrange("(o p) q -> p o q", p=P))
                wv_sb = wpool.tile([P, EC, D], bf16, tag="w")
                nc.gpsimd.dma_start(wv_sb[:], wv.rearrange("(o p) e -> p o e", p=P))
                xv_sb = xv_p.tile([P, EC, M], bf16)
                nc.sync.dma_start(xv_sb[:], xvT.rearrange("(o p) q -> p o q", p=P))
                nc.sync.dma_start(wq_sb[:], wq.rearrange("(o p) e -> p o e", p=P))

                # K^T projection: own 512 k rows -> kT_sb [P, hp, 512]
                for w0 in range(0, HP, 3):
                    hps = list(range(w0, min(w0 + 3, HP)))
                    n = len(hps)
                    ps = psA.tile([P, GQ], f32, tag="a")
                    for j, hp in enumerate(hps):
                        for dc in range(EC):
                            nc.tensor.matmul(
                                ps[:, j * M : (j + 1) * M],
                                lhsT=wk_sb[:, dc, hp * P : (hp + 1) * P],
                                rhs=xk_sb[:, dc, :],
                                start=(dc == 0),
                                stop=(dc == EC - 1),
                            )
                    nc.scalar.copy(kT_sb[:, w0 : w0 + n, :], ps[:, : n * M])

                # V projection: own 512 k rows -> vt_sb [P(k), kc, 1024(v)]
                for kc in range(KC):
                    ps = psA.tile([P, GQ], f32, tag="a")
                    for half in range(2):
                        for dc in range(EC):
                            nc.tensor.matmul(
                                ps[:, half * M : (half + 1) * M],
                                lhsT=xv_sb[:, dc, kc * P : (kc + 1) * P],
                                rhs=wv_sb[:, dc, half * M : (half + 1) * M],
                                start=(dc == 0),
                                stop=(dc == EC - 1),
                            )
                    nc.scalar.copy(vt_sb[:, kc, :], ps[:, :D])

                # Q^T projection, first quarter (covers q-block 0)
                emit_qproj_qu(0, psA, "a", evac_dve=False)

            # ---- Phase C: attention tiles + ctx PSUM chains + partial out ----
            # Software-pipelined: the 8 ctx chains of q-block qb interleave
            # with the first score/exp tiles of qb+1 so neither PE nor the
            # Activation engine idles across the qb boundary.
            cc_view = ctx_cc_in.rearrange("(b o p) q -> p b o q", b=QB, p=P)
            with tc.tile_pool(name="etp", bufs=6) as etp, \
                 tc.tile_pool(name="t8p", bufs=1) as t8p, \
                 tc.tile_pool(name="stg", bufs=3) as stg, \
                 tc.tile_pool(name="psC", bufs=2, space="PSUM") as psC:

                def emit_tile(qb, kc):
                    et = etp.tile([P, H, M], bf16, tag="et")
                    # scores + exp, 3-head waves; head-sum accumulates wave
                    # partials as exps land so `den` is ready ~1 op after the
                    # last wave (keeps ctx chains from stalling on the tree).
                    par = t8p.tile([P, 8, M], bf16, tag="t8")
                    for wi, w0 in enumerate(range(0, H, 3)):
                        hs = list(range(w0, min(w0 + 3, H)))
                        n = len(hs)
                        ps = psC.tile([P, 3 * M], f32, tag="sc")
                        for j, h in enumerate(hs):
                            hp, half = divmod(h, 2)
                            pb = half * DK
                            nc.tensor.matmul(
                                ps[:, j * M : (j + 1) * M],
                                lhsT=kT_sb[pb : pb + DK, hp, kc * P : (kc + 1) * P],
                                rhs=qT_sb[pb : pb + DK, hp, qb * M : (qb + 1) * M],
                                start=True,
                                stop=True,
                            )
                        nc.scalar.activation(
                            et[:, w0 : w0 + n, :], ps[:, : n * M], AF.Exp,
                            scale=0.125,
                        )
                        if n == 3:
                            nc.vector.tensor_tensor(
                                par[:, wi, :], et[:, w0, :], et[:, w0 + 1, :], OP.add
                            )
                            nc.vector.tensor_tensor(
                                par[:, wi, :], par[:, wi, :], et[:, w0 + 2, :], OP.add
                            )
                        if wi == 1:
                            nc.vector.tensor_tensor(
                                par[:, 6, :], par[:, 0, :], par[:, 1, :], OP.add
                            )
                        if wi == 3:
                            nc.vector.tensor_tensor(
                                par[:, 7, :], par[:, 2, :], par[:, 3, :], OP.add
                            )
                    # waves: 5x3 heads + 1 single head (h15)
                    nc.vector.tensor_tensor(
                        par[:, 6, :], par[:, 6, :], par[:, 7, :], OP.add
                    )
                    nc.vector.tensor_tensor(
                        par[:, 4, :], par[:, 4, :], et[:, 15, :], OP.add
                    )
                    den = smal.tile([P, M], f32, tag="den")
                    nc.vector.tensor_tensor(den[:], par[:, 6, :], par[:, 4, :], OP.add)
                    rf = smal.tile([P, M], f32, tag="rf")
                    nc.vector.reciprocal_approx_fast(rf[:], den[:])
                    rb = smal.tile([P, M], bf16, tag="rb")
                    nc.vector.tensor_copy(rb[:], rf[:])
                    # normalize: attn = e * r (in place; split DVE/GpSimd)
                    nsp = 8
                    nc.vector.tensor_tensor(
                        et[:, :nsp, :],
                        et[:, :nsp, :],
                        rb[:, None, :].to_broadcast((P, nsp, M)),
                        OP.mult,
                    )
                    nc.gpsimd.tensor_tensor(
                        et[:, nsp:, :],
                        et[:, nsp:, :],
                        rb[:, None, :].to_broadcast((P, H - nsp, M)),
                        OP.mult,
                    )
                    return et

                def emit_chain(qb, hp, ets):
                    cps = psC.tile([P, M], f32, tag="cx")
                    for kc in range(KC):
                        nc.tensor.matmul(
                            cps[0:DK, :],
                            lhsT=vt_sb[:, kc, (2 * hp) * DK : (2 * hp + 1) * DK],
                            rhs=ets[kc][:, 2 * hp, :],
                            start=(kc == 0),
                            stop=(kc == KC - 1),
                            tile_position=(0, 0),
                        )
                        nc.tensor.matmul(
                            cps[DK:P, :],
                            lhsT=vt_sb[:, kc, (2 * hp + 1) * DK : (2 * hp + 2) * DK],
                            rhs=ets[kc][:, 2 * hp + 1, :],
                            start=(kc == 0),
                            stop=(kc == KC - 1),
                            tile_position=(0, DK),
                        )
                    ost = stg.tile([P, M], bf16, tag="ost")
                    if hp % 8 < 5:
                        nc.scalar.copy(ost[:], cps[:])
                    else:
                        nc.vector.tensor_copy(ost[:], cps[:])
                    nc.sync.dma_start(cc_view[:, qb, hp, :], ost[:])

                # Prologue: q-block 0 tiles interleaved with Q^T quarters 1-3
                # (those matmuls fill PE slots while Act runs the exps; their
                # chains borrow the idle "cx" PSUM tag, evacs go to DVE).
                ets_cur = [emit_tile(0, 0)]
                emit_qproj_qu(1, psC, "cx", evac_dve=True)
                ets_cur.append(emit_tile(0, 1))
                emit_qproj_qu(2, psC, "cx", evac_dve=False)
                ets_cur.append(emit_tile(0, 2))
                emit_qproj_qu(3, psC, "cx", evac_dve=True)
                ets_cur.append(emit_tile(0, 3))
                for qb in range(QB):
                    ets_next = []
                    for hp in range(HP):
                        emit_chain(qb, hp, ets_cur)
                        # 3-tile lookahead into qb+1 (etp bufs=7 allows it)
                        if qb + 1 < QB and hp in (2, 4, 6):
                            ets_next.append(emit_tile(qb + 1, len(ets_next)))
                    if qb + 1 < QB:
                        while len(ets_next) < KC:
                            ets_next.append(emit_tile(qb + 1, len(ets_next)))
                    ets_cur = ets_next

            # ---- ReduceScatter + Phase D: O-proj + residual + LayerNorm ----
            res_view = xres.rearrange("(o p) e -> o p e", p=P)
            out_view = out.rearrange("(o p) e -> o p e", p=P)
            with tc.tile_pool(name="dpool", bufs=1) as dpool, \
                 tc.tile_pool(name="resp", bufs=4) as resp, \
                 tc.tile_pool(name="lnp", bufs=2) as lnp, \
                 tc.tile_pool(name="psD", bufs=2, space="PSUM") as psD:
                wo_sb = dpool.tile([P, EC, D], bf16)
                nc.sync.dma_start(wo_sb[:], wo.rearrange("(o p) e -> p o e", p=P))
                gam_sb = dpool.tile([P, D], f32)
                nc.sync.dma_start(gam_sb[:], gam[:])
                bet_sb = dpool.tile([P, D], f32)
                nc.sync.dma_start(bet_sb[:], bet[:])
                ctxT_sb = dpool.tile([P, EC, M], bf16)  # summed ctx^T own q
                nc.gpsimd.collective_compute(
                    "ReduceScatter",
                    mybir.AluOpType.add,
                    replica_groups=rg,
                    ins=[ctx_cc_in.opt()],
                    outs=[ctx_cc_out.opt()],
                )
                ccv = ctx_cc_out.rearrange("(o p) q -> p o q", p=P)
                for vc in range(EC):
                    nc.sync.dma_start(ctxT_sb[:, vc, :], ccv[:, vc, :])
                rests = []
                for qc in range(M // P):
                    rest = resp.tile([P, D], f32, tag="res")
                    nc.sync.dma_start(rest[:], res_view[qc])
                    rests.append(rest)
                for qc in range(M // P):
                    rest = rests[qc]
                    ps = psD.tile([P, D], f32, tag="o")
                    for half in range(2):
                        for vc in range(EC):
                            nc.tensor.matmul(
                                ps[:, half * M : (half + 1) * M],
                                lhsT=ctxT_sb[:, vc, qc * P : (qc + 1) * P],
                                rhs=wo_sb[:, vc, half * M : (half + 1) * M],
                                start=(vc == 0),
                                stop=(vc == EC - 1),
                            )
                    # residual add + stats in column halves so downstream
                    # normalize/output passes pipeline (shorter drain)
                    xsb = lnp.tile([P, D], f32, tag="x")
                    bst = smal.tile([P, 2, 6], f32, tag="bst")
                    for h in range(2):
                        nc.vector.tensor_tensor(
                            xsb[:, h * M : (h + 1) * M],
                            ps[:, h * M : (h + 1) * M],
                            rest[:, h * M : (h + 1) * M],
                            OP.add,
                        )
                        nc.vector.bn_stats(bst[:, h, :], xsb[:, h * M : (h + 1) * M])
                    agg = smal.tile([P, 2], f32, tag="agg")
                    nc.vector.bn_aggr(agg[:], bst[:])
                    veps = smal.tile([P, 1], f32, tag="veps")
                    nc.vector.tensor_scalar(
                        veps[:], agg[:, 1:2], 1.0, LN_EPS, OP.mult, OP.add
                    )
                    std = smal.tile([P, 1], f32, tag="std")
                    nc.scalar.activation(std[:], veps[:], AF.Sqrt)
                    inv = smal.tile([P, 1], f32, tag="inv")
                    nc.vector.reciprocal(inv[:], std[:])
                    # xn = (x - mu) * inv ; *gamma on DVE, +beta on Pool;
                    # column halves pipeline into the output DMA
                    ot = lnp.tile([P, D], f32, tag="ot")
                    for h in range(2):
                        sl = slice(h * M, (h + 1) * M)
                        nc.vector.tensor_scalar(
                            xsb[:, sl], xsb[:, sl], agg[:, 0:1], inv[:],
                            OP.subtract, OP.mult,
                        )
                        nc.vector.tensor_tensor(
                            xsb[:, sl], xsb[:, sl], gam_sb[:, sl], OP.mult
                        )
                        nc.gpsimd.tensor_tensor(
                            ot[:, sl], xsb[:, sl], bet_sb[:, sl], OP.add
                        )
                        nc.sync.dma_start(out_view[qc][:, sl], ot[:, sl])

    nc.compile()
    return nc


def _get_nc():
    if "nc" not in _CACHE:
        _CACHE["nc"] = _build()
    return _CACHE["nc"]


def _in_maps(input_Q, input_K, input_V, W_Q, W_K, W_V, W_O, ln_gamma, ln_beta):
    bf = ml_dtypes.bfloat16
    f32 = np.float32
    Q_ = np.asarray(input_Q, dtype=f32)
    K_ = np.asarray(input_K, dtype=f32)
    V_ = np.asarray(input_V, dtype=f32)
    wq_b = np.asarray(W_Q, dtype=f32).astype(bf)
    wk_b = np.asarray(W_K, dtype=f32).astype(bf)
    wv_b = np.asarray(W_V, dtype=f32).astype(bf)
    wo_b = np.asarray(W_O, dtype=f32).astype(bf)
    gam_b = np.ascontiguousarray(
        np.broadcast_to(np.asarray(ln_gamma, dtype=f32), (P, D))
    )
    bet_b = np.ascontiguousarray(
        np.broadcast_to(np.asarray(ln_beta, dtype=f32), (P, D))
    )
    maps = []
    for c in range(8):
        b, r = divmod(c, G)
        sl = slice(r * M, (r + 1) * M)
        maps.append(
            {
                "xqT": np.ascontiguousarray(Q_[b].T).astype(bf),
                "xkT": np.ascontiguousarray(K_[b, sl].T).astype(bf),
                "xvT": np.ascontiguousarray(V_[b, sl].T).astype(bf),
                "xres": np.ascontiguousarray(Q_[b, sl]),
                "wq": wq_b,
                "wk": wk_b,
                "wv": wv_b,
                "wo": wo_b,
                "gam": gam_b,
                "bet": bet_b,
            }
        )
    return maps


def _assemble(results):
    B = 2
    out = np.empty((B, SK, D), np.float32)
    for c in range(8):
        b, r = divmod(c, G)
        out[b, r * M : (r + 1) * M] = results[c]["out"]
    return out


def run_traced(trace=False, **inputs):
    """Run on HW; returns (output, BassKernelResults)."""
    from concourse.bass_utils import run_bass_kernel_spmd

    nc = _get_nc()
    maps = _in_maps(**inputs)
    res = run_bass_kernel_spmd(nc, maps, list(range(8)), trace=trace)
    return _assemble(res.results), res


def kernel(**inputs) -> np.ndarray:
    out, _ = run_traced(trace=False, **inputs)
    return out


# revision 9
# speedup vs baseline: 1.0953x; 1.0065x over previous
"""Trainium2 Bass kernel for nn_MultiHeadAttention (head-axis softmax quirk).

v2 strategy (8 NeuronCores, 2 batch-groups x 4 cores):
  - NO pre-attention collectives. Softmax over HEADS is local per (q,k), so
    attention is sharded over the K/V length: each core projects its own
    512-row k-slice of K and V, and redundantly projects Q for the whole
    group's 2048 q rows (+41us PE, but saves ~240us of AllGather).
  - Per core: scores^T [k,q] tiles (128k x 512q x 16h), exp on ScalarE,
    head-sum via DVE tree-adds (bf16 4x mode), normalize split DVE/GpSimd,
    context accumulated across the 4 local k-chunks in PSUM chains (no
    SBUF accumulation traffic).
  - One bf16 ReduceScatter of ctx^T partials (4MB in -> 1MB out) hands each
    core the k-summed context for its own 512 q rows.
  - Output projection + residual + LayerNorm on own q rows.
  - All matmuls bf16 with fp32 PSUM accumulation.
"""

import numpy as np
import ml_dtypes

D = 1024
H = 16
DK = 64
P = 128
M = 512  # q rows per core (own slice) / k rows per core
SK = 2048
G = 4  # cores per batch group
GQ = 2048  # q rows per group
EC = D // P  # 8
HP = H // 2  # 8
KC = M // P  # 4 local k chunks
QB = GQ // M  # 4 q blocks (one per destination core)
LN_EPS = 1e-5

_CACHE = {}


def _build():
    import concourse.bass as bass
    import concourse.mybir as mybir
    import concourse.tile as tile
    from concourse import bacc

    f32 = mybir.dt.float32
    bf16 = mybir.dt.bfloat16
    AF = mybir.ActivationFunctionType
    OP = mybir.AluOpType
    AX = mybir.AxisListType

    nc = bacc.Bacc("TRN2", target_bir_lowering=False, debug=False, num_devices=8)

    xqT = nc.dram_tensor("xqT", [D, GQ], bf16, kind="ExternalInput").ap()
    xkT = nc.dram_tensor("xkT", [D, M], bf16, kind="ExternalInput").ap()
    xvT = nc.dram_tensor("xvT", [D, M], bf16, kind="ExternalInput").ap()
    xres = nc.dram_tensor("xres", [M, D], f32, kind="ExternalInput").ap()
    wq = nc.dram_tensor("wq", [D, D], bf16, kind="ExternalInput").ap()
    wk = nc.dram_tensor("wk", [D, D], bf16, kind="ExternalInput").ap()
    wv = nc.dram_tensor("wv", [D, D], bf16, kind="ExternalInput").ap()
    wo = nc.dram_tensor("wo", [D, D], bf16, kind="ExternalInput").ap()
    gam = nc.dram_tensor("gam", [P, D], f32, kind="ExternalInput").ap()
    bet = nc.dram_tensor("bet", [P, D], f32, kind="ExternalInput").ap()
    out = nc.dram_tensor("out", [M, D], f32, kind="ExternalOutput").ap()

    rg = [[0, 1, 2, 3], [4, 5, 6, 7]]

    from contextlib import ExitStack

    with tile.TileContext(nc) as tc:
        with ExitStack() as ctx:
            const = ctx.enter_context(tc.tile_pool(name="const", bufs=1))
            smal = ctx.enter_context(tc.tile_pool(name="smal", bufs=2))
            dram = ctx.enter_context(tc.tile_pool(name="dram", bufs=1, space="DRAM"))

            # persistent tiles
            kT_sb = const.tile([P, HP, M], bf16)  # K^T own slice, dk-pair packed
            vt_sb = const.tile([P, KC, D], bf16)  # V own slice [k-chunk, v]
            qT_sb = const.tile([P, HP, GQ], bf16)  # Q^T whole group

            ctx_cc_in = dram.tile([QB * D, M], bf16)
            ctx_cc_out = dram.tile([D, M], bf16)

            # ---- Phase A: K^T, V, and first-quarter Q^T projections ----
            # Q quarters 1-3 are emitted inside Phase C where PE has idle
            # slots (C is Activation-paced); wq/xq pools outlive Phase A.
            wq_p = ctx.enter_context(tc.tile_pool(name="wq_p", bufs=1))
            xq_p = ctx.enter_context(tc.tile_pool(name="xq_p", bufs=2))
            wq_sb = wq_p.tile([P, EC, D], bf16)
            xqv = xqT.rearrange("(o p) q -> p o q", p=P)

            def emit_qproj_qu(qu, pool, tag, evac_dve):
                xq_qu = xq_p.tile([P, EC, M], bf16, tag="xq")
                nc.sync.dma_start(xq_qu[:], xqv[:, :, qu * M : (qu + 1) * M])
                for hp in range(HP):
                    ps = pool.tile([P, M] if tag == "cx" else [P, GQ], f32, tag=tag)
                    for dc in range(EC):
                        nc.tensor.matmul(
                            ps[:, 0:M],
                            lhsT=wq_sb[:, dc, hp * P : (hp + 1) * P],
                            rhs=xq_qu[:, dc, :],
                            start=(dc == 0),
                            stop=(dc == EC - 1),
                        )
                    dst = qT_sb[:, hp, qu * M : (qu + 1) * M]
                    if evac_dve:
                        nc.vector.tensor_copy(dst, ps[:, 0:M])
                    else:
                        nc.scalar.copy(dst, ps[:, 0:M])

            with tc.tile_pool(name="wpool", bufs=2) as wpool, \
                 tc.tile_pool(name="xk_p", bufs=1) as xk_p, \
                 tc.tile_pool(name="xv_p", bufs=1) as xv_p, \
                 tc.tile_pool(name="psA", bufs=2, space="PSUM") as psA:
                # Lead-in DMAs spread across engine DGE queues so wk/xk (the
                # Kproj gate) stream in parallel instead of serializing.
                wk_view = wk.rearrange("(o p) e -> p o e", p=P)
                wk_sb = wpool.tile([P, EC, D], bf16, tag="w")
                nc.sync.dma_start(wk_sb[:, 0:4, :], wk_view[:, 0:4, :])
                nc.scalar.dma_start(wk_sb[:, 4:EC, :], wk_view[:, 4:EC, :])
                xk_sb = xk_p.tile([P, EC, M], bf16)
                nc.gpsimd.dma_start(xk_sb[:], xkT.rearrange("(o p) q -> p o q", p=P))
                wv_sb = wpool.tile([P, EC, D], bf16, tag="w")
                nc.gpsimd.dma_start(wv_sb[:], wv.rearrange("(o p) e -> p o e", p=P))
                xv_sb = xv_p.tile([P, EC, M], bf16)
                nc.sync.dma_start(xv_sb[:], xvT.rearrange("(o p) q -> p o q", p=P))
                nc.sync.dma_start(wq_sb[:], wq.rearrange("(o p) e -> p o e", p=P))

                # K^T projection: own 512 k rows -> kT_sb [P, hp, 512]
                for w0 in range(0, HP, 3):
                    hps = list(range(w0, min(w0 + 3, HP)))
                    n = len(hps)
                    ps = psA.tile([P, GQ], f32, tag="a")
                    for j, hp in enumerate(hps):
                        for dc in range(EC):
                            nc.tensor.matmul(
                                ps[:, j * M : (j + 1) * M],
                                lhsT=wk_sb[:, dc, hp * P : (hp + 1) * P],
                                rhs=xk_sb[:, dc, :],
                                start=(dc == 0),
                                stop=(dc == EC - 1),
                            )
                    nc.scalar.copy(kT_sb[:, w0 : w0 + n, :], ps[:, : n * M])

                # V projection: own 512 k rows -> vt_sb [P(k), kc, 1024(v)]
                for kc in range(KC):
                    ps = psA.tile([P, GQ], f32, tag="a")
                    for half in range(2):
                        for dc in range(EC):
                            nc.tensor.matmul(
                                ps[:, half * M : (half + 1) * M],
                                lhsT=xv_sb[:, dc, kc * P : (kc + 1) * P],
                                rhs=wv_sb[:, dc, half * M : (half + 1) * M],
                                start=(dc == 0),
                                stop=(dc == EC - 1),
                            )
                    nc.scalar.copy(vt_sb[:, kc, :], ps[:, :D])

                # Q^T projection, first quarter (covers q-block 0)
                emit_qproj_qu(0, psA, "a", evac_dve=False)

            # ---- Phase C: attention tiles + ctx PSUM chains + partial out ----
            # Software-pipelined: the 8 ctx chains of q-block qb interleave
            # with the first score/exp tiles of qb+1 so neither PE nor the
            # Activation engine idles across the qb boundary.
            cc_view = ctx_cc_in.rearrange("(b o p) q -> p b o q", b=QB, p=P)
            with tc.tile_pool(name="etp", bufs=6) as etp, \
                 tc.tile_pool(name="t8p", bufs=1) as t8p, \
                 tc.tile_pool(name="stg", bufs=3) as stg, \
                 tc.tile_pool(name="psC", bufs=2, space="PSUM") as psC:

                def emit_tile(qb, kc):
                    et = etp.tile([P, H, M], bf16, tag="et")
                    # scores + exp, 3-head waves; head-sum accumulates wave
                    # partials as exps land so `den` is ready ~1 op after the
                    # last wave (keeps ctx chains from stalling on the tree).
                    par = t8p.tile([P, 8, M], bf16, tag="t8")
                    for wi, w0 in enumerate(range(0, H, 3)):
                        hs = list(range(w0, min(w0 + 3, H)))
                        n = len(hs)
                        ps = psC.tile([P, 3 * M], f32, tag="sc")
                        for j, h in enumerate(hs):
                            hp, half = divmod(h, 2)
                            pb = half * DK
                            nc.tensor.matmul(
                                ps[:, j * M : (j + 1) * M],
                                lhsT=kT_sb[pb : pb + DK, hp, kc * P : (kc + 1) * P],
                                rhs=qT_sb[pb : pb + DK, hp, qb * M : (qb + 1) * M],
                                start=True,
                                stop=True,
                            )
                        nc.scalar.activation(
                            et[:, w0 : w0 + n, :], ps[:, : n * M], AF.Exp,
                            scale=0.125,
                        )
                        if n == 3:
                            nc.vector.tensor_tensor(
                                par[:, wi, :], et[:, w0, :], et[:, w0 + 1, :], OP.add
                            )
                            nc.vector.tensor_tensor(
                                par[:, wi, :], par[:, wi, :], et[:, w0 + 2, :], OP.add
                            )
                        if wi == 1:
                            nc.vector.tensor_tensor(
                                par[:, 6, :], par[:, 0, :], par[:, 1, :], OP.add
                            )
                        if wi == 3:
                            nc.vector.tensor_tensor(
                                par[:, 7, :], par[:, 2, :], par[:, 3, :], OP.add
                            )
                    # waves: 5x3 heads + 1 single head (h15)
                    nc.vector.tensor_tensor(
                        par[:, 6, :], par[:, 6, :], par[:, 7, :], OP.add
                    )
                    nc.vector.tensor_tensor(
                        par[:, 4, :], par[:, 4, :], et[:, 15, :], OP.add
                    )
                    den = smal.tile([P, M], f32, tag="den")
                    nc.vector.tensor_tensor(den[:], par[:, 6, :], par[:, 4, :], OP.add)
                    rf = smal.tile([P, M], f32, tag="rf")
                    nc.vector.reciprocal_approx_fast(rf[:], den[:])
                    rb = smal.tile([P, M], bf16, tag="rb")
                    nc.vector.tensor_copy(rb[:], rf[:])
                    # normalize: attn = e * r (in place; split DVE/GpSimd)
                    nsp = 8
                    for h0 in range(0, nsp, 2):
                        nc.vector.tensor_tensor(
                            et[:, h0 : h0 + 2, :],
                            et[:, h0 : h0 + 2, :],
                            rb[:, None, :].to_broadcast((P, 2, M)),
                            OP.mult,
                        )
                    for h0 in range(nsp, H, 2):
                        nc.gpsimd.tensor_tensor(
                            et[:, h0 : h0 + 2, :],
                            et[:, h0 : h0 + 2, :],
                            rb[:, None, :].to_broadcast((P, 2, M)),
                            OP.mult,
                        )
                    return et

                def emit_chain(qb, hp, ets):
                    cps = psC.tile([P, M], f32, tag="cx")
                    for kc in range(KC):
                        nc.tensor.matmul(
                            cps[0:DK, :],
                            lhsT=vt_sb[:, kc, (2 * hp) * DK : (2 * hp + 1) * DK],
                            rhs=ets[kc][:, 2 * hp, :],
                            start=(kc == 0),
                            stop=(kc == KC - 1),
                            tile_position=(0, 0),
                        )
                        nc.tensor.matmul(
                            cps[DK:P, :],
                            lhsT=vt_sb[:, kc, (2 * hp + 1) * DK : (2 * hp + 2) * DK],
                            rhs=ets[kc][:, 2 * hp + 1, :],
                            start=(kc == 0),
                            stop=(kc == KC - 1),
                            tile_position=(0, DK),
                        )
                    ost = stg.tile([P, M], bf16, tag="ost")
                    if hp % 8 < 5:
                        nc.scalar.copy(ost[:], cps[:])
                    else:
                        nc.vector.tensor_copy(ost[:], cps[:])
                    nc.sync.dma_start(cc_view[:, qb, hp, :], ost[:])

                # Prologue: q-block 0 tiles interleaved with Q^T quarters 1-3
                # (those matmuls fill PE slots while Act runs the exps; their
                # chains borrow the idle "cx" PSUM tag, evacs go to DVE).
                ets_cur = [emit_tile(0, 0)]
                emit_qproj_qu(1, psC, "cx", evac_dve=True)
                ets_cur.append(emit_tile(0, 1))
                emit_qproj_qu(2, psC, "cx", evac_dve=False)
                ets_cur.append(emit_tile(0, 2))
                emit_qproj_qu(3, psC, "cx", evac_dve=True)
                ets_cur.append(emit_tile(0, 3))
                for qb in range(QB):
                    ets_next = []
                    for hp in range(HP):
                        emit_chain(qb, hp, ets_cur)
                        # 3-tile lookahead into qb+1 (etp bufs=7 allows it)
                        if qb + 1 < QB and hp in (2, 4, 6):
                            ets_next.append(emit_tile(qb + 1, len(ets_next)))
                    if qb + 1 < QB:
                        while len(ets_next) < KC:
                            ets_next.append(emit_tile(qb + 1, len(ets_next)))
                    ets_cur = ets_next

            # ---- ReduceScatter + Phase D: O-proj + residual + LayerNorm ----
            res_view = xres.rearrange("(o p) e -> o p e", p=P)
            out_view = out.rearrange("(o p) e -> o p e", p=P)
            with tc.tile_pool(name="dpool", bufs=1) as dpool, \
                 tc.tile_pool(name="resp", bufs=4) as resp, \
                 tc.tile_pool(name="lnp", bufs=2) as lnp, \
                 tc.tile_pool(name="psD", bufs=2, space="PSUM") as psD:
                wo_sb = dpool.tile([P, EC, D], bf16)
                nc.sync.dma_start(wo_sb[:], wo.rearrange("(o p) e -> p o e", p=P))
                gam_sb = dpool.tile([P, D], f32)
                nc.sync.dma_start(gam_sb[:], gam[:])
                bet_sb = dpool.tile([P, D], f32)
                nc.sync.dma_start(bet_sb[:], bet[:])
                ctxT_sb = dpool.tile([P, EC, M], bf16)  # summed ctx^T own q
                nc.gpsimd.collective_compute(
                    "ReduceScatter",
                    mybir.AluOpType.add,
                    replica_groups=rg,
                    ins=[ctx_cc_in.opt()],
                    outs=[ctx_cc_out.opt()],
                )
                ccv = ctx_cc_out.rearrange("(o p) q -> p o q", p=P)
                for vc in range(EC):
                    nc.sync.dma_start(ctxT_sb[:, vc, :], ccv[:, vc, :])
                rests = []
                for qc in range(M // P):
                    rest = resp.tile([P, D], f32, tag="res")
                    nc.sync.dma_start(rest[:], res_view[qc])
                    rests.append(rest)
                for qc in range(M // P):
                    rest = rests[qc]
                    ps = psD.tile([P, D], f32, tag="o")
                    for half in range(2):
                        for vc in range(EC):
                            nc.tensor.matmul(
                                ps[:, half * M : (half + 1) * M],
                                lhsT=ctxT_sb[:, vc, qc * P : (qc + 1) * P],
                                rhs=wo_sb[:, vc, half * M : (half + 1) * M],
                                start=(vc == 0),
                                stop=(vc == EC - 1),
                            )
                    # residual add + stats in column halves so downstream
                    # normalize/output passes pipeline (shorter drain)
                    xsb = lnp.tile([P, D], f32, tag="x")
                    bst = smal.tile([P, 2, 6], f32, tag="bst")
                    for h in range(2):
                        nc.vector.tensor_tensor(
                            xsb[:, h * M : (h + 1) * M],
                            ps[:, h * M : (h + 1) * M],
                            rest[:, h * M : (h + 1) * M],
                            OP.add,
                        )
                        nc.vector.bn_stats(bst[:, h, :], xsb[:, h * M : (h + 1) * M])
                    agg = smal.tile([P, 2], f32, tag="agg")
                    nc.vector.bn_aggr(agg[:], bst[:])
                    veps = smal.tile([P, 1], f32, tag="veps")
                    nc.vector.tensor_scalar(
                        veps[:], agg[:, 1:2], 1.0, LN_EPS, OP.mult, OP.add
                    )
                    std = smal.tile([P, 1], f32, tag="std")
                    nc.scalar.activation(std[:], veps[:], AF.Sqrt)
                    inv = smal.tile([P, 1], f32, tag="inv")
                    nc.vector.reciprocal(inv[:], std[:])
                    # xn = (x - mu) * inv ; *gamma on DVE, +beta on Pool;
                    # column halves pipeline into the output DMA
                    ot = lnp.tile([P, D], f32, tag="ot")
                    for h in range(2):
                        sl = slice(h * M, (h + 1) * M)
                        nc.vector.tensor_scalar(
                            xsb[:, sl], xsb[:, sl], agg[:, 0:1], inv[:],
                            OP.subtract, OP.mult,
                        )
                        nc.vector.tensor_tensor(
                            xsb[:, sl], xsb[:, sl], gam_sb[:, sl], OP.mult
                        )
                        nc.gpsimd.tensor_tensor(
                            ot[:, sl], xsb[:, sl], bet_sb[:, sl], OP.add
                        )
                        nc.sync.dma_start(out_view[qc][:, sl], ot[:, sl])

    nc.compile()
    return nc


def _get_nc():
    if "nc" not in _CACHE:
        _CACHE["nc"] = _build()
    return _CACHE["nc"]


def _in_maps(input_Q, input_K, input_V, W_Q, W_K, W_V, W_O, ln_gamma, ln_beta):
    bf = ml_dtypes.bfloat16
    f32 = np.float32
    Q_ = np.asarray(input_Q, dtype=f32)
    K_ = np.asarray(input_K, dtype=f32)
    V_ = np.asarray(input_V, dtype=f32)
    wq_b = np.asarray(W_Q, dtype=f32).astype(bf)
    wk_b = np.asarray(W_K, dtype=f32).astype(bf)
    wv_b = np.asarray(W_V, dtype=f32).astype(bf)
    wo_b = np.asarray(W_O, dtype=f32).astype(bf)
    gam_b = np.ascontiguousarray(
        np.broadcast_to(np.asarray(ln_gamma, dtype=f32), (P, D))
    )
    bet_b = np.ascontiguousarray(
        np.broadcast_to(np.asarray(ln_beta, dtype=f32), (P, D))
    )
    maps = []
    for c in range(8):
        b, r = divmod(c, G)
        sl = slice(r * M, (r + 1) * M)
        maps.append(
            {
                "xqT": np.ascontiguousarray(Q_[b].T).astype(bf),
                "xkT": np.ascontiguousarray(K_[b, sl].T).astype(bf),
                "xvT": np.ascontiguousarray(V_[b, sl].T).astype(bf),
                "xres": np.ascontiguousarray(Q_[b, sl]),
                "wq": wq_b,
                "wk": wk_b,
                "wv": wv_b,
                "wo": wo_b,
                "gam": gam_b,
                "bet": bet_b,
            }
        )
    return maps


def _assemble(results):
    B = 2
    out = np.empty((B, SK, D), np.float32)
    for c in range(8):
        b, r = divmod(c, G)
        out[b, r * M : (r + 1) * M] = results[c]["out"]
    return out


def run_traced(trace=False, **inputs):
    """Run on HW; returns (output, BassKernelResults)."""
    from concourse.bass_utils import run_bass_kernel_spmd

    nc = _get_nc()
    maps = _in_maps(**inputs)
    res = run_bass_kernel_spmd(nc, maps, list(range(8)), trace=trace)
    return _assemble(res.results), res


def kernel(**inputs) -> np.ndarray:
    out, _ = run_traced(trace=False, **inputs)
    return out


# revision 10
# speedup vs baseline: 1.0993x; 1.0036x over previous
"""Trainium2 Bass kernel for nn_MultiHeadAttention (head-axis softmax quirk).

v2 strategy (8 NeuronCores, 2 batch-groups x 4 cores):
  - NO pre-attention collectives. Softmax over HEADS is local per (q,k), so
    attention is sharded over the K/V length: each core projects its own
    512-row k-slice of K and V, and redundantly projects Q for the whole
    group's 2048 q rows (+41us PE, but saves ~240us of AllGather).
  - Per core: scores^T [k,q] tiles (128k x 512q x 16h), exp on ScalarE,
    head-sum via DVE tree-adds (bf16 4x mode), normalize split DVE/GpSimd,
    context accumulated across the 4 local k-chunks in PSUM chains (no
    SBUF accumulation traffic).
  - One bf16 ReduceScatter of ctx^T partials (4MB in -> 1MB out) hands each
    core the k-summed context for its own 512 q rows.
  - Output projection + residual + LayerNorm on own q rows.
  - All matmuls bf16 with fp32 PSUM accumulation.
"""

import numpy as np
import ml_dtypes

D = 1024
H = 16
DK = 64
P = 128
M = 512  # q rows per core (own slice) / k rows per core
SK = 2048
G = 4  # cores per batch group
GQ = 2048  # q rows per group
EC = D // P  # 8
HP = H // 2  # 8
KC = M // P  # 4 local k chunks
QB = GQ // M  # 4 q blocks (one per destination core)
LN_EPS = 1e-5

_CACHE = {}


def _build():
    import concourse.bass as bass
    import concourse.mybir as mybir
    import concourse.tile as tile
    from concourse import bacc

    f32 = mybir.dt.float32
    bf16 = mybir.dt.bfloat16
    AF = mybir.ActivationFunctionType
    OP = mybir.AluOpType
    AX = mybir.AxisListType

    nc = bacc.Bacc("TRN2", target_bir_lowering=False, debug=False, num_devices=8)

    xqT = nc.dram_tensor("xqT", [D, GQ], bf16, kind="ExternalInput").ap()
    xkT = nc.dram_tensor("xkT", [D, M], bf16, kind="ExternalInput").ap()
    xvT = nc.dram_tensor("xvT", [D, M], bf16, kind="ExternalInput").ap()
    xres = nc.dram_tensor("xres", [M, D], f32, kind="ExternalInput").ap()
    wq = nc.dram_tensor("wq", [D, D], bf16, kind="ExternalInput").ap()
    wk = nc.dram_tensor("wk", [D, D], bf16, kind="ExternalInput").ap()
    wv = nc.dram_tensor("wv", [D, D], bf16, kind="ExternalInput").ap()
    wo = nc.dram_tensor("wo", [D, D], bf16, kind="ExternalInput").ap()
    gam = nc.dram_tensor("gam", [P, D], f32, kind="ExternalInput").ap()
    bet = nc.dram_tensor("bet", [P, D], f32, kind="ExternalInput").ap()
    out = nc.dram_tensor("out", [M, D], f32, kind="ExternalOutput").ap()

    rg = [[0, 1, 2, 3], [4, 5, 6, 7]]

    from contextlib import ExitStack

    with tile.TileContext(nc) as tc:
        with ExitStack() as ctx:
            const = ctx.enter_context(tc.tile_pool(name="const", bufs=1))
            smal = ctx.enter_context(tc.tile_pool(name="smal", bufs=2))
            dram = ctx.enter_context(tc.tile_pool(name="dram", bufs=1, space="DRAM"))

            # persistent tiles
            kT_sb = const.tile([P, HP, M], bf16)  # K^T own slice, dk-pair packed
            vt_sb = const.tile([P, KC, D], bf16)  # V own slice [k-chunk, v]
            qT_sb = const.tile([P, HP, GQ], bf16)  # Q^T whole group

            ctx_cc_in = dram.tile([QB * D, M], bf16)
            ctx_cc_out = dram.tile([D, M], bf16)

            # ---- Phase A: K^T, V, and first-quarter Q^T projections ----
            # Q quarters 1-3 are emitted inside Phase C where PE has idle
            # slots (C is Activation-paced); wq/xq pools outlive Phase A.
            wq_p = ctx.enter_context(tc.tile_pool(name="wq_p", bufs=1))
            xq_p = ctx.enter_context(tc.tile_pool(name="xq_p", bufs=2))
            wq_sb = wq_p.tile([P, EC, D], bf16)
            xqv = xqT.rearrange("(o p) q -> p o q", p=P)

            def emit_qproj_qu(qu, pool, tag, evac_dve):
                xq_qu = xq_p.tile([P, EC, M], bf16, tag="xq")
                nc.sync.dma_start(xq_qu[:], xqv[:, :, qu * M : (qu + 1) * M])
                for hp in range(HP):
                    ps = pool.tile([P, M] if tag == "cx" else [P, GQ], f32, tag=tag)
                    for dc in range(EC):
                        nc.tensor.matmul(
                            ps[:, 0:M],
                            lhsT=wq_sb[:, dc, hp * P : (hp + 1) * P],
                            rhs=xq_qu[:, dc, :],
                            start=(dc == 0),
                            stop=(dc == EC - 1),
                        )
                    dst = qT_sb[:, hp, qu * M : (qu + 1) * M]
                    if evac_dve:
                        nc.vector.tensor_copy(dst, ps[:, 0:M])
                    else:
                        nc.scalar.copy(dst, ps[:, 0:M])

            with tc.tile_pool(name="wpool", bufs=2) as wpool, \
                 tc.tile_pool(name="xk_p", bufs=1) as xk_p, \
                 tc.tile_pool(name="xv_p", bufs=1) as xv_p, \
                 tc.tile_pool(name="psA", bufs=2, space="PSUM") as psA:
                # Lead-in DMAs spread across engine DGE queues so wk/xk (the
                # Kproj gate) stream in parallel instead of serializing.
                wk_view = wk.rearrange("(o p) e -> p o e", p=P)
                wk_sb = wpool.tile([P, EC, D], bf16, tag="w")
                nc.sync.dma_start(wk_sb[:, 0:4, :], wk_view[:, 0:4, :])
                nc.scalar.dma_start(wk_sb[:, 4:EC, :], wk_view[:, 4:EC, :])
                xk_sb = xk_p.tile([P, EC, M], bf16)
                nc.gpsimd.dma_start(xk_sb[:], xkT.rearrange("(o p) q -> p o q", p=P))
                wv_sb = wpool.tile([P, EC, D], bf16, tag="w")
                nc.gpsimd.dma_start(wv_sb[:], wv.rearrange("(o p) e -> p o e", p=P))
                xv_sb = xv_p.tile([P, EC, M], bf16)
                nc.sync.dma_start(xv_sb[:], xvT.rearrange("(o p) q -> p o q", p=P))
                nc.sync.dma_start(wq_sb[:], wq.rearrange("(o p) e -> p o e", p=P))

                # K^T projection: own 512 k rows -> kT_sb [P, hp, 512]
                for w0 in range(0, HP, 3):
                    hps = list(range(w0, min(w0 + 3, HP)))
                    n = len(hps)
                    ps = psA.tile([P, GQ], f32, tag="a")
                    for j, hp in enumerate(hps):
                        for dc in range(EC):
                            nc.tensor.matmul(
                                ps[:, j * M : (j + 1) * M],
                                lhsT=wk_sb[:, dc, hp * P : (hp + 1) * P],
                                rhs=xk_sb[:, dc, :],
                                start=(dc == 0),
                                stop=(dc == EC - 1),
                            )
                    nc.scalar.copy(kT_sb[:, w0 : w0 + n, :], ps[:, : n * M])

                # V projection: own 512 k rows -> vt_sb [P(k), kc, 1024(v)]
                for kc in range(KC):
                    ps = psA.tile([P, GQ], f32, tag="a")
                    for half in range(2):
                        for dc in range(EC):
                            nc.tensor.matmul(
                                ps[:, half * M : (half + 1) * M],
                                lhsT=xv_sb[:, dc, kc * P : (kc + 1) * P],
                                rhs=wv_sb[:, dc, half * M : (half + 1) * M],
                                start=(dc == 0),
                                stop=(dc == EC - 1),
                            )
                    nc.scalar.copy(vt_sb[:, kc, :], ps[:, :D])

                # Q^T projection, first quarter (covers q-block 0)
                emit_qproj_qu(0, psA, "a", evac_dve=False)

            # ---- Phase C: attention tiles + ctx PSUM chains + partial out ----
            # Software-pipelined: the 8 ctx chains of q-block qb interleave
            # with the first score/exp tiles of qb+1 so neither PE nor the
            # Activation engine idles across the qb boundary.
            cc_view = ctx_cc_in.rearrange("(b o p) q -> p b o q", b=QB, p=P)
            with tc.tile_pool(name="etp", bufs=6) as etp, \
                 tc.tile_pool(name="t8p", bufs=1) as t8p, \
                 tc.tile_pool(name="stg", bufs=3) as stg, \
                 tc.tile_pool(name="psC", bufs=2, space="PSUM") as psC:

                def emit_tile(qb, kc):
                    et = etp.tile([P, H, M], bf16, tag="et")
                    # scores + exp, 3-head waves; head-sum accumulates wave
                    # partials as exps land so `den` is ready ~1 op after the
                    # last wave (keeps ctx chains from stalling on the tree).
                    par = t8p.tile([P, 8, M], bf16, tag="t8")
                    for wi, w0 in enumerate(range(0, H, 3)):
                        hs = list(range(w0, min(w0 + 3, H)))
                        n = len(hs)
                        ps = psC.tile([P, 3 * M], f32, tag="sc")
                        for j, h in enumerate(hs):
                            hp, half = divmod(h, 2)
                            pb = half * DK
                            nc.tensor.matmul(
                                ps[:, j * M : (j + 1) * M],
                                lhsT=kT_sb[pb : pb + DK, hp, kc * P : (kc + 1) * P],
                                rhs=qT_sb[pb : pb + DK, hp, qb * M : (qb + 1) * M],
                                start=True,
                                stop=True,
                            )
                        nc.scalar.activation(
                            et[:, w0 : w0 + n, :], ps[:, : n * M], AF.Exp,
                            scale=0.125,
                        )
                        if n == 3:
                            nc.vector.tensor_tensor(
                                par[:, wi, :], et[:, w0, :], et[:, w0 + 1, :], OP.add
                            )
                            nc.vector.tensor_tensor(
                                par[:, wi, :], par[:, wi, :], et[:, w0 + 2, :], OP.add
                            )
                        if wi == 1:
                            nc.vector.tensor_tensor(
                                par[:, 6, :], par[:, 0, :], par[:, 1, :], OP.add
                            )
                        if wi == 3:
                            nc.vector.tensor_tensor(
                                par[:, 7, :], par[:, 2, :], par[:, 3, :], OP.add
                            )
                    # waves: 5x3 heads + 1 single head (h15)
                    nc.vector.tensor_tensor(
                        par[:, 6, :], par[:, 6, :], par[:, 7, :], OP.add
                    )
                    nc.vector.tensor_tensor(
                        par[:, 4, :], par[:, 4, :], et[:, 15, :], OP.add
                    )
                    den = smal.tile([P, M], f32, tag="den")
                    nc.vector.tensor_tensor(den[:], par[:, 6, :], par[:, 4, :], OP.add)
                    rf = smal.tile([P, M], f32, tag="rf")
                    nc.vector.reciprocal_approx_fast(rf[:], den[:])
                    rb = smal.tile([P, M], bf16, tag="rb")
                    nc.vector.tensor_copy(rb[:], rf[:])
                    # normalize: attn = e * r (in place; split DVE/GpSimd)
                    nsp = 8
                    for h0 in range(0, nsp, 2):
                        nc.vector.tensor_tensor(
                            et[:, h0 : h0 + 2, :],
                            et[:, h0 : h0 + 2, :],
                            rb[:, None, :].to_broadcast((P, 2, M)),
                            OP.mult,
                        )
                    for h0 in range(nsp, H, 2):
                        nc.gpsimd.tensor_tensor(
                            et[:, h0 : h0 + 2, :],
                            et[:, h0 : h0 + 2, :],
                            rb[:, None, :].to_broadcast((P, 2, M)),
                            OP.mult,
                        )
                    return et

                def emit_chain(qb, hp, ets):
                    cps = psC.tile([P, M], f32, tag="cx")
                    for kc in range(KC):
                        nc.tensor.matmul(
                            cps[0:DK, :],
                            lhsT=vt_sb[:, kc, (2 * hp) * DK : (2 * hp + 1) * DK],
                            rhs=ets[kc][:, 2 * hp, :],
                            start=(kc == 0),
                            stop=(kc == KC - 1),
                            tile_position=(0, 0),
                        )
                        nc.tensor.matmul(
                            cps[DK:P, :],
                            lhsT=vt_sb[:, kc, (2 * hp + 1) * DK : (2 * hp + 2) * DK],
                            rhs=ets[kc][:, 2 * hp + 1, :],
                            start=(kc == 0),
                            stop=(kc == KC - 1),
                            tile_position=(0, DK),
                        )
                    ost = stg.tile([P, M], bf16, tag="ost")
                    if hp % 2 == 0:
                        nc.scalar.copy(ost[:], cps[:])
                    else:
                        nc.vector.tensor_copy(ost[:], cps[:])
                    nc.sync.dma_start(cc_view[:, qb, hp, :], ost[:])

                # Prologue: q-block 0 tiles interleaved with Q^T quarters 1-3
                # (those matmuls fill PE slots while Act runs the exps; their
                # chains borrow the idle "cx" PSUM tag, evacs go to DVE).
                ets_cur = [emit_tile(0, 0)]
                emit_qproj_qu(1, psC, "cx", evac_dve=True)
                ets_cur.append(emit_tile(0, 1))
                emit_qproj_qu(2, psC, "cx", evac_dve=False)
                ets_cur.append(emit_tile(0, 2))
                emit_qproj_qu(3, psC, "cx", evac_dve=True)
                ets_cur.append(emit_tile(0, 3))
                for qb in range(QB):
                    ets_next = []
                    for hp in range(HP):
                        emit_chain(qb, hp, ets_cur)
                        # 3-tile lookahead into qb+1 (etp bufs=7 allows it)
                        if qb + 1 < QB and hp in (2, 4, 6):
                            ets_next.append(emit_tile(qb + 1, len(ets_next)))
                    if qb + 1 < QB:
                        while len(ets_next) < KC:
                            ets_next.append(emit_tile(qb + 1, len(ets_next)))
                    ets_cur = ets_next

            # ---- ReduceScatter + Phase D: O-proj + residual + LayerNorm ----
            res_view = xres.rearrange("(o p) e -> o p e", p=P)
            out_view = out.rearrange("(o p) e -> o p e", p=P)
            with tc.tile_pool(name="dpool", bufs=1) as dpool, \
                 tc.tile_pool(name="resp", bufs=4) as resp, \
                 tc.tile_pool(name="lnp", bufs=2) as lnp, \
                 tc.tile_pool(name="psD", bufs=2, space="PSUM") as psD:
                wo_sb = dpool.tile([P, EC, D], bf16)
                nc.sync.dma_start(wo_sb[:], wo.rearrange("(o p) e -> p o e", p=P))
                gam_sb = dpool.tile([P, D], f32)
                nc.sync.dma_start(gam_sb[:], gam[:])
                bet_sb = dpool.tile([P, D], f32)
                nc.sync.dma_start(bet_sb[:], bet[:])
                ctxT_sb = dpool.tile([P, EC, M], bf16)  # summed ctx^T own q
                nc.gpsimd.collective_compute(
                    "ReduceScatter",
                    mybir.AluOpType.add,
                    replica_groups=rg,
                    ins=[ctx_cc_in.opt()],
                    outs=[ctx_cc_out.opt()],
                )
                ccv = ctx_cc_out.rearrange("(o p) q -> p o q", p=P)
                for vc in range(EC):
                    nc.sync.dma_start(ctxT_sb[:, vc, :], ccv[:, vc, :])
                rests = []
                for qc in range(M // P):
                    rest = resp.tile([P, D], f32, tag="res")
                    nc.sync.dma_start(rest[:], res_view[qc])
                    rests.append(rest)
                for qc in range(M // P):
                    rest = rests[qc]
                    ps = psD.tile([P, D], f32, tag="o")
                    for half in range(2):
                        for vc in range(EC):
                            nc.tensor.matmul(
                                ps[:, half * M : (half + 1) * M],
                                lhsT=ctxT_sb[:, vc, qc * P : (qc + 1) * P],
                                rhs=wo_sb[:, vc, half * M : (half + 1) * M],
                                start=(vc == 0),
                                stop=(vc == EC - 1),
                            )
                    # residual add + stats in column halves so downstream
                    # normalize/output passes pipeline (shorter drain)
                    xsb = lnp.tile([P, D], f32, tag="x")
                    bst = smal.tile([P, 2, 6], f32, tag="bst")
                    for h in range(2):
                        nc.vector.tensor_tensor(
                            xsb[:, h * M : (h + 1) * M],
                            ps[:, h * M : (h + 1) * M],
                            rest[:, h * M : (h + 1) * M],
                            OP.add,
                        )
                        nc.vector.bn_stats(bst[:, h, :], xsb[:, h * M : (h + 1) * M])
                    agg = smal.tile([P, 2], f32, tag="agg")
                    nc.vector.bn_aggr(agg[:], bst[:])
                    veps = smal.tile([P, 1], f32, tag="veps")
                    nc.vector.tensor_scalar(
                        veps[:], agg[:, 1:2], 1.0, LN_EPS, OP.mult, OP.add
                    )
                    rvp = smal.tile([P, 1], f32, tag="rvp")
                    nc.vector.reciprocal_approx_fast(rvp[:], veps[:])
                    inv = smal.tile([P, 1], f32, tag="inv")
                    nc.scalar.activation(inv[:], rvp[:], AF.Sqrt)
                    # xn = (x - mu) * inv ; *gamma on DVE, +beta on Pool;
                    # column halves pipeline into the output DMA
                    ot = lnp.tile([P, D], f32, tag="ot")
                    for h in range(2):
                        sl = slice(h * M, (h + 1) * M)
                        nc.vector.tensor_scalar(
                            xsb[:, sl], xsb[:, sl], agg[:, 0:1], inv[:],
                            OP.subtract, OP.mult,
                        )
                        nc.vector.tensor_tensor(
                            xsb[:, sl], xsb[:, sl], gam_sb[:, sl], OP.mult
                        )
                        nc.gpsimd.tensor_tensor(
                            ot[:, sl], xsb[:, sl], bet_sb[:, sl], OP.add
                        )
                        nc.sync.dma_start(out_view[qc][:, sl], ot[:, sl])

    nc.compile()
    return nc


def _get_nc():
    if "nc" not in _CACHE:
        _CACHE["nc"] = _build()
    return _CACHE["nc"]


def _in_maps(input_Q, input_K, input_V, W_Q, W_K, W_V, W_O, ln_gamma, ln_beta):
    bf = ml_dtypes.bfloat16
    f32 = np.float32
    Q_ = np.asarray(input_Q, dtype=f32)
    K_ = np.asarray(input_K, dtype=f32)
    V_ = np.asarray(input_V, dtype=f32)
    wq_b = np.asarray(W_Q, dtype=f32).astype(bf)
    wk_b = np.asarray(W_K, dtype=f32).astype(bf)
    wv_b = np.asarray(W_V, dtype=f32).astype(bf)
    wo_b = np.asarray(W_O, dtype=f32).astype(bf)
    gam_b = np.ascontiguousarray(
        np.broadcast_to(np.asarray(ln_gamma, dtype=f32), (P, D))
    )
    bet_b = np.ascontiguousarray(
        np.broadcast_to(np.asarray(ln_beta, dtype=f32), (P, D))
    )
    maps = []
    for c in range(8):
        b, r = divmod(c, G)
        sl = slice(r * M, (r + 1) * M)
        maps.append(
            {
                "xqT": np.ascontiguousarray(Q_[b].T).astype(bf),
                "xkT": np.ascontiguousarray(K_[b, sl].T).astype(bf),
                "xvT": np.ascontiguousarray(V_[b, sl].T).astype(bf),
                "xres": np.ascontiguousarray(Q_[b, sl]),
                "wq": wq_b,
                "wk": wk_b,
                "wv": wv_b,
                "wo": wo_b,
                "gam": gam_b,
                "bet": bet_b,
            }
        )
    return maps


def _assemble(results):
    B = 2
    out = np.empty((B, SK, D), np.float32)
    for c in range(8):
        b, r = divmod(c, G)
        out[b, r * M : (r + 1) * M] = results[c]["out"]
    return out


def run_traced(trace=False, **inputs):
    """Run on HW; returns (output, BassKernelResults)."""
    from concourse.bass_utils import run_bass_kernel_spmd

    nc = _get_nc()
    maps = _in_maps(**inputs)
    res = run_bass_kernel_spmd(nc, maps, list(range(8)), trace=trace)
    return _assemble(res.results), res


def kernel(**inputs) -> np.ndarray:
    out, _ = run_traced(trace=False, **inputs)
    return out


# revision 11
# speedup vs baseline: 1.1001x; 1.0008x over previous
"""Trainium2 Bass kernel for nn_MultiHeadAttention (head-axis softmax quirk).

v2 strategy (8 NeuronCores, 2 batch-groups x 4 cores):
  - NO pre-attention collectives. Softmax over HEADS is local per (q,k), so
    attention is sharded over the K/V length: each core projects its own
    512-row k-slice of K and V, and redundantly projects Q for the whole
    group's 2048 q rows (+41us PE, but saves ~240us of AllGather).
  - Per core: scores^T [k,q] tiles (128k x 512q x 16h), exp on ScalarE,
    head-sum via DVE tree-adds (bf16 4x mode), normalize split DVE/GpSimd,
    context accumulated across the 4 local k-chunks in PSUM chains (no
    SBUF accumulation traffic).
  - One bf16 ReduceScatter of ctx^T partials (4MB in -> 1MB out) hands each
    core the k-summed context for its own 512 q rows.
  - Output projection + residual + LayerNorm on own q rows.
  - All matmuls bf16 with fp32 PSUM accumulation.
"""

import numpy as np
import ml_dtypes

D = 1024
H = 16
DK = 64
P = 128
M = 512  # q rows per core (own slice) / k rows per core
SK = 2048
G = 4  # cores per batch group
GQ = 2048  # q rows per group
EC = D // P  # 8
HP = H // 2  # 8
KC = M // P  # 4 local k chunks
QB = GQ // M  # 4 q blocks (one per destination core)
LN_EPS = 1e-5

_CACHE = {}


def _build():
    import concourse.bass as bass
    import concourse.mybir as mybir
    import concourse.tile as tile
    from concourse import bacc

    f32 = mybir.dt.float32
    bf16 = mybir.dt.bfloat16
    AF = mybir.ActivationFunctionType
    OP = mybir.AluOpType
    AX = mybir.AxisListType

    nc = bacc.Bacc("TRN2", target_bir_lowering=False, debug=False, num_devices=8)

    xqT = nc.dram_tensor("xqT", [D, GQ], bf16, kind="ExternalInput").ap()
    xkT = nc.dram_tensor("xkT", [D, M], bf16, kind="ExternalInput").ap()
    xvT = nc.dram_tensor("xvT", [D, M], bf16, kind="ExternalInput").ap()
    xres = nc.dram_tensor("xres", [M, D], f32, kind="ExternalInput").ap()
    wq = nc.dram_tensor("wq", [D, D], bf16, kind="ExternalInput").ap()
    wk = nc.dram_tensor("wk", [D, D], bf16, kind="ExternalInput").ap()
    wv = nc.dram_tensor("wv", [D, D], bf16, kind="ExternalInput").ap()
    wo = nc.dram_tensor("wo", [D, D], bf16, kind="ExternalInput").ap()
    gam = nc.dram_tensor("gam", [P, D], f32, kind="ExternalInput").ap()
    bet = nc.dram_tensor("bet", [P, D], f32, kind="ExternalInput").ap()
    out = nc.dram_tensor("out", [M, D], f32, kind="ExternalOutput").ap()

    rg = [[0, 1, 2, 3], [4, 5, 6, 7]]

    from contextlib import ExitStack

    with tile.TileContext(nc) as tc:
        with ExitStack() as ctx:
            const = ctx.enter_context(tc.tile_pool(name="const", bufs=1))
            smal = ctx.enter_context(tc.tile_pool(name="smal", bufs=4))
            dram = ctx.enter_context(tc.tile_pool(name="dram", bufs=1, space="DRAM"))

            # persistent tiles
            kT_sb = const.tile([P, HP, M], bf16)  # K^T own slice, dk-pair packed
            vt_sb = const.tile([P, KC, D], bf16)  # V own slice [k-chunk, v]
            qT_sb = const.tile([P, HP, GQ], bf16)  # Q^T whole group

            ctx_cc_in = dram.tile([QB * D, M], bf16)
            ctx_cc_out = dram.tile([D, M], bf16)

            # ---- Phase A: K^T, V, and first-quarter Q^T projections ----
            # Q quarters 1-3 are emitted inside Phase C where PE has idle
            # slots (C is Activation-paced); wq/xq pools outlive Phase A.
            wq_p = ctx.enter_context(tc.tile_pool(name="wq_p", bufs=1))
            xq_p = ctx.enter_context(tc.tile_pool(name="xq_p", bufs=2))
            wq_sb = wq_p.tile([P, EC, D], bf16)
            xqv = xqT.rearrange("(o p) q -> p o q", p=P)

            def emit_qproj_qu(qu, pool, tag, evac_dve):
                xq_qu = xq_p.tile([P, EC, M], bf16, tag="xq")
                nc.sync.dma_start(xq_qu[:], xqv[:, :, qu * M : (qu + 1) * M])
                for hp in range(HP):
                    ps = pool.tile([P, M] if tag == "cx" else [P, GQ], f32, tag=tag)
                    for dc in range(EC):
                        nc.tensor.matmul(
                            ps[:, 0:M],
                            lhsT=wq_sb[:, dc, hp * P : (hp + 1) * P],
                            rhs=xq_qu[:, dc, :],
                            start=(dc == 0),
                            stop=(dc == EC - 1),
                        )
                    dst = qT_sb[:, hp, qu * M : (qu + 1) * M]
                    if evac_dve:
                        nc.vector.tensor_copy(dst, ps[:, 0:M])
                    else:
                        nc.scalar.copy(dst, ps[:, 0:M])

            with tc.tile_pool(name="wpool", bufs=2) as wpool, \
                 tc.tile_pool(name="xk_p", bufs=1) as xk_p, \
                 tc.tile_pool(name="xv_p", bufs=1) as xv_p, \
                 tc.tile_pool(name="psA", bufs=2, space="PSUM") as psA:
                # Lead-in DMAs spread across engine DGE queues so wk/xk (the
                # Kproj gate) stream in parallel instead of serializing.
                wk_view = wk.rearrange("(o p) e -> p o e", p=P)
                wk_sb = wpool.tile([P, EC, D], bf16, tag="w")
                nc.sync.dma_start(wk_sb[:, 0:4, :], wk_view[:, 0:4, :])
                nc.scalar.dma_start(wk_sb[:, 4:EC, :], wk_view[:, 4:EC, :])
                xk_sb = xk_p.tile([P, EC, M], bf16)
                nc.gpsimd.dma_start(xk_sb[:], xkT.rearrange("(o p) q -> p o q", p=P))
                wv_sb = wpool.tile([P, EC, D], bf16, tag="w")
                nc.gpsimd.dma_start(wv_sb[:], wv.rearrange("(o p) e -> p o e", p=P))
                xv_sb = xv_p.tile([P, EC, M], bf16)
                nc.sync.dma_start(xv_sb[:], xvT.rearrange("(o p) q -> p o q", p=P))
                nc.sync.dma_start(wq_sb[:], wq.rearrange("(o p) e -> p o e", p=P))

                # K^T projection: own 512 k rows -> kT_sb [P, hp, 512]
                for w0 in range(0, HP, 3):
                    hps = list(range(w0, min(w0 + 3, HP)))
                    n = len(hps)
                    ps = psA.tile([P, GQ], f32, tag="a")
                    for j, hp in enumerate(hps):
                        for dc in range(EC):
                            nc.tensor.matmul(
                                ps[:, j * M : (j + 1) * M],
                                lhsT=wk_sb[:, dc, hp * P : (hp + 1) * P],
                                rhs=xk_sb[:, dc, :],
                                start=(dc == 0),
                                stop=(dc == EC - 1),
                            )
                    nc.scalar.copy(kT_sb[:, w0 : w0 + n, :], ps[:, : n * M])

                # V projection: own 512 k rows -> vt_sb [P(k), kc, 1024(v)]
                for kc in range(KC):
                    ps = psA.tile([P, GQ], f32, tag="a")
                    for half in range(2):
                        for dc in range(EC):
                            nc.tensor.matmul(
                                ps[:, half * M : (half + 1) * M],
                                lhsT=xv_sb[:, dc, kc * P : (kc + 1) * P],
                                rhs=wv_sb[:, dc, half * M : (half + 1) * M],
                                start=(dc == 0),
                                stop=(dc == EC - 1),
                            )
                    nc.scalar.copy(vt_sb[:, kc, :], ps[:, :D])

                # Q^T projection, first quarter (covers q-block 0)
                emit_qproj_qu(0, psA, "a", evac_dve=False)

            # ---- Phase C: attention tiles + ctx PSUM chains + partial out ----
            # Software-pipelined: the 8 ctx chains of q-block qb interleave
            # with the first score/exp tiles of qb+1 so neither PE nor the
            # Activation engine idles across the qb boundary.
            cc_view = ctx_cc_in.rearrange("(b o p) q -> p b o q", b=QB, p=P)
            with tc.tile_pool(name="etp", bufs=6) as etp, \
                 tc.tile_pool(name="t8p", bufs=1) as t8p, \
                 tc.tile_pool(name="stg", bufs=3) as stg, \
                 tc.tile_pool(name="psC", bufs=2, space="PSUM") as psC:

                def emit_tile(qb, kc):
                    et = etp.tile([P, H, M], bf16, tag="et")
                    # scores + exp, 3-head waves; head-sum accumulates wave
                    # partials as exps land so `den` is ready ~1 op after the
                    # last wave (keeps ctx chains from stalling on the tree).
                    par = t8p.tile([P, 8, M], bf16, tag="t8")
                    for wi, w0 in enumerate(range(0, H, 3)):
                        hs = list(range(w0, min(w0 + 3, H)))
                        n = len(hs)
                        ps = psC.tile([P, 3 * M], f32, tag="sc")
                        for j, h in enumerate(hs):
                            hp, half = divmod(h, 2)
                            pb = half * DK
                            nc.tensor.matmul(
                                ps[:, j * M : (j + 1) * M],
                                lhsT=kT_sb[pb : pb + DK, hp, kc * P : (kc + 1) * P],
                                rhs=qT_sb[pb : pb + DK, hp, qb * M : (qb + 1) * M],
                                start=True,
                                stop=True,
                            )
                        nc.scalar.activation(
                            et[:, w0 : w0 + n, :], ps[:, : n * M], AF.Exp,
                            scale=0.125,
                        )
                        if n == 3:
                            nc.vector.tensor_tensor(
                                par[:, wi, :], et[:, w0, :], et[:, w0 + 1, :], OP.add
                            )
                            nc.vector.tensor_tensor(
                                par[:, wi, :], par[:, wi, :], et[:, w0 + 2, :], OP.add
                            )
                        if wi == 1:
                            nc.vector.tensor_tensor(
                                par[:, 6, :], par[:, 0, :], par[:, 1, :], OP.add
                            )
                        if wi == 3:
                            nc.vector.tensor_tensor(
                                par[:, 7, :], par[:, 2, :], par[:, 3, :], OP.add
                            )
                    # waves: 5x3 heads + 1 single head (h15)
                    nc.vector.tensor_tensor(
                        par[:, 6, :], par[:, 6, :], par[:, 7, :], OP.add
                    )
                    nc.vector.tensor_tensor(
                        par[:, 4, :], par[:, 4, :], et[:, 15, :], OP.add
                    )
                    den = smal.tile([P, M], f32, tag="den")
                    nc.vector.tensor_tensor(den[:], par[:, 6, :], par[:, 4, :], OP.add)
                    rf = smal.tile([P, M], f32, tag="rf")
                    nc.vector.reciprocal_approx_fast(rf[:], den[:])
                    rb = smal.tile([P, M], bf16, tag="rb")
                    nc.vector.tensor_copy(rb[:], rf[:])
                    # normalize: attn = e * r (in place; split DVE/GpSimd)
                    nsp = 8
                    for h0 in range(0, nsp, 2):
                        nc.vector.tensor_tensor(
                            et[:, h0 : h0 + 2, :],
                            et[:, h0 : h0 + 2, :],
                            rb[:, None, :].to_broadcast((P, 2, M)),
                            OP.mult,
                        )
                    for h0 in range(nsp, H, 2):
                        nc.gpsimd.tensor_tensor(
                            et[:, h0 : h0 + 2, :],
                            et[:, h0 : h0 + 2, :],
                            rb[:, None, :].to_broadcast((P, 2, M)),
                            OP.mult,
                        )
                    return et

                def emit_chain(qb, hp, ets):
                    cps = psC.tile([P, M], f32, tag="cx")
                    for kc in range(KC):
                        nc.tensor.matmul(
                            cps[0:DK, :],
                            lhsT=vt_sb[:, kc, (2 * hp) * DK : (2 * hp + 1) * DK],
                            rhs=ets[kc][:, 2 * hp, :],
                            start=(kc == 0),
                            stop=(kc == KC - 1),
                            tile_position=(0, 0),
                        )
                        nc.tensor.matmul(
                            cps[DK:P, :],
                            lhsT=vt_sb[:, kc, (2 * hp + 1) * DK : (2 * hp + 2) * DK],
                            rhs=ets[kc][:, 2 * hp + 1, :],
                            start=(kc == 0),
                            stop=(kc == KC - 1),
                            tile_position=(0, DK),
                        )
                    ost = stg.tile([P, M], bf16, tag="ost")
                    if hp % 2 == 0:
                        nc.scalar.copy(ost[:], cps[:])
                    else:
                        nc.vector.tensor_copy(ost[:], cps[:])
                    nc.sync.dma_start(cc_view[:, qb, hp, :], ost[:])

                # Prologue: q-block 0 tiles interleaved with Q^T quarters 1-3
                # (those matmuls fill PE slots while Act runs the exps; their
                # chains borrow the idle "cx" PSUM tag, evacs go to DVE).
                ets_cur = [emit_tile(0, 0)]
                emit_qproj_qu(1, psC, "cx", evac_dve=True)
                ets_cur.append(emit_tile(0, 1))
                emit_qproj_qu(2, psC, "cx", evac_dve=False)
                ets_cur.append(emit_tile(0, 2))
                emit_qproj_qu(3, psC, "cx", evac_dve=True)
                ets_cur.append(emit_tile(0, 3))
                for qb in range(QB):
                    ets_next = []
                    for hp in range(HP):
                        emit_chain(qb, hp, ets_cur)
                        # 3-tile lookahead into qb+1 (etp bufs=7 allows it)
                        if qb + 1 < QB and hp in (2, 4, 6):
                            ets_next.append(emit_tile(qb + 1, len(ets_next)))
                    if qb + 1 < QB:
                        while len(ets_next) < KC:
                            ets_next.append(emit_tile(qb + 1, len(ets_next)))
                    ets_cur = ets_next

            # ---- ReduceScatter + Phase D: O-proj + residual + LayerNorm ----
            res_view = xres.rearrange("(o p) e -> o p e", p=P)
            out_view = out.rearrange("(o p) e -> o p e", p=P)
            with tc.tile_pool(name="dpool", bufs=1) as dpool, \
                 tc.tile_pool(name="resp", bufs=4) as resp, \
                 tc.tile_pool(name="lnp", bufs=2) as lnp, \
                 tc.tile_pool(name="psD", bufs=2, space="PSUM") as psD:
                wo_sb = dpool.tile([P, EC, D], bf16)
                nc.sync.dma_start(wo_sb[:], wo.rearrange("(o p) e -> p o e", p=P))
                gam_sb = dpool.tile([P, D], f32)
                nc.sync.dma_start(gam_sb[:], gam[:])
                bet_sb = dpool.tile([P, D], f32)
                nc.sync.dma_start(bet_sb[:], bet[:])
                ctxT_sb = dpool.tile([P, EC, M], bf16)  # summed ctx^T own q
                nc.gpsimd.collective_compute(
                    "ReduceScatter",
                    mybir.AluOpType.add,
                    replica_groups=rg,
                    ins=[ctx_cc_in.opt()],
                    outs=[ctx_cc_out.opt()],
                )
                ccv = ctx_cc_out.rearrange("(o p) q -> p o q", p=P)
                for vc in range(EC):
                    nc.sync.dma_start(ctxT_sb[:, vc, :], ccv[:, vc, :])
                rests = []
                for qc in range(M // P):
                    rest = resp.tile([P, D], f32, tag="res")
                    nc.sync.dma_start(rest[:], res_view[qc])
                    rests.append(rest)
                for qc in range(M // P):
                    rest = rests[qc]
                    ps = psD.tile([P, D], f32, tag="o")
                    for half in range(2):
                        for vc in range(EC):
                            nc.tensor.matmul(
                                ps[:, half * M : (half + 1) * M],
                                lhsT=ctxT_sb[:, vc, qc * P : (qc + 1) * P],
                                rhs=wo_sb[:, vc, half * M : (half + 1) * M],
                                start=(vc == 0),
                                stop=(vc == EC - 1),
                            )
                    # residual add + stats in column halves so downstream
                    # normalize/output passes pipeline (shorter drain)
                    xsb = lnp.tile([P, D], f32, tag="x")
                    bst = smal.tile([P, 2, 6], f32, tag="bst")
                    for h in range(2):
                        nc.vector.tensor_tensor(
                            xsb[:, h * M : (h + 1) * M],
                            ps[:, h * M : (h + 1) * M],
                            rest[:, h * M : (h + 1) * M],
                            OP.add,
                        )
                        nc.vector.bn_stats(bst[:, h, :], xsb[:, h * M : (h + 1) * M])
                    agg = smal.tile([P, 2], f32, tag="agg")
                    nc.vector.bn_aggr(agg[:], bst[:])
                    veps = smal.tile([P, 1], f32, tag="veps")
                    nc.vector.tensor_scalar(
                        veps[:], agg[:, 1:2], 1.0, LN_EPS, OP.mult, OP.add
                    )
                    rvp = smal.tile([P, 1], f32, tag="rvp")
                    nc.vector.reciprocal_approx_fast(rvp[:], veps[:])
                    inv = smal.tile([P, 1], f32, tag="inv")
                    nc.scalar.activation(inv[:], rvp[:], AF.Sqrt)
                    # xn = (x - mu) * inv ; *gamma on DVE, +beta on Pool;
                    # column halves pipeline into the output DMA
                    ot = lnp.tile([P, D], f32, tag="ot")
                    for h in range(2):
                        sl = slice(h * M, (h + 1) * M)
                        nc.vector.tensor_scalar(
                            xsb[:, sl], xsb[:, sl], agg[:, 0:1], inv[:],
                            OP.subtract, OP.mult,
                        )
                        nc.vector.tensor_tensor(
                            xsb[:, sl], xsb[:, sl], gam_sb[:, sl], OP.mult
                        )
                        nc.gpsimd.tensor_tensor(
                            ot[:, sl], xsb[:, sl], bet_sb[:, sl], OP.add
                        )
                        nc.sync.dma_start(out_view[qc][:, sl], ot[:, sl])

    nc.compile()
    return nc


def _get_nc():
    if "nc" not in _CACHE:
        _CACHE["nc"] = _build()
    return _CACHE["nc"]


def _in_maps(input_Q, input_K, input_V, W_Q, W_K, W_V, W_O, ln_gamma, ln_beta):
    bf = ml_dtypes.bfloat16
    f32 = np.float32
    Q_ = np.asarray(input_Q, dtype=f32)
    K_ = np.asarray(input_K, dtype=f32)
    V_ = np.asarray(input_V, dtype=f32)
    wq_b = np.asarray(W_Q, dtype=f32).astype(bf)
    wk_b = np.asarray(W_K, dtype=f32).astype(bf)
    wv_b = np.asarray(W_V, dtype=f32).astype(bf)
    wo_b = np.asarray(W_O, dtype=f32).astype(bf)
    gam_b = np.ascontiguousarray(
        np.broadcast_to(np.asarray(ln_gamma, dtype=f32), (P, D))
    )
    bet_b = np.ascontiguousarray(
        np.broadcast_to(np.asarray(ln_beta, dtype=f32), (P, D))
    )
    maps = []
    for c in range(8):
        b, r = divmod(c, G)
        sl = slice(r * M, (r + 1) * M)
        maps.append(
            {
                "xqT": np.ascontiguousarray(Q_[b].T).astype(bf),
                "xkT": np.ascontiguousarray(K_[b, sl].T).astype(bf),
                "xvT": np.ascontiguousarray(V_[b, sl].T).astype(bf),
                "xres": np.ascontiguousarray(Q_[b, sl]),
                "wq": wq_b,
                "wk": wk_b,
                "wv": wv_b,
                "wo": wo_b,
                "gam": gam_b,
                "bet": bet_b,
            }
        )
    return maps


def _assemble(results):
    B = 2
    out = np.empty((B, SK, D), np.float32)
    for c in range(8):
        b, r = divmod(c, G)
        out[b, r * M : (r + 1) * M] = results[c]["out"]
    return out


def run_traced(trace=False, **inputs):
    """Run on HW; returns (output, BassKernelResults)."""
    from concourse.bass_utils import run_bass_kernel_spmd

    nc = _get_nc()
    maps = _in_maps(**inputs)
    res = run_bass_kernel_spmd(nc, maps, list(range(8)), trace=trace)
    return _assemble(res.results), res


def kernel(**inputs) -> np.ndarray:
    out, _ = run_traced(trace=False, **inputs)
    return out


# revision 12
# speedup vs baseline: 1.1053x; 1.0047x over previous
"""Trainium2 Bass kernel for nn_MultiHeadAttention (head-axis softmax quirk).

v2 strategy (8 NeuronCores, 2 batch-groups x 4 cores):
  - NO pre-attention collectives. Softmax over HEADS is local per (q,k), so
    attention is sharded over the K/V length: each core projects its own
    512-row k-slice of K and V, and redundantly projects Q for the whole
    group's 2048 q rows (+41us PE, but saves ~240us of AllGather).
  - Per core: scores^T [k,q] tiles (128k x 512q x 16h), exp on ScalarE,
    head-sum via DVE tree-adds (bf16 4x mode), normalize split DVE/GpSimd,
    context accumulated across the 4 local k-chunks in PSUM chains (no
    SBUF accumulation traffic).
  - One bf16 ReduceScatter of ctx^T partials (4MB in -> 1MB out) hands each
    core the k-summed context for its own 512 q rows.
  - Output projection + residual + LayerNorm on own q rows.
  - All matmuls bf16 with fp32 PSUM accumulation.
"""

import numpy as np
import ml_dtypes

D = 1024
H = 16
DK = 64
P = 128
M = 512  # q rows per core (own slice) / k rows per core
SK = 2048
G = 4  # cores per batch group
GQ = 2048  # q rows per group
EC = D // P  # 8
HP = H // 2  # 8
KC = M // P  # 4 local k chunks
QB = GQ // M  # 4 q blocks (one per destination core)
LN_EPS = 1e-5

_CACHE = {}


def _build():
    import concourse.bass as bass
    import concourse.mybir as mybir
    import concourse.tile as tile
    from concourse import bacc

    f32 = mybir.dt.float32
    bf16 = mybir.dt.bfloat16
    AF = mybir.ActivationFunctionType
    OP = mybir.AluOpType
    AX = mybir.AxisListType

    nc = bacc.Bacc("TRN2", target_bir_lowering=False, debug=False, num_devices=8)

    xqT = nc.dram_tensor("xqT", [D, GQ], bf16, kind="ExternalInput").ap()
    xkT = nc.dram_tensor("xkT", [D, M], bf16, kind="ExternalInput").ap()
    xvT = nc.dram_tensor("xvT", [D, M], bf16, kind="ExternalInput").ap()
    xres = nc.dram_tensor("xres", [M, D], f32, kind="ExternalInput").ap()
    wq = nc.dram_tensor("wq", [D, D], bf16, kind="ExternalInput").ap()
    wk = nc.dram_tensor("wk", [D, D], bf16, kind="ExternalInput").ap()
    wv = nc.dram_tensor("wv", [D, D], bf16, kind="ExternalInput").ap()
    wo = nc.dram_tensor("wo", [D, D], bf16, kind="ExternalInput").ap()
    gam = nc.dram_tensor("gam", [P, D], f32, kind="ExternalInput").ap()
    bet = nc.dram_tensor("bet", [P, D], f32, kind="ExternalInput").ap()
    out = nc.dram_tensor("out", [M, D], f32, kind="ExternalOutput").ap()

    rg = [[0, 1, 2, 3], [4, 5, 6, 7]]

    from contextlib import ExitStack

    with tile.TileContext(nc) as tc:
        with ExitStack() as ctx:
            const = ctx.enter_context(tc.tile_pool(name="const", bufs=1))
            smal = ctx.enter_context(tc.tile_pool(name="smal", bufs=4))
            dram = ctx.enter_context(tc.tile_pool(name="dram", bufs=1, space="DRAM"))

            # persistent tiles
            kT_sb = const.tile([P, HP, M], bf16)  # K^T own slice, dk-pair packed
            vt_sb = const.tile([P, KC, D], bf16)  # V own slice [k-chunk, v]
            qT_sb = const.tile([P, HP, GQ], bf16)  # Q^T whole group

            ctx_cc_in = dram.tile([QB * D, M], bf16)
            ctx_cc_out = dram.tile([D, M], bf16)

            # ---- Phase A: K^T, V, and first-quarter Q^T projections ----
            # Q quarters 1-3 are emitted inside Phase C where PE has idle
            # slots (C is Activation-paced); wq/xq pools outlive Phase A.
            wq_p = ctx.enter_context(tc.tile_pool(name="wq_p", bufs=1))
            xq_p = ctx.enter_context(tc.tile_pool(name="xq_p", bufs=2))
            wq_sb = wq_p.tile([P, EC, D], bf16)
            xqv = xqT.rearrange("(o p) q -> p o q", p=P)

            def emit_qproj_qu(qu, pool, tag, evac_dve):
                xq_qu = xq_p.tile([P, EC, M], bf16, tag="xq")
                nc.sync.dma_start(xq_qu[:], xqv[:, :, qu * M : (qu + 1) * M])
                for hp in range(HP):
                    ps = pool.tile([P, M] if tag == "cx" else [P, GQ], f32, tag=tag)
                    for dc in range(EC):
                        nc.tensor.matmul(
                            ps[:, 0:M],
                            lhsT=wq_sb[:, dc, hp * P : (hp + 1) * P],
                            rhs=xq_qu[:, dc, :],
                            start=(dc == 0),
                            stop=(dc == EC - 1),
                        )
                    dst = qT_sb[:, hp, qu * M : (qu + 1) * M]
                    if evac_dve:
                        nc.vector.tensor_copy(dst, ps[:, 0:M])
                    else:
                        nc.scalar.copy(dst, ps[:, 0:M])

            with tc.tile_pool(name="wpool", bufs=2) as wpool, \
                 tc.tile_pool(name="xk_p", bufs=1) as xk_p, \
                 tc.tile_pool(name="xv_p", bufs=1) as xv_p, \
                 tc.tile_pool(name="psA", bufs=2, space="PSUM") as psA:
                # Lead-in DMAs spread across engine DGE queues so wk/xk (the
                # Kproj gate) stream in parallel instead of serializing.
                wk_view = wk.rearrange("(o p) e -> p o e", p=P)
                wk_sb = wpool.tile([P, EC, D], bf16, tag="w")
                nc.sync.dma_start(wk_sb[:, 0:4, :], wk_view[:, 0:4, :])
                nc.scalar.dma_start(wk_sb[:, 4:EC, :], wk_view[:, 4:EC, :])
                xk_sb = xk_p.tile([P, EC, M], bf16)
                nc.gpsimd.dma_start(xk_sb[:], xkT.rearrange("(o p) q -> p o q", p=P))
                wv_sb = wpool.tile([P, EC, D], bf16, tag="w")
                nc.gpsimd.dma_start(wv_sb[:], wv.rearrange("(o p) e -> p o e", p=P))
                xv_sb = xv_p.tile([P, EC, M], bf16)
                nc.sync.dma_start(xv_sb[:], xvT.rearrange("(o p) q -> p o q", p=P))
                nc.sync.dma_start(wq_sb[:], wq.rearrange("(o p) e -> p o e", p=P))

                # K^T projection: own 512 k rows -> kT_sb [P, hp, 512]
                for w0 in range(0, HP, 3):
                    hps = list(range(w0, min(w0 + 3, HP)))
                    n = len(hps)
                    ps = psA.tile([P, GQ], f32, tag="a")
                    for j, hp in enumerate(hps):
                        for dc in range(EC):
                            nc.tensor.matmul(
                                ps[:, j * M : (j + 1) * M],
                                lhsT=wk_sb[:, dc, hp * P : (hp + 1) * P],
                                rhs=xk_sb[:, dc, :],
                                start=(dc == 0),
                                stop=(dc == EC - 1),
                            )
                    nc.scalar.copy(kT_sb[:, w0 : w0 + n, :], ps[:, : n * M])

                # V projection: own 512 k rows -> vt_sb [P(k), kc, 1024(v)]
                for kc in range(KC):
                    ps = psA.tile([P, GQ], f32, tag="a")
                    for half in range(2):
                        for dc in range(EC):
                            nc.tensor.matmul(
                                ps[:, half * M : (half + 1) * M],
                                lhsT=xv_sb[:, dc, kc * P : (kc + 1) * P],
                                rhs=wv_sb[:, dc, half * M : (half + 1) * M],
                                start=(dc == 0),
                                stop=(dc == EC - 1),
                            )
                    nc.scalar.copy(vt_sb[:, kc, :], ps[:, :D])

                # Q^T projection, first quarter (covers q-block 0)
                emit_qproj_qu(0, psA, "a", evac_dve=False)

            # ---- Phase C: attention tiles + ctx PSUM chains + partial out ----
            # Software-pipelined: the 8 ctx chains of q-block qb interleave
            # with the first score/exp tiles of qb+1 so neither PE nor the
            # Activation engine idles across the qb boundary.
            cc_view = ctx_cc_in.rearrange("(b o p) q -> p b o q", b=QB, p=P)
            with tc.tile_pool(name="etp", bufs=6) as etp, \
                 tc.tile_pool(name="t8p", bufs=1) as t8p, \
                 tc.tile_pool(name="stg", bufs=3) as stg, \
                 tc.tile_pool(name="psC", bufs=2, space="PSUM") as psC:

                def emit_tile(qb, kc):
                    et = etp.tile([P, H, M], bf16, tag="et")
                    # scores + exp, 3-head waves; head-sum accumulates wave
                    # partials as exps land so `den` is ready ~1 op after the
                    # last wave (keeps ctx chains from stalling on the tree).
                    par = t8p.tile([P, 8, M], bf16, tag="t8")
                    for wi, w0 in enumerate(range(0, H, 3)):
                        hs = list(range(w0, min(w0 + 3, H)))
                        n = len(hs)
                        ps = psC.tile([P, 3 * M], f32, tag="sc")
                        for j, h in enumerate(hs):
                            hp, half = divmod(h, 2)
                            pb = half * DK
                            nc.tensor.matmul(
                                ps[:, j * M : (j + 1) * M],
                                lhsT=kT_sb[pb : pb + DK, hp, kc * P : (kc + 1) * P],
                                rhs=qT_sb[pb : pb + DK, hp, qb * M : (qb + 1) * M],
                                start=True,
                                stop=True,
                            )
                        nc.scalar.activation(
                            et[:, w0 : w0 + n, :], ps[:, : n * M], AF.Exp,
                            scale=0.125,
                        )
                        if n == 3:
                            nc.vector.tensor_tensor(
                                par[:, wi, :], et[:, w0, :], et[:, w0 + 1, :], OP.add
                            )
                            nc.vector.tensor_tensor(
                                par[:, wi, :], par[:, wi, :], et[:, w0 + 2, :], OP.add
                            )
                        if wi == 1:
                            nc.vector.tensor_tensor(
                                par[:, 6, :], par[:, 0, :], par[:, 1, :], OP.add
                            )
                        if wi == 3:
                            nc.vector.tensor_tensor(
                                par[:, 7, :], par[:, 2, :], par[:, 3, :], OP.add
                            )
                    # waves: 5x3 heads + 1 single head (h15)
                    nc.vector.tensor_tensor(
                        par[:, 6, :], par[:, 6, :], par[:, 7, :], OP.add
                    )
                    nc.vector.tensor_tensor(
                        par[:, 4, :], par[:, 4, :], et[:, 15, :], OP.add
                    )
                    den = smal.tile([P, M], f32, tag="den")
                    nc.vector.tensor_tensor(den[:], par[:, 6, :], par[:, 4, :], OP.add)
                    rf = smal.tile([P, M], f32, tag="rf")
                    nc.vector.reciprocal_approx_fast(rf[:], den[:])
                    rb = smal.tile([P, M], bf16, tag="rb")
                    nc.vector.tensor_copy(rb[:], rf[:])
                    # normalize: attn = e * r (in place; split DVE/GpSimd)
                    nsp = 8
                    for h0 in range(0, nsp, 2):
                        nc.vector.tensor_tensor(
                            et[:, h0 : h0 + 2, :],
                            et[:, h0 : h0 + 2, :],
                            rb[:, None, :].to_broadcast((P, 2, M)),
                            OP.mult,
                        )
                    for h0 in range(nsp, H, 2):
                        nc.gpsimd.tensor_tensor(
                            et[:, h0 : h0 + 2, :],
                            et[:, h0 : h0 + 2, :],
                            rb[:, None, :].to_broadcast((P, 2, M)),
                            OP.mult,
                        )
                    return et

                def emit_chain(qb, hp, ets):
                    cps = psC.tile([P, M], f32, tag="cx")
                    for kc in range(KC):
                        nc.tensor.matmul(
                            cps[0:DK, :],
                            lhsT=vt_sb[:, kc, (2 * hp) * DK : (2 * hp + 1) * DK],
                            rhs=ets[kc][:, 2 * hp, :],
                            start=(kc == 0),
                            stop=(kc == KC - 1),
                            tile_position=(0, 0),
                        )
                        nc.tensor.matmul(
                            cps[DK:P, :],
                            lhsT=vt_sb[:, kc, (2 * hp + 1) * DK : (2 * hp + 2) * DK],
                            rhs=ets[kc][:, 2 * hp + 1, :],
                            start=(kc == 0),
                            stop=(kc == KC - 1),
                            tile_position=(0, DK),
                        )
                    ost = stg.tile([P, M], bf16, tag="ost")
                    if hp % 2 == 0:
                        nc.scalar.copy(ost[:], cps[:])
                    else:
                        nc.vector.tensor_copy(ost[:], cps[:])
                    nc.sync.dma_start(cc_view[:, qb, hp, :], ost[:])

                # Prologue: q-block 0 tiles interleaved with Q^T quarters 1-3
                # (those matmuls fill PE slots while Act runs the exps; their
                # chains borrow the idle "cx" PSUM tag, evacs go to DVE).
                ets_cur = [emit_tile(0, 0)]
                emit_qproj_qu(1, psC, "cx", evac_dve=True)
                ets_cur.append(emit_tile(0, 1))
                emit_qproj_qu(2, psC, "cx", evac_dve=False)
                ets_cur.append(emit_tile(0, 2))
                emit_qproj_qu(3, psC, "cx", evac_dve=True)
                ets_cur.append(emit_tile(0, 3))
                for qb in range(QB):
                    ets_next = []
                    for hp in range(HP):
                        emit_chain(qb, hp, ets_cur)
                        # 3-tile lookahead into qb+1 (etp bufs=7 allows it)
                        if qb + 1 < QB and hp in (2, 4, 6):
                            ets_next.append(emit_tile(qb + 1, len(ets_next)))
                    if qb + 1 < QB:
                        while len(ets_next) < KC:
                            ets_next.append(emit_tile(qb + 1, len(ets_next)))
                    ets_cur = ets_next

            # ---- ReduceScatter + Phase D: O-proj + residual + LayerNorm ----
            res_view = xres.rearrange("(o p) e -> o p e", p=P)
            out_view = out.rearrange("(o p) e -> o p e", p=P)
            with tc.tile_pool(name="dpool", bufs=1) as dpool, \
                 tc.tile_pool(name="resp", bufs=4) as resp, \
                 tc.tile_pool(name="lnp", bufs=4) as lnp, \
                 tc.tile_pool(name="psD", bufs=2, space="PSUM") as psD:
                wo_sb = dpool.tile([P, EC, D], bf16)
                nc.sync.dma_start(wo_sb[:], wo.rearrange("(o p) e -> p o e", p=P))
                gam_sb = dpool.tile([P, D], f32)
                nc.sync.dma_start(gam_sb[:], gam[:])
                bet_sb = dpool.tile([P, D], f32)
                nc.sync.dma_start(bet_sb[:], bet[:])
                ctxT_sb = dpool.tile([P, EC, M], bf16)  # summed ctx^T own q
                nc.gpsimd.collective_compute(
                    "ReduceScatter",
                    mybir.AluOpType.add,
                    replica_groups=rg,
                    ins=[ctx_cc_in.opt()],
                    outs=[ctx_cc_out.opt()],
                )
                ccv = ctx_cc_out.rearrange("(o p) q -> p o q", p=P)
                for vc in range(EC):
                    nc.sync.dma_start(ctxT_sb[:, vc, :], ccv[:, vc, :])
                rests = []
                for qc in range(M // P):
                    rest = resp.tile([P, D], f32, tag="res")
                    nc.sync.dma_start(rest[:], res_view[qc])
                    rests.append(rest)
                for qc in range(M // P):
                    rest = rests[qc]
                    ps = psD.tile([P, D], f32, tag="o")
                    for half in range(2):
                        for vc in range(EC):
                            nc.tensor.matmul(
                                ps[:, half * M : (half + 1) * M],
                                lhsT=ctxT_sb[:, vc, qc * P : (qc + 1) * P],
                                rhs=wo_sb[:, vc, half * M : (half + 1) * M],
                                start=(vc == 0),
                                stop=(vc == EC - 1),
                            )
                    # residual add + stats in column halves so downstream
                    # normalize/output passes pipeline (shorter drain)
                    xsb = lnp.tile([P, D], f32, tag="x")
                    bst = smal.tile([P, 2, 6], f32, tag="bst")
                    for h in range(2):
                        nc.vector.tensor_tensor(
                            xsb[:, h * M : (h + 1) * M],
                            ps[:, h * M : (h + 1) * M],
                            rest[:, h * M : (h + 1) * M],
                            OP.add,
                        )
                        nc.vector.bn_stats(bst[:, h, :], xsb[:, h * M : (h + 1) * M])
                    agg = smal.tile([P, 2], f32, tag="agg")
                    nc.vector.bn_aggr(agg[:], bst[:])
                    veps = smal.tile([P, 1], f32, tag="veps")
                    nc.vector.tensor_scalar(
                        veps[:], agg[:, 1:2], 1.0, LN_EPS, OP.mult, OP.add
                    )
                    rvp = smal.tile([P, 1], f32, tag="rvp")
                    nc.vector.reciprocal_approx_fast(rvp[:], veps[:])
                    inv = smal.tile([P, 1], f32, tag="inv")
                    nc.scalar.activation(inv[:], rvp[:], AF.Sqrt)
                    # xn = (x - mu) * inv ; *gamma on DVE, +beta on Pool;
                    # column halves pipeline into the output DMA
                    ot = lnp.tile([P, D], f32, tag="ot")
                    for h in range(2):
                        sl = slice(h * M, (h + 1) * M)
                        nc.vector.tensor_scalar(
                            xsb[:, sl], xsb[:, sl], agg[:, 0:1], inv[:],
                            OP.subtract, OP.mult,
                        )
                        nc.vector.tensor_tensor(
                            xsb[:, sl], xsb[:, sl], gam_sb[:, sl], OP.mult
                        )
                        nc.gpsimd.tensor_tensor(
                            ot[:, sl], xsb[:, sl], bet_sb[:, sl], OP.add
                        )
                        nc.sync.dma_start(out_view[qc][:, sl], ot[:, sl])

    nc.compile()
    return nc


def _get_nc():
    if "nc" not in _CACHE:
        _CACHE["nc"] = _build()
    return _CACHE["nc"]


def _in_maps(input_Q, input_K, input_V, W_Q, W_K, W_V, W_O, ln_gamma, ln_beta):
    bf = ml_dtypes.bfloat16
    f32 = np.float32
    Q_ = np.asarray(input_Q, dtype=f32)
    K_ = np.asarray(input_K, dtype=f32)
    V_ = np.asarray(input_V, dtype=f32)
    wq_b = np.asarray(W_Q, dtype=f32).astype(bf)
    wk_b = np.asarray(W_K, dtype=f32).astype(bf)
    wv_b = np.asarray(W_V, dtype=f32).astype(bf)
    wo_b = np.asarray(W_O, dtype=f32).astype(bf)
    gam_b = np.ascontiguousarray(
        np.broadcast_to(np.asarray(ln_gamma, dtype=f32), (P, D))
    )
    bet_b = np.ascontiguousarray(
        np.broadcast_to(np.asarray(ln_beta, dtype=f32), (P, D))
    )
    maps = []
    for c in range(8):
        b, r = divmod(c, G)
        sl = slice(r * M, (r + 1) * M)
        maps.append(
            {
                "xqT": np.ascontiguousarray(Q_[b].T).astype(bf),
                "xkT": np.ascontiguousarray(K_[b, sl].T).astype(bf),
                "xvT": np.ascontiguousarray(V_[b, sl].T).astype(bf),
                "xres": np.ascontiguousarray(Q_[b, sl]),
                "wq": wq_b,
                "wk": wk_b,
                "wv": wv_b,
                "wo": wo_b,
                "gam": gam_b,
                "bet": bet_b,
            }
        )
    return maps


def _assemble(results):
    B = 2
    out = np.empty((B, SK, D), np.float32)
    for c in range(8):
        b, r = divmod(c, G)
        out[b, r * M : (r + 1) * M] = results[c]["out"]
    return out


def run_traced(trace=False, **inputs):
    """Run on HW; returns (output, BassKernelResults)."""
    from concourse.bass_utils import run_bass_kernel_spmd

    nc = _get_nc()
    maps = _in_maps(**inputs)
    res = run_bass_kernel_spmd(nc, maps, list(range(8)), trace=trace)
    return _assemble(res.results), res


def kernel(**inputs) -> np.ndarray:
    out, _ = run_traced(trace=False, **inputs)
    return out


# revision 13
# speedup vs baseline: 1.1078x; 1.0023x over previous
"""Trainium2 Bass kernel for nn_MultiHeadAttention (head-axis softmax quirk).

v2 strategy (8 NeuronCores, 2 batch-groups x 4 cores):
  - NO pre-attention collectives. Softmax over HEADS is local per (q,k), so
    attention is sharded over the K/V length: each core projects its own
    512-row k-slice of K and V, and redundantly projects Q for the whole
    group's 2048 q rows (+41us PE, but saves ~240us of AllGather).
  - Per core: scores^T [k,q] tiles (128k x 512q x 16h), exp on ScalarE,
    head-sum via DVE tree-adds (bf16 4x mode), normalize split DVE/GpSimd,
    context accumulated across the 4 local k-chunks in PSUM chains (no
    SBUF accumulation traffic).
  - One bf16 ReduceScatter of ctx^T partials (4MB in -> 1MB out) hands each
    core the k-summed context for its own 512 q rows.
  - Output projection + residual + LayerNorm on own q rows.
  - All matmuls bf16 with fp32 PSUM accumulation.
"""

import numpy as np
import ml_dtypes

D = 1024
H = 16
DK = 64
P = 128
M = 512  # q rows per core (own slice) / k rows per core
SK = 2048
G = 4  # cores per batch group
GQ = 2048  # q rows per group
EC = D // P  # 8
HP = H // 2  # 8
KC = M // P  # 4 local k chunks
QB = GQ // M  # 4 q blocks (one per destination core)
LN_EPS = 1e-5

_CACHE = {}


def _build():
    import concourse.bass as bass
    import concourse.mybir as mybir
    import concourse.tile as tile
    from concourse import bacc

    f32 = mybir.dt.float32
    bf16 = mybir.dt.bfloat16
    AF = mybir.ActivationFunctionType
    OP = mybir.AluOpType
    AX = mybir.AxisListType

    nc = bacc.Bacc("TRN2", target_bir_lowering=False, debug=False, num_devices=8)

    xqT = nc.dram_tensor("xqT", [D, GQ], bf16, kind="ExternalInput").ap()
    xkT = nc.dram_tensor("xkT", [D, M], bf16, kind="ExternalInput").ap()
    xvT = nc.dram_tensor("xvT", [D, M], bf16, kind="ExternalInput").ap()
    xres = nc.dram_tensor("xres", [M, D], f32, kind="ExternalInput").ap()
    wq = nc.dram_tensor("wq", [D, D], bf16, kind="ExternalInput").ap()
    wk = nc.dram_tensor("wk", [D, D], bf16, kind="ExternalInput").ap()
    wv = nc.dram_tensor("wv", [D, D], bf16, kind="ExternalInput").ap()
    wo = nc.dram_tensor("wo", [D, D], bf16, kind="ExternalInput").ap()
    gam = nc.dram_tensor("gam", [P, D], f32, kind="ExternalInput").ap()
    bet = nc.dram_tensor("bet", [P, D], f32, kind="ExternalInput").ap()
    out = nc.dram_tensor("out", [M, D], f32, kind="ExternalOutput").ap()

    rg = [[0, 1, 2, 3], [4, 5, 6, 7]]

    from contextlib import ExitStack

    with tile.TileContext(nc) as tc:
        with ExitStack() as ctx:
            const = ctx.enter_context(tc.tile_pool(name="const", bufs=1))
            smal = ctx.enter_context(tc.tile_pool(name="smal", bufs=4))
            dram = ctx.enter_context(tc.tile_pool(name="dram", bufs=1, space="DRAM"))

            # persistent tiles
            kT_sb = const.tile([P, HP, M], bf16)  # K^T own slice, dk-pair packed
            vt_sb = const.tile([P, KC, D], bf16)  # V own slice [k-chunk, v]
            qT_sb = const.tile([P, HP, GQ], bf16)  # Q^T whole group

            ctx_cc_in = dram.tile([QB * D, M], bf16)
            ctx_cc_out = dram.tile([D, M], bf16)

            # ---- Phase A: K^T, V, and first-quarter Q^T projections ----
            # Q quarters 1-3 are emitted inside Phase C where PE has idle
            # slots (C is Activation-paced); wq/xq pools outlive Phase A.
            wq_p = ctx.enter_context(tc.tile_pool(name="wq_p", bufs=1))
            xq_p = ctx.enter_context(tc.tile_pool(name="xq_p", bufs=2))
            wq_sb = wq_p.tile([P, EC, D], bf16)
            xqv = xqT.rearrange("(o p) q -> p o q", p=P)

            def emit_qproj_qu(qu, pool, tag, evac_dve):
                xq_qu = xq_p.tile([P, EC, M], bf16, tag="xq")
                nc.sync.dma_start(xq_qu[:], xqv[:, :, qu * M : (qu + 1) * M])
                for hp in range(HP):
                    ps = pool.tile([P, M] if tag == "cx" else [P, GQ], f32, tag=tag)
                    for dc in range(EC):
                        nc.tensor.matmul(
                            ps[:, 0:M],
                            lhsT=wq_sb[:, dc, hp * P : (hp + 1) * P],
                            rhs=xq_qu[:, dc, :],
                            start=(dc == 0),
                            stop=(dc == EC - 1),
                        )
                    dst = qT_sb[:, hp, qu * M : (qu + 1) * M]
                    if evac_dve:
                        nc.vector.tensor_copy(dst, ps[:, 0:M])
                    else:
                        nc.scalar.copy(dst, ps[:, 0:M])

            with tc.tile_pool(name="wpool", bufs=2) as wpool, \
                 tc.tile_pool(name="xk_p", bufs=1) as xk_p, \
                 tc.tile_pool(name="xv_p", bufs=1) as xv_p, \
                 tc.tile_pool(name="psA", bufs=2, space="PSUM") as psA:
                # Lead-in DMAs spread across engine DGE queues so wk/xk (the
                # Kproj gate) stream in parallel instead of serializing.
                wk_view = wk.rearrange("(o p) e -> p o e", p=P)
                wk_sb = wpool.tile([P, EC, D], bf16, tag="w")
                nc.sync.dma_start(wk_sb[:, 0:4, :], wk_view[:, 0:4, :])
                nc.scalar.dma_start(wk_sb[:, 4:EC, :], wk_view[:, 4:EC, :])
                xk_sb = xk_p.tile([P, EC, M], bf16)
                nc.gpsimd.dma_start(xk_sb[:], xkT.rearrange("(o p) q -> p o q", p=P))
                wv_sb = wpool.tile([P, EC, D], bf16, tag="w")
                nc.gpsimd.dma_start(wv_sb[:], wv.rearrange("(o p) e -> p o e", p=P))
                xv_sb = xv_p.tile([P, EC, M], bf16)
                nc.sync.dma_start(xv_sb[:], xvT.rearrange("(o p) q -> p o q", p=P))
                nc.sync.dma_start(wq_sb[:], wq.rearrange("(o p) e -> p o e", p=P))

                # K^T projection: own 512 k rows -> kT_sb [P, hp, 512]
                for w0 in range(0, HP, 3):
                    hps = list(range(w0, min(w0 + 3, HP)))
                    n = len(hps)
                    ps = psA.tile([P, GQ], f32, tag="a")
                    for j, hp in enumerate(hps):
                        for dc in range(EC):
                            nc.tensor.matmul(
                                ps[:, j * M : (j + 1) * M],
                                lhsT=wk_sb[:, dc, hp * P : (hp + 1) * P],
                                rhs=xk_sb[:, dc, :],
                                start=(dc == 0),
                                stop=(dc == EC - 1),
                            )
                    nc.scalar.copy(kT_sb[:, w0 : w0 + n, :], ps[:, : n * M])

                # V projection: own 512 k rows -> vt_sb [P(k), kc, 1024(v)]
                for kc in range(KC):
                    ps = psA.tile([P, GQ], f32, tag="a")
                    for half in range(2):
                        for dc in range(EC):
                            nc.tensor.matmul(
                                ps[:, half * M : (half + 1) * M],
                                lhsT=xv_sb[:, dc, kc * P : (kc + 1) * P],
                                rhs=wv_sb[:, dc, half * M : (half + 1) * M],
                                start=(dc == 0),
                                stop=(dc == EC - 1),
                            )
                    nc.scalar.copy(vt_sb[:, kc, :], ps[:, :D])

                # Q^T projection, first quarter (covers q-block 0)
                emit_qproj_qu(0, psA, "a", evac_dve=False)

            # ---- Phase C: attention tiles + ctx PSUM chains + partial out ----
            # Software-pipelined: the 8 ctx chains of q-block qb interleave
            # with the first score/exp tiles of qb+1 so neither PE nor the
            # Activation engine idles across the qb boundary.
            cc_view = ctx_cc_in.rearrange("(b o p) q -> p b o q", b=QB, p=P)
            with tc.tile_pool(name="etp", bufs=6) as etp, \
                 tc.tile_pool(name="t8p", bufs=1) as t8p, \
                 tc.tile_pool(name="stg", bufs=3) as stg, \
                 tc.tile_pool(name="psC", bufs=2, space="PSUM") as psC:

                def emit_tile(qb, kc):
                    et = etp.tile([P, H, M], bf16, tag="et")
                    # scores + exp, 3-head waves; head-sum accumulates wave
                    # partials as exps land so `den` is ready ~1 op after the
                    # last wave (keeps ctx chains from stalling on the tree).
                    par = t8p.tile([P, 8, M], bf16, tag="t8")
                    for wi, w0 in enumerate(range(0, H, 3)):
                        hs = list(range(w0, min(w0 + 3, H)))
                        n = len(hs)
                        ps = psC.tile([P, 3 * M], f32, tag="sc")
                        for j, h in enumerate(hs):
                            hp, half = divmod(h, 2)
                            pb = half * DK
                            nc.tensor.matmul(
                                ps[:, j * M : (j + 1) * M],
                                lhsT=kT_sb[pb : pb + DK, hp, kc * P : (kc + 1) * P],
                                rhs=qT_sb[pb : pb + DK, hp, qb * M : (qb + 1) * M],
                                start=True,
                                stop=True,
                            )
                        nc.scalar.activation(
                            et[:, w0 : w0 + n, :], ps[:, : n * M], AF.Exp,
                            scale=0.125,
                        )
                        if n == 3:
                            nc.vector.tensor_tensor(
                                par[:, wi, :], et[:, w0, :], et[:, w0 + 1, :], OP.add
                            )
                            nc.vector.tensor_tensor(
                                par[:, wi, :], par[:, wi, :], et[:, w0 + 2, :], OP.add
                            )
                        if wi == 1:
                            nc.vector.tensor_tensor(
                                par[:, 6, :], par[:, 0, :], par[:, 1, :], OP.add
                            )
                        if wi == 3:
                            nc.vector.tensor_tensor(
                                par[:, 7, :], par[:, 2, :], par[:, 3, :], OP.add
                            )
                    # waves: 5x3 heads + 1 single head (h15)
                    nc.vector.tensor_tensor(
                        par[:, 6, :], par[:, 6, :], par[:, 7, :], OP.add
                    )
                    nc.vector.tensor_tensor(
                        par[:, 4, :], par[:, 4, :], et[:, 15, :], OP.add
                    )
                    den = smal.tile([P, M], f32, tag="den")
                    nc.vector.tensor_tensor(den[:], par[:, 6, :], par[:, 4, :], OP.add)
                    rf = smal.tile([P, M], f32, tag="rf")
                    nc.vector.reciprocal_approx_fast(rf[:], den[:])
                    rb = smal.tile([P, M], bf16, tag="rb")
                    nc.vector.tensor_copy(rb[:], rf[:])
                    # normalize: attn = e * r (in place; split DVE/GpSimd)
                    nsp = 8
                    for h0 in range(0, nsp, 2):
                        nc.vector.tensor_tensor(
                            et[:, h0 : h0 + 2, :],
                            et[:, h0 : h0 + 2, :],
                            rb[:, None, :].to_broadcast((P, 2, M)),
                            OP.mult,
                        )
                    for h0 in range(nsp, H, 2):
                        nc.gpsimd.tensor_tensor(
                            et[:, h0 : h0 + 2, :],
                            et[:, h0 : h0 + 2, :],
                            rb[:, None, :].to_broadcast((P, 2, M)),
                            OP.mult,
                        )
                    return et

                def emit_chain(qb, hp, ets):
                    cps = psC.tile([P, M], f32, tag="cx")
                    for kc in range(KC):
                        nc.tensor.matmul(
                            cps[0:DK, :],
                            lhsT=vt_sb[:, kc, (2 * hp) * DK : (2 * hp + 1) * DK],
                            rhs=ets[kc][:, 2 * hp, :],
                            start=(kc == 0),
                            stop=(kc == KC - 1),
                            tile_position=(0, 0),
                        )
                        nc.tensor.matmul(
                            cps[DK:P, :],
                            lhsT=vt_sb[:, kc, (2 * hp + 1) * DK : (2 * hp + 2) * DK],
                            rhs=ets[kc][:, 2 * hp + 1, :],
                            start=(kc == 0),
                            stop=(kc == KC - 1),
                            tile_position=(0, DK),
                        )
                    ost = stg.tile([P, M], bf16, tag="ost")
                    if hp % 2 == 0:
                        nc.scalar.copy(ost[:], cps[:])
                    else:
                        nc.vector.tensor_copy(ost[:], cps[:])
                    nc.sync.dma_start(cc_view[:, qb, hp, :], ost[:])

                # Prologue: q-block 0 tiles interleaved with Q^T quarters 1-3
                # (those matmuls fill PE slots while Act runs the exps; their
                # chains borrow the idle "cx" PSUM tag, evacs go to DVE).
                ets_cur = [emit_tile(0, 0)]
                emit_qproj_qu(1, psC, "cx", evac_dve=True)
                ets_cur.append(emit_tile(0, 1))
                emit_qproj_qu(2, psC, "cx", evac_dve=True)
                ets_cur.append(emit_tile(0, 2))
                emit_qproj_qu(3, psC, "cx", evac_dve=True)
                ets_cur.append(emit_tile(0, 3))
                for qb in range(QB):
                    ets_next = []
                    for hp in range(HP):
                        emit_chain(qb, hp, ets_cur)
                        # 3-tile lookahead into qb+1 (etp bufs=7 allows it)
                        if qb + 1 < QB and hp in (2, 4, 6):
                            ets_next.append(emit_tile(qb + 1, len(ets_next)))
                    if qb + 1 < QB:
                        while len(ets_next) < KC:
                            ets_next.append(emit_tile(qb + 1, len(ets_next)))
                    ets_cur = ets_next

            # ---- ReduceScatter + Phase D: O-proj + residual + LayerNorm ----
            res_view = xres.rearrange("(o p) e -> o p e", p=P)
            out_view = out.rearrange("(o p) e -> o p e", p=P)
            with tc.tile_pool(name="dpool", bufs=1) as dpool, \
                 tc.tile_pool(name="resp", bufs=4) as resp, \
                 tc.tile_pool(name="lnp", bufs=4) as lnp, \
                 tc.tile_pool(name="psD", bufs=2, space="PSUM") as psD:
                wo_sb = dpool.tile([P, EC, D], bf16)
                nc.sync.dma_start(wo_sb[:], wo.rearrange("(o p) e -> p o e", p=P))
                gam_sb = dpool.tile([P, D], f32)
                nc.sync.dma_start(gam_sb[:], gam[:])
                bet_sb = dpool.tile([P, D], f32)
                nc.sync.dma_start(bet_sb[:], bet[:])
                ctxT_sb = dpool.tile([P, EC, M], bf16)  # summed ctx^T own q
                nc.gpsimd.collective_compute(
                    "ReduceScatter",
                    mybir.AluOpType.add,
                    replica_groups=rg,
                    ins=[ctx_cc_in.opt()],
                    outs=[ctx_cc_out.opt()],
                )
                ccv = ctx_cc_out.rearrange("(o p) q -> p o q", p=P)
                for vc in range(EC):
                    nc.sync.dma_start(ctxT_sb[:, vc, :], ccv[:, vc, :])
                rests = []
                for qc in range(M // P):
                    rest = resp.tile([P, D], f32, tag="res")
                    nc.sync.dma_start(rest[:], res_view[qc])
                    rests.append(rest)
                for qc in range(M // P):
                    rest = rests[qc]
                    ps = psD.tile([P, D], f32, tag="o")
                    for half in range(2):
                        for vc in range(EC):
                            nc.tensor.matmul(
                                ps[:, half * M : (half + 1) * M],
                                lhsT=ctxT_sb[:, vc, qc * P : (qc + 1) * P],
                                rhs=wo_sb[:, vc, half * M : (half + 1) * M],
                                start=(vc == 0),
                                stop=(vc == EC - 1),
                            )
                    # residual add + stats in column halves so downstream
                    # normalize/output passes pipeline (shorter drain)
                    xsb = lnp.tile([P, D], f32, tag="x")
                    bst = smal.tile([P, 2, 6], f32, tag="bst")
                    for h in range(2):
                        nc.vector.tensor_tensor(
                            xsb[:, h * M : (h + 1) * M],
                            ps[:, h * M : (h + 1) * M],
                            rest[:, h * M : (h + 1) * M],
                            OP.add,
                        )
                        nc.vector.bn_stats(bst[:, h, :], xsb[:, h * M : (h + 1) * M])
                    agg = smal.tile([P, 2], f32, tag="agg")
                    nc.vector.bn_aggr(agg[:], bst[:])
                    veps = smal.tile([P, 1], f32, tag="veps")
                    nc.vector.tensor_scalar(
                        veps[:], agg[:, 1:2], 1.0, LN_EPS, OP.mult, OP.add
                    )
                    rvp = smal.tile([P, 1], f32, tag="rvp")
                    nc.vector.reciprocal_approx_fast(rvp[:], veps[:])
                    inv = smal.tile([P, 1], f32, tag="inv")
                    nc.scalar.activation(inv[:], rvp[:], AF.Sqrt)
                    # xn = (x - mu) * inv ; *gamma on DVE, +beta on Pool;
                    # column halves pipeline into the output DMA
                    ot = lnp.tile([P, D], f32, tag="ot")
                    for h in range(2):
                        sl = slice(h * M, (h + 1) * M)
                        nc.vector.tensor_scalar(
                            xsb[:, sl], xsb[:, sl], agg[:, 0:1], inv[:],
                            OP.subtract, OP.mult,
                        )
                        nc.vector.tensor_tensor(
                            xsb[:, sl], xsb[:, sl], gam_sb[:, sl], OP.mult
                        )
                        nc.gpsimd.tensor_tensor(
                            ot[:, sl], xsb[:, sl], bet_sb[:, sl], OP.add
                        )
                        nc.sync.dma_start(out_view[qc][:, sl], ot[:, sl])

    nc.compile()
    return nc


def _get_nc():
    if "nc" not in _CACHE:
        _CACHE["nc"] = _build()
    return _CACHE["nc"]


def _in_maps(input_Q, input_K, input_V, W_Q, W_K, W_V, W_O, ln_gamma, ln_beta):
    bf = ml_dtypes.bfloat16
    f32 = np.float32
    Q_ = np.asarray(input_Q, dtype=f32)
    K_ = np.asarray(input_K, dtype=f32)
    V_ = np.asarray(input_V, dtype=f32)
    wq_b = np.asarray(W_Q, dtype=f32).astype(bf)
    wk_b = np.asarray(W_K, dtype=f32).astype(bf)
    wv_b = np.asarray(W_V, dtype=f32).astype(bf)
    wo_b = np.asarray(W_O, dtype=f32).astype(bf)
    gam_b = np.ascontiguousarray(
        np.broadcast_to(np.asarray(ln_gamma, dtype=f32), (P, D))
    )
    bet_b = np.ascontiguousarray(
        np.broadcast_to(np.asarray(ln_beta, dtype=f32), (P, D))
    )
    maps = []
    for c in range(8):
        b, r = divmod(c, G)
        sl = slice(r * M, (r + 1) * M)
        maps.append(
            {
                "xqT": np.ascontiguousarray(Q_[b].T).astype(bf),
                "xkT": np.ascontiguousarray(K_[b, sl].T).astype(bf),
                "xvT": np.ascontiguousarray(V_[b, sl].T).astype(bf),
                "xres": np.ascontiguousarray(Q_[b, sl]),
                "wq": wq_b,
                "wk": wk_b,
                "wv": wv_b,
                "wo": wo_b,
                "gam": gam_b,
                "bet": bet_b,
            }
        )
    return maps


def _assemble(results):
    B = 2
    out = np.empty((B, SK, D), np.float32)
    for c in range(8):
        b, r = divmod(c, G)
        out[b, r * M : (r + 1) * M] = results[c]["out"]
    return out


def run_traced(trace=False, **inputs):
    """Run on HW; returns (output, BassKernelResults)."""
    from concourse.bass_utils import run_bass_kernel_spmd

    nc = _get_nc()
    maps = _in_maps(**inputs)
    res = run_bass_kernel_spmd(nc, maps, list(range(8)), trace=trace)
    return _assemble(res.results), res


def kernel(**inputs) -> np.ndarray:
    out, _ = run_traced(trace=False, **inputs)
    return out
